# revision 24
# baseline (speedup 1.0000x reference)
"""Trainium2 Bass kernel for nn_MultiHeadSelfAttention_11158325035343.

GQA multi-head self-attention (B=4, T=2048, E=2048, H=16, HKV=8, HD=128)
with XPos rotary embedding and causal softmax.

Sharding: 8 cores = 4 batches x 2 head-groups. Each core computes, for its
batch b and head-group g (8 q heads, 4 kv heads):
  QT/KT = W.T @ x.T   ([head_dim, T] per head, head_dim on partitions)
  V     = x @ W_v     ([T, head_dim] per kv head)
  XPos rope applied via two host-precomputed fused tables + half-swap
  scoresT[j, i] per (head, i-chunk, j-tile), exp without max subtraction
  (scores are bounded: XPos decay keeps them small), causal mask applied
  post-exp: diagonal j-tiles narrow their score/exp/AV/den work to the
  causally-live columns and a [128,128] triangular 0/1 multiply on GpSimd
  zeroes the within-tile j>i region, softmax denominator via ones-matmul
  on PE over DVE-presummed prob pairs, AV/den matmuls deferred two steps
  behind the scores so the PE always has independent work while ACT runs
  exp, attnT = V.T-contraction with probs as moving operand, normalized
  by the broadcast reciprocal denominator (fin chain staged across chunk
  boundaries), partial out = attnT.T @ W_o rows-for-this-group, written
  as bf16 partials.
Host sums the two group partials per batch in f32.

Scheduling structure:
  - PE warmed up via memset-ones matmuls at t=0 (HAM clock-gate release)
  - x/W_v/W_o staged via single large prearranged DMAs (sequencer issue
    cost is ~600ns per dma_start, so fewer+bigger wins)
  - phase 2 runs chunk-pair-major (all heads' token pair 0 first): after
    ~26% of attention work the first half of phase 3 unlocks, giving the
    scheduler PE filler for the ACT-bound remainder of attention
"""

import sys
import types

sys.path.insert(0, "/opt/trn_rl_repo")

import numpy as np
import ml_dtypes

BF16 = ml_dtypes.bfloat16

# ---------------------------------------------------------------------------
# NTFF profile hook injection (missing antenv.axon_hooks in this image).
# Needed only when trace=True; harmless otherwise.
# ---------------------------------------------------------------------------
def _ensure_axon_hooks():
    if "antenv.axon_hooks" in sys.modules:
        return
    try:
        import antenv
        mod = types.ModuleType("antenv.axon_hooks")
        holder = {"hook": None}
        mod.set_axon_ntff_profile_hook = lambda h: holder.__setitem__("hook", h)
        mod.get_axon_ntff_profile_hook = lambda: holder["hook"]
        sys.modules["antenv.axon_hooks"] = mod
        antenv.axon_hooks = mod
        from trn_agent_boot.trn_boot import _ntff_profile_via_ctypes
        mod.set_axon_ntff_profile_hook(
            _ntff_profile_via_ctypes("/opt/axon/libaxon_pjrt.so")
        )
    except Exception:
        pass


_ensure_axon_hooks()

import concourse.bass as bass
import concourse.bacc as bacc
import concourse.mybir as mybir
import concourse.tile as tile
from concourse.bass_utils import run_bass_kernel_spmd

# Problem constants (hardcoded per spec).
B, T, E = 4, 2048, 2048
H, HKV, HD = 16, 8, 128
THETA, SCALE_BASE = 10000.0, 512.0
G = 2                   # head groups (cores per batch)
HL = H // G             # 8 local q heads
KVL = HKV // G          # 4 local kv heads
REP = H // HKV          # GQA repeat
CH = 512                # i-chunk / matmul free dim
NE = E // 128           # 16 contraction tiles
NF = HL + KVL           # 12 projection f-tiles (8 Q + 4 K)
HALFT = T // 2          # token half for phase-1 SBUF staging
NJT = T // 128          # 16 j tiles
NCH = T // CH           # 4 i chunks
INV_SQRT_D = 1.0 / float(np.sqrt(np.float32(HD)))

F32 = mybir.dt.float32
F16 = mybir.dt.float16
BF = mybir.dt.bfloat16

_COMPILED = None


def _build_nc():
    nc = bacc.Bacc("TRN2", target_bir_lowering=False, debug=False, num_devices=8)

    # xt prearranged on host: [half, p, e, chunk, i]
    xt_d = nc.dram_tensor("xt", [2, 128, NE, 2, CH], BF, kind="ExternalInput")
    wqk_d = nc.dram_tensor("wqk", [NF, 128, NE, 128], BF, kind="ExternalInput")
    # wv prearranged: [p, e, f]
    wv_d = nc.dram_tensor("wv", [128, NE, KVL * HD], BF, kind="ExternalInput")
    # wo prearranged: [Ehalf, p, fb*2+e2, i]
    wo_d = nc.dram_tensor("wo", [2, 128, NE, CH], BF, kind="ExternalInput")
    aq_d = nc.dram_tensor("aq", [HD, T], BF, kind="ExternalInput")
    bq_d = nc.dram_tensor("bq", [HD, T], BF, kind="ExternalInput")
    ak_d = nc.dram_tensor("ak", [HD, T], BF, kind="ExternalInput")
    bk_d = nc.dram_tensor("bk", [HD, T], BF, kind="ExternalInput")
    tri_d = nc.dram_tensor("tri", [128, 128], BF, kind="ExternalInput")
    ident_d = nc.dram_tensor("ident", [128, 128], BF, kind="ExternalInput")
    out_d = nc.dram_tensor("out_p", [T, E], BF, kind="ExternalOutput")

    with tile.TileContext(nc) as tc:
        with (
            tc.tile_pool(name="big", bufs=2) as pool_big,       # xt chunks / wo
            tc.tile_pool(name="qk", bufs=NF) as pool_qk,        # rope'd QT/KT bf16
            tc.tile_pool(name="v", bufs=NJT) as pool_v,         # V bf16
            tc.tile_pool(name="at", bufs=HL) as pool_at,        # attnT bf16
            tc.tile_pool(name="tab", bufs=4) as pool_tab,       # rope tables
            tc.tile_pool(name="wv", bufs=1) as pool_wv,         # resident W_v
            tc.tile_pool(name="w", bufs=3) as pool_w,           # streamed W_q/W_k
            tc.tile_pool(name="tmp", bufs=2) as pool_tmp,       # rope temp
            tc.tile_pool(name="p", bufs=4) as pool_p,           # exp probs bf16
            tc.tile_pool(name="sp", bufs=3) as pool_sp,         # den pair presums
            tc.tile_pool(name="o", bufs=2) as pool_o,           # out staging
            tc.tile_pool(name="sm", bufs=1) as pool_sm,         # small constants
            tc.tile_pool(name="dv", bufs=4) as pool_dv,         # recip denominators
            tc.tile_pool(name="dvr", bufs=4, space="DRAM") as pool_dvr,  # dinv DRAM bounce
            tc.tile_pool(name="bch", bufs=3) as pool_bch,       # dinv bcast per chunk
            tc.tile_pool(name="ps", bufs=2, space=bass.MemorySpace.PSUM) as pool_ps,
        ):
            # ---- PE warmup: all-ones tile via memset (no DMA dependency)
            # so the HAM clock gate releases at ~3.4us, before the input
            # DMA wave completes. The same tile serves the den ones-matmul.
            ones_t = pool_sm.tile([128, CH], BF, tag="oc", name="ones_t")
            nc.vector.memset(ones_t[:], 1.0)
            warm_ps = pool_ps.tile([128, CH], F32, tag="psden", bufs=1,
                                   name="warm_ps")
            for _wi in range(40):
                nc.tensor.matmul(warm_ps[:], ones_t[:, 0:128], ones_t[:],
                                 start=True, stop=True)

            # tiny mask constants on the gpsimd queue (needed in phase 2)
            tri_t = pool_sm.tile([128, 128], BF, tag="tri", name="tri_t")
            nc.gpsimd.dma_start(tri_t[:], tri_d[:])
            ident_t = pool_sm.tile([128, 128], BF, tag="id", name="ident_t")
            nc.gpsimd.dma_start(ident_t[:], ident_d[:])

            # ---- persistent activation tensors ----
            qk_t = [pool_qk.tile([128, T], BF, tag="qk", name=f"qk{i}") for i in range(NF)]
            v_t = [pool_v.tile([128, KVL * HD], BF, tag="v", name=f"v{i}") for i in range(NJT)]
            at_t = [pool_at.tile([128, T], BF, tag="at", name=f"at{i}") for i in range(HL)]

            # ================= Phase 1: QKV projections + rope =============
            # DMA priority on the sync queue: w0, then the chunk-0 x wave
            # (exactly what the first f-tile's first matmuls need), then
            # chunk 1, then rope tables (K first - K heads rope first),
            # then W_v. x/W_v go as single large prearranged transfers to
            # keep the sequencer issue count low.
            for half in range(2):
                hs = half * HALFT
                wq_pre = []

                def w_prefetch(f, half=half):
                    w = pool_w.tile([128, NE, 128], BF, tag="w",
                                    name=f"w_pre{half}_{f}")
                    nc.sync.dma_start(w[:], wqk_d[f])
                    wq_pre.append(w)

                forder = list(range(HL, NF)) + list(range(HL))
                w_prefetch(forder[0])
                xt_t = []
                for cc in range(2):
                    xx = pool_big.tile([128, NE, CH], BF, tag="big",
                                       name=f"xt{half}_{cc}")
                    nc.sync.dma_start(xx[:], xt_d[half, :, :, cc, :])
                    xt_t.append(xx)
                w_prefetch(forder[1])
                w_prefetch(forder[2])
                if half == 0:
                    ak_t = pool_tab.tile([HD, T], BF, tag="tab", name="ak_t")
                    nc.sync.dma_start(ak_t[:], ak_d[:])
                    bk_t = pool_tab.tile([HD, T], BF, tag="tab", name="bk_t")
                    nc.sync.dma_start(bk_t[:], bk_d[:])
                    aq_t = pool_tab.tile([HD, T], BF, tag="tab", name="aq_t")
                    nc.sync.dma_start(aq_t[:], aq_d[:])
                    bq_t = pool_tab.tile([HD, T], BF, tag="tab", name="bq_t")
                    nc.sync.dma_start(bq_t[:], bq_d[:])
                    wv_t = pool_wv.tile([128, NE, KVL * HD], BF, tag="wv",
                                        name="wv_t")
                    nc.sync.dma_start(wv_t[:], wv_d[:])

                for fi, f in enumerate(forder):
                    # host-prearranged W column block, contiguous per partition
                    w_t = wq_pre.pop(0)
                    if fi + 3 < NF:
                        w_prefetch(forder[fi + 3])
                    for c in range(HALFT // CH):
                        ps = pool_ps.tile([128, CH], F32, tag="psacc", bufs=2)
                        for e in range(NE):
                            nc.tensor.matmul(
                                ps[:],
                                w_t[:, e, :],
                                xt_t[c][:, e, :],
                                start=(e == 0),
                                stop=(e == NE - 1),
                            )
                        nc.vector.tensor_copy(
                            qk_t[f][:, hs + c * CH: hs + (c + 1) * CH], ps[:]
                        )
                    # rope over this token half
                    A_t, B_t = (aq_t, bq_t) if f < HL else (ak_t, bk_t)
                    q = qk_t[f]
                    sl = slice(hs, hs + HALFT)
                    qs = pool_tmp.tile([128, HALFT], BF, tag="qs")
                    nc.sync.dma_start(qs[0:64, :], q[64:128, sl])
                    nc.sync.dma_start(qs[64:128, :], q[0:64, sl])
                    nc.vector.tensor_mul(qs[:, :], qs[:, :], B_t[:, sl])
                    nc.vector.tensor_mul(q[:, sl], q[:, sl], A_t[:, sl])
                    nc.vector.tensor_add(q[:, sl], q[:, sl], qs[:])

                for tt in range(NJT // 2):
                    tglob = half * (NJT // 2) + tt
                    cc, co = tt // 4, (tt % 4) * 128
                    psv = pool_ps.tile([128, KVL * HD], F32, tag="psacc", bufs=2)
                    for e in range(NE):
                        nc.tensor.matmul(
                            psv[:],
                            xt_t[cc][:, e, co:co + 128],
                            wv_t[:, e, :],
                            start=(e == 0),
                            stop=(e == NE - 1),
                        )
                    nc.vector.tensor_copy(v_t[tglob][:], psv[:])

            # W_o loads reuse the xt big-tile ring (freed after phase 1):
            # two [128, NE, CH] tiles, mapping [p, fb*2+e2, i] so phase 3's
            # (fb, ec) slice is wo_t[ec//2][:, fb*2 + ec%2, :]
            wo_t = []
            for eh in range(2):
                w = pool_big.tile([128, NE, CH], BF, tag="big",
                                  name=f"wo{eh}")
                nc.sync.dma_start(w[:], wo_d[eh])
                wo_t.append(w)

            # ================= Phase 2: attention ==========================
            # Chunk-pair-major over heads: all heads' token pair 0 (chunks
            # 0-1) first, then pair 1 (chunks 2-3). Tokens 0-1023 of every
            # head finish after the first sweep, unblocking the first half
            # of phase 3 as PE filler while the rest of attention (which is
            # ACT-exp-bound per chunk) runs. AV/den matmuls of each j-block
            # are deferred TWO steps so the PE always has independent work
            # while ACT runs exp. Per-chunk normalization chains are staged
            # one boundary later per stage.
            pend_q = []     # deferred AV/den emitters, one list per j-block
            fin_chains = []  # normalization chains, one stage/boundary
            den2_map = {}

            def make_fin_a(hl, c, acc, den, den2):
                # per-chunk psum evacuation: acc -> at_t, den row -> its half
                # of the pair's den2 buffer (x 1/4096 for the fp16 recip)
                def stage_a():
                    nc.vector.tensor_copy(
                        at_t[hl][:, c * CH:(c + 1) * CH], acc[:]
                    )
                    nc.vector.tensor_scalar_mul(
                        den2[0:1, (c % 2) * CH:(c % 2 + 1) * CH],
                        den[0:1, :], 1.0 / 4096.0,
                    )
                return [stage_a]

            def make_fin_bc(hl, cp, den2):
                # per chunk-PAIR: reciprocal + broadcast + normalize over a
                # [128, 2*CH] region; half the DMA-descriptor bursts of the
                # per-chunk variant
                state = {}

                def stage_b():
                    d32 = pool_dv.tile([32, 2 * CH // 32], F32, tag="d32",
                                       bufs=2, name=f"d32_{hl}_{cp}")
                    nc.sync.dma_start(d32[:], den2[:])
                    dr = pool_dv.tile([32, 2 * CH // 32], F16, tag="dr",
                                      bufs=2, name=f"dr{hl}_{cp}")
                    with nc.allow_low_precision(reason="fp16 dinv; x4096 scaling keeps it normal"):
                        nc.vector.reciprocal(dr[:], d32[:])
                    dd_t = pool_dvr.tile([1, 2 * CH], F16, tag="dvrow",
                                         name=f"dinv_dram{hl}_{cp}")
                    nc.sync.dma_start(dd_t[:], dr[:])
                    bch = pool_bch.tile([128, 2 * CH], F16, tag="bch", bufs=2,
                                        name=f"bch{hl}_{cp}")
                    nc.sync.dma_start(bch[:], dd_t[:].to_broadcast((128, 2 * CH)))
                    state["bch"] = bch

                def stage_c():
                    nc.gpsimd.tensor_mul(
                        at_t[hl][:, cp * 2 * CH:(cp + 1) * 2 * CH],
                        at_t[hl][:, cp * 2 * CH:(cp + 1) * 2 * CH],
                        state["bch"][:],
                    )

                return [stage_b, stage_c]

            def fin_boundary():
                for chain in fin_chains:
                    chain.pop(0)()
                fin_chains[:] = [ch for ch in fin_chains if ch]

            # Phase-3 output-projection emitters. The first token half
            # (it < 8) only needs the cp0 attention sweep, so those groups
            # are emitted INTO the cp1 sweep's PE stream (the PE queue runs
            # in emission order - work emitted later cannot fill earlier
            # stalls). Interleaved groups use their own 1-bank psum tag so
            # they never WAR against the still-accumulating attention psum.
            os_map = {}

            def p3_group(it, eh, e2, tag, bufs):
                def emit():
                    key = (it, eh)
                    if key not in os_map:
                        os_map[key] = pool_o.tile(
                            [128, E // 2], BF, tag="o", bufs=2,
                            name=f"os{it}_{eh}")
                    os_t = os_map[key]
                    po = pool_ps.tile([128, CH], F32, tag=tag, bufs=bufs,
                                      name=f"po{it}_{eh}_{e2}")
                    for fb in range(HL):
                        nc.tensor.matmul(
                            po[:],
                            at_t[fb][:, it * 128:(it + 1) * 128],
                            wo_t[eh][:, fb * 2 + e2, :],
                            start=(fb == 0),
                            stop=(fb == HL - 1),
                        )
                    nc.vector.tensor_copy(
                        os_t[:, e2 * CH:(e2 + 1) * CH], po[:]
                    )
                    if e2 == 1:
                        nc.sync.dma_start(
                            out_d[it * 128:(it + 1) * 128,
                                  eh * HALFT:(eh + 1) * HALFT],
                            os_t[:],
                        )
                return emit

            p3_queue = [(it, eh, e2)
                        for it in range(T // 256)
                        for eh in range(2)
                        for e2 in range(2)]
            p3_budget = [24]   # interleaved groups; 8 reserved for the tail

            def p3_slot():
                if p3_budget[0] > 0 and p3_queue:
                    p3_budget[0] -= 1
                    p3_group(*p3_queue.pop(0), tag="pso", bufs=1)()

            chunk_order = [(hl, cp * 2 + ci)
                           for cp in range(NCH // 2)
                           for hl in range(HL)
                           for ci in range(2)]
            for ci_idx, (hl, c) in enumerate(chunk_order):
                kf = HL + hl // REP
                kvc = (hl // REP) * HD
                njt = (c + 1) * (CH // 128)
                if c % 2 == 0:
                    den2 = pool_dv.tile([1, 2 * CH], F32, tag="den2",
                                        bufs=2, name=f"den2_{hl}_{c // 2}")
                    den2_map[hl] = den2
                else:
                    den2 = den2_map[hl]
                acc = pool_ps.tile([128, CH], F32, tag="psacc", bufs=2,
                                   name=f"acc{hl}_{c}")
                den = pool_ps.tile([128, CH], F32, tag="psden", bufs=1,
                                   name=f"den{hl}_{c}")
                j0_order = list(range(0, njt, 2))
                start_jt = 0
                stop_jt = njt - 1
                for step, j0 in enumerate(j0_order):
                    # causally-live column start per j-tile: diagonal
                    # tiles (d >= 0) only need cols [128*d, 512)
                    i0s = []
                    for u in range(2):
                        d = (j0 + u) - (njt - 4)
                        i0s.append(128 * d if d > 0 else 0)
                    diag = (j0 >= njt - 4)
                    s2 = pool_ps.tile([128, 2, CH], F32, tag="ps", bufs=2,
                                      name=f"s2_{hl}_{c}_{j0}")
                    for u in range(2):
                        jt = j0 + u
                        masked = (jt >= njt - 4)
                        nc.tensor.matmul(
                            s2[:, u, i0s[u]:],
                            qk_t[kf][:, jt * 128:(jt + 1) * 128],
                            qk_t[hl][:, c * CH + i0s[u]:(c + 1) * CH],
                            start=True,
                            stop=not masked,
                        )
                        if masked:
                            nc.tensor.matmul(
                                s2[:, u, i0s[u]:i0s[u] + 128],
                                ident_t[:],
                                tri_t[:],
                                start=False,
                                stop=True,
                            )
                        p2 = pool_p.tile([128, 2, CH], BF, tag="p", bufs=4,
                                     name=f"p2_{hl}_{c}_{j0}")
                    # one activation per step; for diagonal pairs the
                    # region [i0s[0], CH) covers both u-slices (u=1's
                    # cols [i0s[0], i0s[1]) hold unread garbage)
                    nc.scalar.activation(
                        p2[:, :, i0s[0]:], s2[:, :, i0s[0]:],
                        mybir.ActivationFunctionType.Exp,
                        scale=INV_SQRT_D,
                    )
                    psum2 = None
                    quad = None
                    if diag:
                        pass    # causal mask already folded into the scores
                    else:
                        # pre-sum the probs pair on DVE, then merge step
                        # pairs into quads so the den ones-matmul streams a
                        # quarter of the rows
                        psum2 = pool_sp.tile([128, CH], BF, tag="sp",
                                             name=f"sp{hl}_{c}_{j0}")
                        nc.vector.tensor_add(
                            psum2[:], p2[:, 0, :], p2[:, 1, :]
                        )
                        if step % 2 == 0:
                            prev_psum2 = psum2
                        else:
                            quad = pool_sp.tile([128, CH], BF, tag="qd",
                                                bufs=2,
                                                name=f"qd{hl}_{c}_{j0}")
                            nc.vector.tensor_add(
                                quad[:], prev_psum2[:], psum2[:]
                            )
                    if len(pend_q) >= 3:
                        for fn in pend_q.pop(0):
                            fn()
                    if step == 2:
                        fin_boundary()
                    # feed first-half output-projection groups into the cp1
                    # sweep (at_t tokens 0-1023 are final for all heads two
                    # boundaries into the sweep)
                    if ci_idx >= 18 and step == 3:
                        p3_slot()
                    step_fns = []
                    for u in range(2):
                        jt = j0 + u
                        def av(jt=jt, p2=p2, u=u, acc=acc, kvc=kvc,
                               i0=i0s[u], sjt=start_jt, pjt=stop_jt):
                            nc.tensor.matmul(
                                acc[:, i0:],
                                v_t[jt][:, kvc:kvc + HD],
                                p2[:, u, i0:],
                                start=(jt == sjt),
                                stop=(jt == pjt),
                            )
                        step_fns.append(av)
                    if not diag:
                        if quad is not None:
                            def den_quad(quad=quad, den=den,
                                         first=(step == 1)):
                                nc.tensor.matmul(
                                    den[:],
                                    ones_t[:, 0:128],
                                    quad[:],
                                    start=first,
                                    stop=False,
                                )
                            step_fns.append(den_quad)
                    else:
                        for u in range(2):
                            jt = j0 + u
                            def den_u(jt=jt, p2=p2, u=u, den=den,
                                      i0=i0s[u],
                                      sjt=start_jt, pjt=stop_jt):
                                nc.tensor.matmul(
                                    den[:, i0:],
                                    ones_t[:, 0:128],
                                    p2[:, u, i0:],
                                    start=(jt == sjt),
                                    stop=(jt == pjt),
                                )
                            step_fns.append(den_u)
                    pend_q.append(step_fns)
                fin_chains.append(make_fin_a(hl, c, acc, den, den2))
                if c % 2 == 1:
                    fin_chains.append(make_fin_bc(hl, c // 2, den2))
                if ci_idx >= 17:
                    p3_slot()
            while pend_q:
                for fn in pend_q.pop(0):
                    fn()
                # ready first-half output tiles keep the PE fed while the
                # final AV/den/normalize chains drain
                if p3_queue:
                    p3_group(*p3_queue.pop(0), tag="pso", bufs=1)()
            while fin_chains:
                fin_boundary()
                if p3_queue:
                    p3_group(*p3_queue.pop(0), tag="psacc", bufs=2)()

            # ================= Phase 3 tail: remaining output tiles ========
            while p3_queue:
                p3_group(*p3_queue.pop(0), tag="psacc", bufs=2)()
            for it in range(T // 256, T // 128):
                for eh in range(2):
                    for e2 in range(2):
                        p3_group(it, eh, e2, tag="psacc", bufs=2)()

    nc.compile()
    return nc


def _get_compiled():
    global _COMPILED
    if _COMPILED is None:
        _COMPILED = _build_nc()
    return _COMPILED


def _host_tables():
    half = np.arange(0, HD, 2, dtype=np.float64)
    inv_freq = 1.0 / (THETA ** (half / HD))
    t_idx = np.arange(T, dtype=np.float64)
    freqs = np.outer(t_idx, inv_freq)
    emb = np.concatenate([freqs, freqs], axis=-1)
    cos, sin = np.cos(emb), np.sin(emb)
    scale_vec = (half + 0.4 * HD) / (1.4 * HD)
    power = (t_idx - T // 2) / SCALE_BASE
    scale = scale_vec[None, :] ** power[:, None]
    scale = np.concatenate([scale, scale], axis=-1)
    sgn = np.where(np.arange(HD) < HD // 2, -1.0, 1.0)
    aq = (scale * cos).T
    bq = sgn[:, None] * (scale * sin).T
    ak = (cos / scale).T
    bk = sgn[:, None] * (sin / scale).T

    # within-tile causal mask, additive: -1e9 where j > i (applied to the
    # scores via an identity-stationary matmul before exp)
    dj = np.arange(128)[:, None]
    r = np.arange(128)[None, :]
    tri = np.where(dj > r, -1e9, 0.0)
    ident = np.eye(128)
    return (
        aq.astype(BF16), bq.astype(BF16), ak.astype(BF16), bk.astype(BF16),
        tri.astype(BF16), ident.astype(BF16),
    )


def _arrange_wqk(wq, wk):
    # [E, F] -> per 128-wide f-block: [128(part=e%128), NE(e//128), 128(f)]
    w = np.concatenate([wq, wk], axis=1)          # [E, NF*128]
    nf = w.shape[1] // 128
    w = w.reshape(NE, 128, nf, 128)               # [n, p, f_blk, fc]
    w = w.transpose(2, 1, 0, 3)                   # [f_blk, p, n, fc]
    return np.ascontiguousarray(w).astype(BF16)


def _arrange_xt(xt):
    # [E, T] -> [half, p, e, chunk, i]
    w = xt.reshape(NE, 128, 2, 2, CH)             # [e, p, half, cc, i]
    w = w.transpose(2, 1, 0, 3, 4)                # [half, p, e, cc, i]
    return np.ascontiguousarray(w).astype(BF16)


def _arrange_wv(wv):
    # [E, KVL*HD] -> [p, e, f]
    w = wv.reshape(NE, 128, KVL * HD)
    w = w.transpose(1, 0, 2)
    return np.ascontiguousarray(w).astype(BF16)


def _arrange_wo(wo):
    # [HL*HD, E] -> [Ehalf, p, fb*2+e2, i] so (fb, ec) slice is
    # [eh=ec//2][:, fb*2 + ec%2, :]
    w = wo.reshape(HL, 128, 2, 2, CH)             # [fb, p, eh, e2, i]
    w = w.transpose(2, 1, 0, 3, 4)                # [eh, p, fb, e2, i]
    w = w.reshape(2, 128, NE, CH)
    return np.ascontiguousarray(w).astype(BF16)


def _make_in_maps(x, W_q, W_k, W_v, W_o):
    aq, bq, ak, bk, tri, ident = _host_tables()
    xts = [_arrange_xt(np.ascontiguousarray(x[b].T)) for b in range(B)]
    in_maps = []
    for core in range(8):
        b, g = core // G, core % G
        in_maps.append({
            "xt": xts[b],
            "wqk": _arrange_wqk(W_q[:, g * HL * HD:(g + 1) * HL * HD],
                                W_k[:, g * KVL * HD:(g + 1) * KVL * HD]),
            "wv": _arrange_wv(W_v[:, g * KVL * HD:(g + 1) * KVL * HD]),
            "wo": _arrange_wo(W_o[g * HL * HD:(g + 1) * HL * HD, :] / 4096.0),
            "aq": aq, "bq": bq, "ak": ak, "bk": bk,
            "tri": tri,
            "ident": ident,
        })
    return in_maps


def _run(x, W_q, W_k, W_v, W_o, trace=False):
    nc = _get_compiled()
    in_maps = _make_in_maps(x, W_q, W_k, W_v, W_o)
    res = run_bass_kernel_spmd(nc, in_maps, list(range(8)), trace=trace)
    out = np.empty((B, T, E), np.float32)
    for b in range(B):
        out[b] = (res.results[2 * b]["out_p"].astype(np.float32)
                  + res.results[2 * b + 1]["out_p"].astype(np.float32))
    return out, res.exec_time_ns


def kernel(x, W_q, W_k, W_v, W_o):
    out, _ = _run(
        np.asarray(x), np.asarray(W_q), np.asarray(W_k),
        np.asarray(W_v), np.asarray(W_o),
    )
    return out


# revision 25
# speedup vs baseline: 1.1746x; 1.1746x over previous
"""Trainium2 Bass kernel for nn_MultiHeadSelfAttention_11158325035343.

GQA multi-head self-attention (B=4, T=2048, E=2048, H=16, HKV=8, HD=128)
with XPos rotary embedding and causal softmax.

Sharding: 8 cores = 4 batches x 2 head-groups. Each core computes, for its
batch b and head-group g (8 q heads, 4 kv heads):
  QT/KT = W.T @ x.T   ([head_dim, T] per head, head_dim on partitions)
  V     = x @ W_v     ([T, head_dim] per kv head)
  XPos rope applied via two host-precomputed fused tables + half-swap
  scoresT[j, i] per (head, i-chunk, j-tile), exp without max subtraction
  (scores are bounded: XPos decay keeps them small), causal mask applied
  post-exp: diagonal j-tiles narrow their score/exp/AV/den work to the
  causally-live columns and a [128,128] triangular 0/1 multiply on GpSimd
  zeroes the within-tile j>i region, softmax denominator via ones-matmul
  on PE over DVE-presummed prob pairs, AV/den matmuls deferred two steps
  behind the scores so the PE always has independent work while ACT runs
  exp, attnT = V.T-contraction with probs as moving operand, normalized
  by the broadcast reciprocal denominator (fin chain staged across chunk
  boundaries), partial out = attnT.T @ W_o rows-for-this-group, written
  as bf16 partials.
Host sums the two group partials per batch in f32.

Scheduling structure:
  - PE warmed up via memset-ones matmuls at t=0 (HAM clock-gate release)
  - x/W_v/W_o staged via single large prearranged DMAs (sequencer issue
    cost is ~600ns per dma_start, so fewer+bigger wins)
  - phase 2 runs chunk-pair-major (all heads' token pair 0 first): after
    ~26% of attention work the first half of phase 3 unlocks, giving the
    scheduler PE filler for the ACT-bound remainder of attention
"""

import sys
import types

sys.path.insert(0, "/opt/trn_rl_repo")

import numpy as np
import ml_dtypes

BF16 = ml_dtypes.bfloat16

# ---------------------------------------------------------------------------
# NTFF profile hook injection (missing antenv.axon_hooks in this image).
# Needed only when trace=True; harmless otherwise.
# ---------------------------------------------------------------------------
def _ensure_axon_hooks():
    if "antenv.axon_hooks" in sys.modules:
        return
    try:
        import antenv
        mod = types.ModuleType("antenv.axon_hooks")
        holder = {"hook": None}
        mod.set_axon_ntff_profile_hook = lambda h: holder.__setitem__("hook", h)
        mod.get_axon_ntff_profile_hook = lambda: holder["hook"]
        sys.modules["antenv.axon_hooks"] = mod
        antenv.axon_hooks = mod
        from trn_agent_boot.trn_boot import _ntff_profile_via_ctypes
        mod.set_axon_ntff_profile_hook(
            _ntff_profile_via_ctypes("/opt/axon/libaxon_pjrt.so")
        )
    except Exception:
        pass


_ensure_axon_hooks()

import concourse.bass as bass
import concourse.bacc as bacc
import concourse.mybir as mybir
import concourse.tile as tile
from concourse.bass_utils import run_bass_kernel_spmd

# Problem constants (hardcoded per spec).
B, T, E = 4, 2048, 2048
H, HKV, HD = 16, 8, 128
THETA, SCALE_BASE = 10000.0, 512.0
G = 2                   # head groups (cores per batch)
HL = H // G             # 8 local q heads
KVL = HKV // G          # 4 local kv heads
REP = H // HKV          # GQA repeat
CH = 512                # i-chunk / matmul free dim
NE = E // 128           # 16 contraction tiles
NF = HL + KVL           # 12 projection f-tiles (8 Q + 4 K)
HALFT = T // 2          # token half for phase-1 SBUF staging
NJT = T // 128          # 16 j tiles
NCH = T // CH           # 4 i chunks
INV_SQRT_D = 1.0 / float(np.sqrt(np.float32(HD)))

F32 = mybir.dt.float32
F16 = mybir.dt.float16
BF = mybir.dt.bfloat16

_COMPILED = None


def _build_nc():
    nc = bacc.Bacc("TRN2", target_bir_lowering=False, debug=False, num_devices=8)

    # xt prearranged on host: [half, p, e, chunk, i]
    xt_d = nc.dram_tensor("xt", [2, 128, NE, 2, CH], BF, kind="ExternalInput")
    wqk_d = nc.dram_tensor("wqk", [NF, 128, NE, 128], BF, kind="ExternalInput")
    # wv prearranged: [p, e, f]
    wv_d = nc.dram_tensor("wv", [128, NE, KVL * HD], BF, kind="ExternalInput")
    # wo prearranged: [Ehalf, p, fb*2+e2, i]
    wo_d = nc.dram_tensor("wo", [2, 128, NE, CH], BF, kind="ExternalInput")
    aq_d = nc.dram_tensor("aq", [HD, T], BF, kind="ExternalInput")
    bq_d = nc.dram_tensor("bq", [HD, T], BF, kind="ExternalInput")
    ak_d = nc.dram_tensor("ak", [HD, T], BF, kind="ExternalInput")
    bk_d = nc.dram_tensor("bk", [HD, T], BF, kind="ExternalInput")
    tri_d = nc.dram_tensor("tri", [128, 128], BF, kind="ExternalInput")
    ident_d = nc.dram_tensor("ident", [128, 128], BF, kind="ExternalInput")
    out_d = nc.dram_tensor("out_p", [T, E], BF, kind="ExternalOutput")

    with tile.TileContext(nc) as tc:
        with (
            tc.tile_pool(name="big", bufs=2) as pool_big,       # xt chunks / wo
            tc.tile_pool(name="qk", bufs=NF) as pool_qk,        # rope'd QT/KT bf16
            tc.tile_pool(name="v", bufs=NJT) as pool_v,         # V bf16
            tc.tile_pool(name="at", bufs=HL) as pool_at,        # attnT bf16
            tc.tile_pool(name="tab", bufs=4) as pool_tab,       # rope tables
            tc.tile_pool(name="wv", bufs=1) as pool_wv,         # resident W_v
            tc.tile_pool(name="w", bufs=3) as pool_w,           # streamed W_q/W_k
            tc.tile_pool(name="tmp", bufs=2) as pool_tmp,       # rope temp
            tc.tile_pool(name="p", bufs=4) as pool_p,           # exp probs bf16
            tc.tile_pool(name="sp", bufs=3) as pool_sp,         # den pair presums
            tc.tile_pool(name="o", bufs=2) as pool_o,           # out staging
            tc.tile_pool(name="sm", bufs=1) as pool_sm,         # small constants
            tc.tile_pool(name="dv", bufs=4) as pool_dv,         # recip denominators
            tc.tile_pool(name="dvr", bufs=4, space="DRAM") as pool_dvr,  # dinv DRAM bounce
            tc.tile_pool(name="bch", bufs=3) as pool_bch,       # dinv bcast per chunk
            tc.tile_pool(name="ps", bufs=2, space=bass.MemorySpace.PSUM) as pool_ps,
        ):
            # ---- PE warmup: all-ones tile via memset (no DMA dependency)
            # so the HAM clock gate releases at ~3.4us, before the input
            # DMA wave completes. The same tile serves the den ones-matmul.
            ones_t = pool_sm.tile([128, CH], BF, tag="oc", name="ones_t")
            nc.vector.memset(ones_t[:], 1.0)
            warm_ps = pool_ps.tile([128, CH], F32, tag="psden", bufs=1,
                                   name="warm_ps")
            for _wi in range(40):
                nc.tensor.matmul(warm_ps[:], ones_t[:, 0:128], ones_t[:],
                                 start=True, stop=True)

            # tiny mask constants on the gpsimd queue (needed in phase 2)
            tri_t = pool_sm.tile([128, 128], BF, tag="tri", name="tri_t")
            nc.gpsimd.dma_start(tri_t[:], tri_d[:])
            ident_t = pool_sm.tile([128, 128], BF, tag="id", name="ident_t")
            nc.gpsimd.dma_start(ident_t[:], ident_d[:])

            # ---- persistent activation tensors ----
            qk_t = [pool_qk.tile([128, T], BF, tag="qk", name=f"qk{i}") for i in range(NF)]
            v_t = [pool_v.tile([128, KVL * HD], BF, tag="v", name=f"v{i}") for i in range(NJT)]
            at_t = [pool_at.tile([128, T], BF, tag="at", name=f"at{i}") for i in range(HL)]

            # ================= Phase 1: QKV projections + rope =============
            # DMA priority on the sync queue: w0, then the chunk-0 x wave
            # (exactly what the first f-tile's first matmuls need), then
            # chunk 1, then rope tables (K first - K heads rope first),
            # then W_v. x/W_v go as single large prearranged transfers to
            # keep the sequencer issue count low.
            for half in range(2):
                hs = half * HALFT
                wq_pre = []

                def w_prefetch(f, half=half):
                    w = pool_w.tile([128, NE, 128], BF, tag="w",
                                    name=f"w_pre{half}_{f}")
                    nc.sync.dma_start(w[:], wqk_d[f])
                    wq_pre.append(w)

                forder = list(range(HL, NF)) + list(range(HL))
                w_prefetch(forder[0])
                xt_t = []
                for cc in range(2):
                    xx = pool_big.tile([128, NE, CH], BF, tag="big",
                                       name=f"xt{half}_{cc}")
                    nc.sync.dma_start(xx[:], xt_d[half, :, :, cc, :])
                    xt_t.append(xx)
                w_prefetch(forder[1])
                w_prefetch(forder[2])
                if half == 0:
                    ak_t = pool_tab.tile([HD, T], BF, tag="tab", name="ak_t")
                    nc.sync.dma_start(ak_t[:], ak_d[:])
                    bk_t = pool_tab.tile([HD, T], BF, tag="tab", name="bk_t")
                    nc.sync.dma_start(bk_t[:], bk_d[:])
                    aq_t = pool_tab.tile([HD, T], BF, tag="tab", name="aq_t")
                    nc.sync.dma_start(aq_t[:], aq_d[:])
                    bq_t = pool_tab.tile([HD, T], BF, tag="tab", name="bq_t")
                    nc.sync.dma_start(bq_t[:], bq_d[:])
                    wv_t = pool_wv.tile([128, NE, KVL * HD], BF, tag="wv",
                                        name="wv_t")
                    nc.sync.dma_start(wv_t[:], wv_d[:])

                for fi, f in enumerate(forder):
                    # host-prearranged W column block, contiguous per partition
                    w_t = wq_pre.pop(0)
                    if fi + 3 < NF:
                        w_prefetch(forder[fi + 3])
                    for c in range(HALFT // CH):
                        ps = pool_ps.tile([128, CH], F32, tag="psacc", bufs=2)
                        for e in range(NE):
                            nc.tensor.matmul(
                                ps[:],
                                w_t[:, e, :],
                                xt_t[c][:, e, :],
                                start=(e == 0),
                                stop=(e == NE - 1),
                            )
                        nc.vector.tensor_copy(
                            qk_t[f][:, hs + c * CH: hs + (c + 1) * CH], ps[:]
                        )
                    # rope over this token half
                    A_t, B_t = (aq_t, bq_t) if f < HL else (ak_t, bk_t)
                    q = qk_t[f]
                    sl = slice(hs, hs + HALFT)
                    qs = pool_tmp.tile([128, HALFT], BF, tag="qs")
                    nc.sync.dma_start(qs[0:64, :], q[64:128, sl])
                    nc.sync.dma_start(qs[64:128, :], q[0:64, sl])
                    nc.vector.tensor_mul(qs[:, :], qs[:, :], B_t[:, sl])
                    nc.vector.tensor_mul(q[:, sl], q[:, sl], A_t[:, sl])
                    nc.vector.tensor_add(q[:, sl], q[:, sl], qs[:])

                for tt in range(NJT // 2):
                    tglob = half * (NJT // 2) + tt
                    cc, co = tt // 4, (tt % 4) * 128
                    psv = pool_ps.tile([128, KVL * HD], F32, tag="psacc", bufs=2)
                    for e in range(NE):
                        nc.tensor.matmul(
                            psv[:],
                            xt_t[cc][:, e, co:co + 128],
                            wv_t[:, e, :],
                            start=(e == 0),
                            stop=(e == NE - 1),
                        )
                    nc.vector.tensor_copy(v_t[tglob][:], psv[:])

            # W_o loads reuse the xt big-tile ring (freed after phase 1):
            # two [128, NE, CH] tiles, mapping [p, fb*2+e2, i] so phase 3's
            # (fb, ec) slice is wo_t[ec//2][:, fb*2 + ec%2, :]
            wo_t = []
            for eh in range(2):
                w = pool_big.tile([128, NE, CH], BF, tag="big",
                                  name=f"wo{eh}")
                nc.sync.dma_start(w[:], wo_d[eh])
                wo_t.append(w)

            # ================= Phase 2: attention ==========================
            # Chunk-pair-major over heads: all heads' token pair 0 (chunks
            # 0-1) first, then pair 1 (chunks 2-3). Tokens 0-1023 of every
            # head finish after the first sweep, unblocking the first half
            # of phase 3 as PE filler while the rest of attention (which is
            # ACT-exp-bound per chunk) runs. AV/den matmuls of each j-block
            # are deferred TWO steps so the PE always has independent work
            # while ACT runs exp. Per-chunk normalization chains are staged
            # one boundary later per stage.
            pend_q = []     # deferred AV/den emitters, one list per j-block
            fin_chains = []  # normalization chains, one stage/boundary
            den2_map = {}

            def make_fin_a(hl, c, acc, den, den2):
                # per-chunk psum evacuation: acc -> at_t, den row -> its half
                # of the pair's den2 buffer (x 1/4096 for the fp16 recip)
                def stage_a():
                    nc.vector.tensor_copy(
                        at_t[hl][:, c * CH:(c + 1) * CH], acc[:]
                    )
                    nc.vector.tensor_scalar_mul(
                        den2[0:1, (c % 2) * CH:(c % 2 + 1) * CH],
                        den[0:1, :], 1.0 / 4096.0,
                    )
                return [stage_a]

            def make_fin_bc(hl, cp, den2):
                # per chunk-PAIR: reciprocal + broadcast + normalize over a
                # [128, 2*CH] region; half the DMA-descriptor bursts of the
                # per-chunk variant
                state = {}

                def stage_b():
                    d32 = pool_dv.tile([32, 2 * CH // 32], F32, tag="d32",
                                       bufs=2, name=f"d32_{hl}_{cp}")
                    nc.sync.dma_start(d32[:], den2[:])
                    dr = pool_dv.tile([32, 2 * CH // 32], F16, tag="dr",
                                      bufs=2, name=f"dr{hl}_{cp}")
                    with nc.allow_low_precision(reason="fp16 dinv; x4096 scaling keeps it normal"):
                        nc.vector.reciprocal(dr[:], d32[:])
                    dd_t = pool_dvr.tile([1, 2 * CH], F16, tag="dvrow",
                                         name=f"dinv_dram{hl}_{cp}")
                    nc.sync.dma_start(dd_t[:], dr[:])
                    bch = pool_bch.tile([128, 2 * CH], F16, tag="bch", bufs=2,
                                        name=f"bch{hl}_{cp}")
                    nc.sync.dma_start(bch[:], dd_t[:].to_broadcast((128, 2 * CH)))
                    state["bch"] = bch

                def stage_c():
                    nc.gpsimd.tensor_mul(
                        at_t[hl][:, cp * 2 * CH:(cp + 1) * 2 * CH],
                        at_t[hl][:, cp * 2 * CH:(cp + 1) * 2 * CH],
                        state["bch"][:],
                    )

                return [stage_b, stage_c]

            def fin_boundary():
                for chain in fin_chains:
                    chain.pop(0)()
                fin_chains[:] = [ch for ch in fin_chains if ch]

            # Phase-3 output-projection emitters. The first token half
            # (it < 8) only needs the cp0 attention sweep, so those groups
            # are emitted INTO the cp1 sweep's PE stream (the PE queue runs
            # in emission order - work emitted later cannot fill earlier
            # stalls). Interleaved groups use their own 1-bank psum tag so
            # they never WAR against the still-accumulating attention psum.
            os_map = {}

            def p3_group(it, eh, e2, tag, bufs):
                def emit():
                    key = (it, eh)
                    if key not in os_map:
                        os_map[key] = pool_o.tile(
                            [128, E // 2], BF, tag="o", bufs=2,
                            name=f"os{it}_{eh}")
                    os_t = os_map[key]
                    po = pool_ps.tile([128, CH], F32, tag=tag, bufs=bufs,
                                      name=f"po{it}_{eh}_{e2}")
                    for fb in range(HL):
                        nc.tensor.matmul(
                            po[:],
                            at_t[fb][:, it * 128:(it + 1) * 128],
                            wo_t[eh][:, fb * 2 + e2, :],
                            start=(fb == 0),
                            stop=(fb == HL - 1),
                        )
                    nc.vector.tensor_copy(
                        os_t[:, e2 * CH:(e2 + 1) * CH], po[:]
                    )
                    if e2 == 1:
                        nc.sync.dma_start(
                            out_d[it * 128:(it + 1) * 128,
                                  eh * HALFT:(eh + 1) * HALFT],
                            os_t[:],
                        )
                return emit

            p3_queue = [(it, eh, e2)
                        for it in range(T // 256)
                        for eh in range(2)
                        for e2 in range(2)]
            p3_budget = [24]   # interleaved groups; 8 reserved for the tail

            def p3_slot():
                if p3_budget[0] > 0 and p3_queue:
                    p3_budget[0] -= 1
                    p3_group(*p3_queue.pop(0), tag="pso", bufs=1)()

            chunk_order = [(hl, cp * 2 + ci)
                           for cp in range(NCH // 2)
                           for hl in range(HL)
                           for ci in range(2)]
            for ci_idx, (hl, c) in enumerate(chunk_order):
                kf = HL + hl // REP
                kvc = (hl // REP) * HD
                njt = (c + 1) * (CH // 128)
                if c % 2 == 0:
                    den2 = pool_dv.tile([1, 2 * CH], F32, tag="den2",
                                        bufs=2, name=f"den2_{hl}_{c // 2}")
                    den2_map[hl] = den2
                else:
                    den2 = den2_map[hl]
                acc = pool_ps.tile([128, CH], F32, tag="psacc", bufs=2,
                                   name=f"acc{hl}_{c}")
                den = pool_ps.tile([128, CH], F32, tag="psden", bufs=1,
                                   name=f"den{hl}_{c}")
                j0_order = list(range(0, njt, 2))
                start_jt = 0
                stop_jt = njt - 1
                for step, j0 in enumerate(j0_order):
                    # causally-live column start per j-tile: diagonal
                    # tiles (d >= 0) only need cols [128*d, 512)
                    i0s = []
                    for u in range(2):
                        d = (j0 + u) - (njt - 4)
                        i0s.append(128 * d if d > 0 else 0)
                    diag = (j0 >= njt - 4)
                    s2 = pool_ps.tile([128, 2, CH], F32, tag="ps", bufs=2,
                                      name=f"s2_{hl}_{c}_{j0}")
                    for u in range(2):
                        jt = j0 + u
                        masked = (jt >= njt - 4)
                        nc.tensor.matmul(
                            s2[:, u, i0s[u]:],
                            qk_t[kf][:, jt * 128:(jt + 1) * 128],
                            qk_t[hl][:, c * CH + i0s[u]:(c + 1) * CH],
                            start=True,
                            stop=not masked,
                        )
                        if masked:
                            nc.tensor.matmul(
                                s2[:, u, i0s[u]:i0s[u] + 128],
                                ident_t[:],
                                tri_t[:],
                                start=False,
                                stop=True,
                            )
                    p2 = pool_p.tile([128, 2, CH], BF, tag="p", bufs=4,
                                     name=f"p2_{hl}_{c}_{j0}")
                    # one activation per step; for diagonal pairs the
                    # region [i0s[0], CH) covers both u-slices (u=1's
                    # cols [i0s[0], i0s[1]) hold unread garbage)
                    nc.scalar.activation(
                        p2[:, :, i0s[0]:], s2[:, :, i0s[0]:],
                        mybir.ActivationFunctionType.Exp,
                        scale=INV_SQRT_D,
                    )
                    psum2 = None
                    quad = None
                    if diag:
                        pass    # causal mask already folded into the scores
                    else:
                        # pre-sum the probs pair on DVE, then merge step
                        # pairs into quads so the den ones-matmul streams a
                        # quarter of the rows
                        psum2 = pool_sp.tile([128, CH], BF, tag="sp",
                                             name=f"sp{hl}_{c}_{j0}")
                        nc.vector.tensor_add(
                            psum2[:], p2[:, 0, :], p2[:, 1, :]
                        )
                        if step % 2 == 0:
                            prev_psum2 = psum2
                        else:
                            quad = pool_sp.tile([128, CH], BF, tag="qd",
                                                bufs=2,
                                                name=f"qd{hl}_{c}_{j0}")
                            nc.vector.tensor_add(
                                quad[:], prev_psum2[:], psum2[:]
                            )
                    if len(pend_q) >= 3:
                        for fn in pend_q.pop(0):
                            fn()
                    if step == 2:
                        fin_boundary()
                    # feed first-half output-projection groups into the cp1
                    # sweep (at_t tokens 0-1023 are final for all heads two
                    # boundaries into the sweep)
                    if ci_idx >= 18 and step == 3:
                        p3_slot()
                    step_fns = []
                    for u in range(2):
                        jt = j0 + u
                        def av(jt=jt, p2=p2, u=u, acc=acc, kvc=kvc,
                               i0=i0s[u], sjt=start_jt, pjt=stop_jt):
                            nc.tensor.matmul(
                                acc[:, i0:],
                                v_t[jt][:, kvc:kvc + HD],
                                p2[:, u, i0:],
                                start=(jt == sjt),
                                stop=(jt == pjt),
                            )
                        step_fns.append(av)
                    if not diag:
                        if quad is not None:
                            def den_quad(quad=quad, den=den,
                                         first=(step == 1)):
                                nc.tensor.matmul(
                                    den[:],
                                    ones_t[:, 0:128],
                                    quad[:],
                                    start=first,
                                    stop=False,
                                )
                            step_fns.append(den_quad)
                    else:
                        for u in range(2):
                            jt = j0 + u
                            def den_u(jt=jt, p2=p2, u=u, den=den,
                                      i0=i0s[u],
                                      sjt=start_jt, pjt=stop_jt):
                                nc.tensor.matmul(
                                    den[:, i0:],
                                    ones_t[:, 0:128],
                                    p2[:, u, i0:],
                                    start=(jt == sjt),
                                    stop=(jt == pjt),
                                )
                            step_fns.append(den_u)
                    pend_q.append(step_fns)
                fin_chains.append(make_fin_a(hl, c, acc, den, den2))
                if c % 2 == 1:
                    fin_chains.append(make_fin_bc(hl, c // 2, den2))
                if ci_idx >= 17:
                    p3_slot()
            while pend_q:
                for fn in pend_q.pop(0):
                    fn()
                # ready first-half output tiles keep the PE fed while the
                # final AV/den/normalize chains drain
                if p3_queue:
                    p3_group(*p3_queue.pop(0), tag="pso", bufs=1)()
            while fin_chains:
                fin_boundary()
                if p3_queue:
                    p3_group(*p3_queue.pop(0), tag="psacc", bufs=2)()

            # ================= Phase 3 tail: remaining output tiles ========
            while p3_queue:
                p3_group(*p3_queue.pop(0), tag="psacc", bufs=2)()
            for it in range(T // 256, T // 128):
                for eh in range(2):
                    for e2 in range(2):
                        p3_group(it, eh, e2, tag="psacc", bufs=2)()

    nc.compile()
    return nc


def _get_compiled():
    global _COMPILED
    if _COMPILED is None:
        _COMPILED = _build_nc()
    return _COMPILED


def _host_tables():
    half = np.arange(0, HD, 2, dtype=np.float64)
    inv_freq = 1.0 / (THETA ** (half / HD))
    t_idx = np.arange(T, dtype=np.float64)
    freqs = np.outer(t_idx, inv_freq)
    emb = np.concatenate([freqs, freqs], axis=-1)
    cos, sin = np.cos(emb), np.sin(emb)
    scale_vec = (half + 0.4 * HD) / (1.4 * HD)
    power = (t_idx - T // 2) / SCALE_BASE
    scale = scale_vec[None, :] ** power[:, None]
    scale = np.concatenate([scale, scale], axis=-1)
    sgn = np.where(np.arange(HD) < HD // 2, -1.0, 1.0)
    aq = (scale * cos).T
    bq = sgn[:, None] * (scale * sin).T
    ak = (cos / scale).T
    bk = sgn[:, None] * (sin / scale).T

    # within-tile causal mask, additive: -1e9 where j > i (applied to the
    # scores via an identity-stationary matmul before exp)
    dj = np.arange(128)[:, None]
    r = np.arange(128)[None, :]
    tri = np.where(dj > r, -1e9, 0.0)
    ident = np.eye(128)
    return (
        aq.astype(BF16), bq.astype(BF16), ak.astype(BF16), bk.astype(BF16),
        tri.astype(BF16), ident.astype(BF16),
    )


def _arrange_wqk(wq, wk):
    # [E, F] -> per 128-wide f-block: [128(part=e%128), NE(e//128), 128(f)]
    w = np.concatenate([wq, wk], axis=1)          # [E, NF*128]
    nf = w.shape[1] // 128
    w = w.reshape(NE, 128, nf, 128)               # [n, p, f_blk, fc]
    w = w.transpose(2, 1, 0, 3)                   # [f_blk, p, n, fc]
    return np.ascontiguousarray(w).astype(BF16)


def _arrange_xt(xt):
    # [E, T] -> [half, p, e, chunk, i]
    w = xt.reshape(NE, 128, 2, 2, CH)             # [e, p, half, cc, i]
    w = w.transpose(2, 1, 0, 3, 4)                # [half, p, e, cc, i]
    return np.ascontiguousarray(w).astype(BF16)


def _arrange_wv(wv):
    # [E, KVL*HD] -> [p, e, f]
    w = wv.reshape(NE, 128, KVL * HD)
    w = w.transpose(1, 0, 2)
    return np.ascontiguousarray(w).astype(BF16)


def _arrange_wo(wo):
    # [HL*HD, E] -> [Ehalf, p, fb*2+e2, i] so (fb, ec) slice is
    # [eh=ec//2][:, fb*2 + ec%2, :]
    w = wo.reshape(HL, 128, 2, 2, CH)             # [fb, p, eh, e2, i]
    w = w.transpose(2, 1, 0, 3, 4)                # [eh, p, fb, e2, i]
    w = w.reshape(2, 128, NE, CH)
    return np.ascontiguousarray(w).astype(BF16)


def _make_in_maps(x, W_q, W_k, W_v, W_o):
    aq, bq, ak, bk, tri, ident = _host_tables()
    xts = [_arrange_xt(np.ascontiguousarray(x[b].T)) for b in range(B)]
    in_maps = []
    for core in range(8):
        b, g = core // G, core % G
        in_maps.append({
            "xt": xts[b],
            "wqk": _arrange_wqk(W_q[:, g * HL * HD:(g + 1) * HL * HD],
                                W_k[:, g * KVL * HD:(g + 1) * KVL * HD]),
            "wv": _arrange_wv(W_v[:, g * KVL * HD:(g + 1) * KVL * HD]),
            "wo": _arrange_wo(W_o[g * HL * HD:(g + 1) * HL * HD, :] / 4096.0),
            "aq": aq, "bq": bq, "ak": ak, "bk": bk,
            "tri": tri,
            "ident": ident,
        })
    return in_maps


def _run(x, W_q, W_k, W_v, W_o, trace=False):
    nc = _get_compiled()
    in_maps = _make_in_maps(x, W_q, W_k, W_v, W_o)
    res = run_bass_kernel_spmd(nc, in_maps, list(range(8)), trace=trace)
    out = np.empty((B, T, E), np.float32)
    for b in range(B):
        out[b] = (res.results[2 * b]["out_p"].astype(np.float32)
                  + res.results[2 * b + 1]["out_p"].astype(np.float32))
    return out, res.exec_time_ns


def kernel(x, W_q, W_k, W_v, W_o):
    out, _ = _run(
        np.asarray(x), np.asarray(W_q), np.asarray(W_k),
        np.asarray(W_v), np.asarray(W_o),
    )
    return out


# revision 26
# speedup vs baseline: 1.2045x; 1.0254x over previous
"""Trainium2 Bass kernel for nn_MultiHeadSelfAttention_11158325035343.

GQA multi-head self-attention (B=4, T=2048, E=2048, H=16, HKV=8, HD=128)
with XPos rotary embedding and causal softmax.

Sharding: 8 cores = 4 batches x 2 head-groups. Each core computes, for its
batch b and head-group g (8 q heads, 4 kv heads):
  QT/KT = W.T @ x.T   ([head_dim, T] per head, head_dim on partitions)
  V     = x @ W_v     ([T, head_dim] per kv head)
  XPos rope applied via two host-precomputed fused tables + half-swap
  scoresT[j, i] per (head, i-chunk, j-tile), exp without max subtraction
  (scores are bounded: XPos decay keeps them small), causal mask applied
  post-exp: diagonal j-tiles narrow their score/exp/AV/den work to the
  causally-live columns and a [128,128] triangular 0/1 multiply on GpSimd
  zeroes the within-tile j>i region, softmax denominator via ones-matmul
  on PE over DVE-presummed prob pairs, AV/den matmuls deferred two steps
  behind the scores so the PE always has independent work while ACT runs
  exp, attnT = V.T-contraction with probs as moving operand, normalized
  by the broadcast reciprocal denominator (fin chain staged across chunk
  boundaries), partial out = attnT.T @ W_o rows-for-this-group, written
  as bf16 partials.
Host sums the two group partials per batch in f32.

Scheduling structure:
  - PE warmed up via memset-ones matmuls at t=0 (HAM clock-gate release)
  - x/W_v/W_o staged via single large prearranged DMAs (sequencer issue
    cost is ~600ns per dma_start, so fewer+bigger wins)
  - phase 2 runs chunk-pair-major (all heads' token pair 0 first): after
    ~26% of attention work the first half of phase 3 unlocks, giving the
    scheduler PE filler for the ACT-bound remainder of attention
"""

import sys
import types

sys.path.insert(0, "/opt/trn_rl_repo")

import numpy as np
import ml_dtypes

BF16 = ml_dtypes.bfloat16

# ---------------------------------------------------------------------------
# NTFF profile hook injection (missing antenv.axon_hooks in this image).
# Needed only when trace=True; harmless otherwise.
# ---------------------------------------------------------------------------
def _ensure_axon_hooks():
    if "antenv.axon_hooks" in sys.modules:
        return
    try:
        import antenv
        mod = types.ModuleType("antenv.axon_hooks")
        holder = {"hook": None}
        mod.set_axon_ntff_profile_hook = lambda h: holder.__setitem__("hook", h)
        mod.get_axon_ntff_profile_hook = lambda: holder["hook"]
        sys.modules["antenv.axon_hooks"] = mod
        antenv.axon_hooks = mod
        from trn_agent_boot.trn_boot import _ntff_profile_via_ctypes
        mod.set_axon_ntff_profile_hook(
            _ntff_profile_via_ctypes("/opt/axon/libaxon_pjrt.so")
        )
    except Exception:
        pass


_ensure_axon_hooks()

import concourse.bass as bass
import concourse.bacc as bacc
import concourse.mybir as mybir
import concourse.tile as tile
from concourse.bass_utils import run_bass_kernel_spmd

# Problem constants (hardcoded per spec).
B, T, E = 4, 2048, 2048
H, HKV, HD = 16, 8, 128
THETA, SCALE_BASE = 10000.0, 512.0
G = 2                   # head groups (cores per batch)
HL = H // G             # 8 local q heads
KVL = HKV // G          # 4 local kv heads
REP = H // HKV          # GQA repeat
CH = 512                # i-chunk / matmul free dim
NE = E // 128           # 16 contraction tiles
NF = HL + KVL           # 12 projection f-tiles (8 Q + 4 K)
HALFT = T // 2          # token half for phase-1 SBUF staging
NJT = T // 128          # 16 j tiles
NCH = T // CH           # 4 i chunks
INV_SQRT_D = 1.0 / float(np.sqrt(np.float32(HD)))

F32 = mybir.dt.float32
F16 = mybir.dt.float16
BF = mybir.dt.bfloat16

_COMPILED = None


def _build_nc():
    nc = bacc.Bacc("TRN2", target_bir_lowering=False, debug=False, num_devices=8)

    # xt prearranged on host: [half, p, e, chunk, i]
    xt_d = nc.dram_tensor("xt", [2, 128, NE, 2, CH], BF, kind="ExternalInput")
    wqk_d = nc.dram_tensor("wqk", [NF, 128, NE, 128], BF, kind="ExternalInput")
    # wv prearranged: [p, e, f]
    wv_d = nc.dram_tensor("wv", [128, NE, KVL * HD], BF, kind="ExternalInput")
    # wo prearranged: [Ehalf, p, fb*2+e2, i]
    wo_d = nc.dram_tensor("wo", [2, 128, NE, CH], BF, kind="ExternalInput")
    aq_d = nc.dram_tensor("aq", [HD, T], BF, kind="ExternalInput")
    bq_d = nc.dram_tensor("bq", [HD, T], BF, kind="ExternalInput")
    ak_d = nc.dram_tensor("ak", [HD, T], BF, kind="ExternalInput")
    bk_d = nc.dram_tensor("bk", [HD, T], BF, kind="ExternalInput")
    tri_d = nc.dram_tensor("tri", [128, 128], BF, kind="ExternalInput")
    ident_d = nc.dram_tensor("ident", [128, 128], BF, kind="ExternalInput")
    out_d = nc.dram_tensor("out_p", [T, E], BF, kind="ExternalOutput")

    with tile.TileContext(nc) as tc:
        with (
            tc.tile_pool(name="big", bufs=2) as pool_big,       # xt chunks / wo
            tc.tile_pool(name="qk", bufs=NF) as pool_qk,        # rope'd QT/KT bf16
            tc.tile_pool(name="v", bufs=NJT) as pool_v,         # V bf16
            tc.tile_pool(name="at", bufs=HL) as pool_at,        # attnT bf16
            tc.tile_pool(name="tab", bufs=4) as pool_tab,       # rope tables
            tc.tile_pool(name="wv", bufs=1) as pool_wv,         # resident W_v
            tc.tile_pool(name="w", bufs=3) as pool_w,           # streamed W_q/W_k
            tc.tile_pool(name="tmp", bufs=2) as pool_tmp,       # rope temp
            tc.tile_pool(name="p", bufs=4) as pool_p,           # exp probs bf16
            tc.tile_pool(name="sp", bufs=3) as pool_sp,         # den pair presums
            tc.tile_pool(name="o", bufs=2) as pool_o,           # out staging
            tc.tile_pool(name="sm", bufs=1) as pool_sm,         # small constants
            tc.tile_pool(name="dv", bufs=4) as pool_dv,         # recip denominators
            tc.tile_pool(name="dvr", bufs=4, space="DRAM") as pool_dvr,  # dinv DRAM bounce
            tc.tile_pool(name="bch", bufs=3) as pool_bch,       # dinv bcast per chunk
            tc.tile_pool(name="ps", bufs=2, space=bass.MemorySpace.PSUM) as pool_ps,
        ):
            # ---- PE warmup: all-ones tile via memset (no DMA dependency)
            # so the HAM clock gate releases at ~3.4us, before the input
            # DMA wave completes. The same tile serves the den ones-matmul.
            ones_t = pool_sm.tile([128, CH], BF, tag="oc", name="ones_t")
            nc.vector.memset(ones_t[:], 1.0)
            warm_ps = pool_ps.tile([128, CH], F32, tag="psden", bufs=1,
                                   name="warm_ps")
            for _wi in range(40):
                nc.tensor.matmul(warm_ps[:], ones_t[:, 0:128], ones_t[:],
                                 start=True, stop=True)

            # tiny mask constants on the gpsimd queue (needed in phase 2)
            tri_t = pool_sm.tile([128, 128], BF, tag="tri", name="tri_t")
            nc.gpsimd.dma_start(tri_t[:], tri_d[:])
            ident_t = pool_sm.tile([128, 128], BF, tag="id", name="ident_t")
            nc.gpsimd.dma_start(ident_t[:], ident_d[:])

            # ---- persistent activation tensors ----
            qk_t = [pool_qk.tile([128, T], BF, tag="qk", name=f"qk{i}") for i in range(NF)]
            v_t = [pool_v.tile([128, KVL * HD], BF, tag="v", name=f"v{i}") for i in range(NJT)]
            at_t = [pool_at.tile([128, T], BF, tag="at", name=f"at{i}") for i in range(HL)]

            # ================= Phase 1: QKV projections + rope =============
            # DMA priority on the sync queue: w0, then the chunk-0 x wave
            # (exactly what the first f-tile's first matmuls need), then
            # chunk 1, then rope tables (K first - K heads rope first),
            # then W_v. x/W_v go as single large prearranged transfers to
            # keep the sequencer issue count low.
            for half in range(2):
                hs = half * HALFT
                wq_pre = []

                def w_prefetch(f, half=half):
                    w = pool_w.tile([128, NE, 128], BF, tag="w",
                                    name=f"w_pre{half}_{f}")
                    nc.sync.dma_start(w[:], wqk_d[f])
                    wq_pre.append(w)

                forder = list(range(HL, NF)) + list(range(HL))
                w_prefetch(forder[0])
                xt_t = []
                for cc in range(2):
                    xx = pool_big.tile([128, NE, CH], BF, tag="big",
                                       name=f"xt{half}_{cc}")
                    nc.sync.dma_start(xx[:], xt_d[half, :, :, cc, :])
                    xt_t.append(xx)
                w_prefetch(forder[1])
                w_prefetch(forder[2])
                if half == 0:
                    ak_t = pool_tab.tile([HD, T], BF, tag="tab", name="ak_t")
                    nc.sync.dma_start(ak_t[:], ak_d[:])
                    bk_t = pool_tab.tile([HD, T], BF, tag="tab", name="bk_t")
                    nc.sync.dma_start(bk_t[:], bk_d[:])
                    aq_t = pool_tab.tile([HD, T], BF, tag="tab", name="aq_t")
                    nc.sync.dma_start(aq_t[:], aq_d[:])
                    bq_t = pool_tab.tile([HD, T], BF, tag="tab", name="bq_t")
                    nc.sync.dma_start(bq_t[:], bq_d[:])
                    wv_t = pool_wv.tile([128, NE, KVL * HD], BF, tag="wv",
                                        name="wv_t")
                    nc.sync.dma_start(wv_t[:], wv_d[:])

                for fi, f in enumerate(forder):
                    # host-prearranged W column block, contiguous per partition
                    w_t = wq_pre.pop(0)
                    if fi + 3 < NF:
                        w_prefetch(forder[fi + 3])
                    for c in range(HALFT // CH):
                        ps = pool_ps.tile([128, CH], F32, tag="psacc", bufs=2)
                        for e in range(NE):
                            nc.tensor.matmul(
                                ps[:],
                                w_t[:, e, :],
                                xt_t[c][:, e, :],
                                start=(e == 0),
                                stop=(e == NE - 1),
                            )
                        nc.vector.tensor_copy(
                            qk_t[f][:, hs + c * CH: hs + (c + 1) * CH], ps[:]
                        )
                    # rope over this token half
                    A_t, B_t = (aq_t, bq_t) if f < HL else (ak_t, bk_t)
                    q = qk_t[f]
                    sl = slice(hs, hs + HALFT)
                    qs = pool_tmp.tile([128, HALFT], BF, tag="qs")
                    nc.sync.dma_start(qs[0:64, :], q[64:128, sl])
                    nc.sync.dma_start(qs[64:128, :], q[0:64, sl])
                    nc.vector.tensor_mul(qs[:, :], qs[:, :], B_t[:, sl])
                    nc.vector.tensor_mul(q[:, sl], q[:, sl], A_t[:, sl])
                    nc.vector.tensor_add(q[:, sl], q[:, sl], qs[:])

                for tt in range(NJT // 2):
                    tglob = half * (NJT // 2) + tt
                    cc, co = tt // 4, (tt % 4) * 128
                    psv = pool_ps.tile([128, KVL * HD], F32, tag="psacc", bufs=2)
                    for e in range(NE):
                        nc.tensor.matmul(
                            psv[:],
                            xt_t[cc][:, e, co:co + 128],
                            wv_t[:, e, :],
                            start=(e == 0),
                            stop=(e == NE - 1),
                        )
                    nc.vector.tensor_copy(v_t[tglob][:], psv[:])

            # W_o loads reuse the xt big-tile ring (freed after phase 1):
            # two [128, NE, CH] tiles, mapping [p, fb*2+e2, i] so phase 3's
            # (fb, ec) slice is wo_t[ec//2][:, fb*2 + ec%2, :]
            wo_t = []
            for eh in range(2):
                w = pool_big.tile([128, NE, CH], BF, tag="big",
                                  name=f"wo{eh}")
                nc.sync.dma_start(w[:], wo_d[eh])
                wo_t.append(w)

            # ================= Phase 2: attention ==========================
            # Chunk-pair-major over heads: all heads' token pair 0 (chunks
            # 0-1) first, then pair 1 (chunks 2-3). Tokens 0-1023 of every
            # head finish after the first sweep, unblocking the first half
            # of phase 3 as PE filler while the rest of attention (which is
            # ACT-exp-bound per chunk) runs. AV/den matmuls of each j-block
            # are deferred TWO steps so the PE always has independent work
            # while ACT runs exp. Per-chunk normalization chains are staged
            # one boundary later per stage.
            pend_q = []     # deferred AV/den emitters, one list per j-block
            fin_chains = []  # normalization chains, one stage/boundary
            den2_map = {}

            def make_fin_a(hl, c, acc, den, den2):
                # per-chunk psum evacuation: acc -> at_t, den row -> its half
                # of the pair's den2 buffer (x 1/4096 for the fp16 recip)
                def stage_a():
                    nc.vector.tensor_copy(
                        at_t[hl][:, c * CH:(c + 1) * CH], acc[:]
                    )
                    nc.vector.tensor_scalar_mul(
                        den2[0:1, (c % 2) * CH:(c % 2 + 1) * CH],
                        den[0:1, :], 1.0 / 4096.0,
                    )
                return [stage_a]

            def make_fin_bc(hl, cp, den2):
                # per chunk-PAIR: reciprocal + broadcast + normalize over a
                # [128, 2*CH] region; half the DMA-descriptor bursts of the
                # per-chunk variant
                state = {}

                def stage_b():
                    d32 = pool_dv.tile([32, 2 * CH // 32], F32, tag="d32",
                                       bufs=2, name=f"d32_{hl}_{cp}")
                    nc.sync.dma_start(d32[:], den2[:])
                    dr = pool_dv.tile([32, 2 * CH // 32], F16, tag="dr",
                                      bufs=2, name=f"dr{hl}_{cp}")
                    with nc.allow_low_precision(reason="fp16 dinv; x4096 scaling keeps it normal"):
                        nc.vector.reciprocal(dr[:], d32[:])
                    dd_t = pool_dvr.tile([1, 2 * CH], F16, tag="dvrow",
                                         name=f"dinv_dram{hl}_{cp}")
                    nc.sync.dma_start(dd_t[:], dr[:])
                    bch = pool_bch.tile([128, 2 * CH], F16, tag="bch", bufs=2,
                                        name=f"bch{hl}_{cp}")
                    nc.sync.dma_start(bch[:], dd_t[:].to_broadcast((128, 2 * CH)))
                    state["bch"] = bch

                def stage_c():
                    for hh in range(2):
                        lo = cp * 2 * CH + hh * CH
                        nc.gpsimd.tensor_mul(
                            at_t[hl][:, lo:lo + CH],
                            at_t[hl][:, lo:lo + CH],
                            state["bch"][:, hh * CH:(hh + 1) * CH],
                        )

                return [stage_b, stage_c]

            def fin_boundary():
                for chain in fin_chains:
                    chain.pop(0)()
                fin_chains[:] = [ch for ch in fin_chains if ch]

            # Phase-3 output-projection emitters. The first token half
            # (it < 8) only needs the cp0 attention sweep, so those groups
            # are emitted INTO the cp1 sweep's PE stream (the PE queue runs
            # in emission order - work emitted later cannot fill earlier
            # stalls). Interleaved groups use their own 1-bank psum tag so
            # they never WAR against the still-accumulating attention psum.
            os_map = {}

            def p3_group(it, eh, e2, tag, bufs):
                def emit():
                    key = (it, eh)
                    if key not in os_map:
                        os_map[key] = pool_o.tile(
                            [128, E // 2], BF, tag="o", bufs=2,
                            name=f"os{it}_{eh}")
                    os_t = os_map[key]
                    po = pool_ps.tile([128, CH], F32, tag=tag, bufs=bufs,
                                      name=f"po{it}_{eh}_{e2}")
                    for fb in range(HL):
                        nc.tensor.matmul(
                            po[:],
                            at_t[fb][:, it * 128:(it + 1) * 128],
                            wo_t[eh][:, fb * 2 + e2, :],
                            start=(fb == 0),
                            stop=(fb == HL - 1),
                        )
                    nc.vector.tensor_copy(
                        os_t[:, e2 * CH:(e2 + 1) * CH], po[:]
                    )
                    if e2 == 1:
                        nc.sync.dma_start(
                            out_d[it * 128:(it + 1) * 128,
                                  eh * HALFT:(eh + 1) * HALFT],
                            os_t[:],
                        )
                return emit

            p3_queue = [(it, eh, e2)
                        for it in range(T // 256)
                        for eh in range(2)
                        for e2 in range(2)]
            p3_budget = [24]   # interleaved groups; 8 reserved for the tail

            def p3_slot():
                if p3_budget[0] > 0 and p3_queue:
                    p3_budget[0] -= 1
                    p3_group(*p3_queue.pop(0), tag="pso", bufs=1)()

            chunk_order = [(hl, cp * 2 + ci)
                           for cp in range(NCH // 2)
                           for hl in range(HL)
                           for ci in range(2)]
            for ci_idx, (hl, c) in enumerate(chunk_order):
                kf = HL + hl // REP
                kvc = (hl // REP) * HD
                njt = (c + 1) * (CH // 128)
                if c % 2 == 0:
                    den2 = pool_dv.tile([1, 2 * CH], F32, tag="den2",
                                        bufs=2, name=f"den2_{hl}_{c // 2}")
                    den2_map[hl] = den2
                else:
                    den2 = den2_map[hl]
                acc = pool_ps.tile([128, CH], F32, tag="psacc", bufs=2,
                                   name=f"acc{hl}_{c}")
                den = pool_ps.tile([128, CH], F32, tag="psden", bufs=1,
                                   name=f"den{hl}_{c}")
                j0_order = list(range(0, njt, 2))
                start_jt = 0
                stop_jt = njt - 1
                for step, j0 in enumerate(j0_order):
                    # causally-live column start per j-tile: diagonal
                    # tiles (d >= 0) only need cols [128*d, 512)
                    i0s = []
                    for u in range(2):
                        d = (j0 + u) - (njt - 4)
                        i0s.append(128 * d if d > 0 else 0)
                    diag = (j0 >= njt - 4)
                    s2 = pool_ps.tile([128, 2, CH], F32, tag="ps", bufs=2,
                                      name=f"s2_{hl}_{c}_{j0}")
                    for u in range(2):
                        jt = j0 + u
                        masked = (jt >= njt - 4)
                        nc.tensor.matmul(
                            s2[:, u, i0s[u]:],
                            qk_t[kf][:, jt * 128:(jt + 1) * 128],
                            qk_t[hl][:, c * CH + i0s[u]:(c + 1) * CH],
                            start=True,
                            stop=not masked,
                        )
                        if masked:
                            nc.tensor.matmul(
                                s2[:, u, i0s[u]:i0s[u] + 128],
                                ident_t[:],
                                tri_t[:],
                                start=False,
                                stop=True,
                            )
                    p2 = pool_p.tile([128, 2, CH], BF, tag="p", bufs=4,
                                     name=f"p2_{hl}_{c}_{j0}")
                    # one activation per step; for diagonal pairs the
                    # region [i0s[0], CH) covers both u-slices (u=1's
                    # cols [i0s[0], i0s[1]) hold unread garbage)
                    nc.scalar.activation(
                        p2[:, :, i0s[0]:], s2[:, :, i0s[0]:],
                        mybir.ActivationFunctionType.Exp,
                        scale=INV_SQRT_D,
                    )
                    psum2 = None
                    quad = None
                    if diag:
                        pass    # causal mask already folded into the scores
                    else:
                        # pre-sum the probs pair on DVE, then merge step
                        # pairs into quads so the den ones-matmul streams a
                        # quarter of the rows
                        psum2 = pool_sp.tile([128, CH], BF, tag="sp",
                                             name=f"sp{hl}_{c}_{j0}")
                        nc.vector.tensor_add(
                            psum2[:], p2[:, 0, :], p2[:, 1, :]
                        )
                        if step % 2 == 0:
                            prev_psum2 = psum2
                        else:
                            quad = pool_sp.tile([128, CH], BF, tag="qd",
                                                bufs=2,
                                                name=f"qd{hl}_{c}_{j0}")
                            nc.vector.tensor_add(
                                quad[:], prev_psum2[:], psum2[:]
                            )
                    if len(pend_q) >= 3:
                        for fn in pend_q.pop(0):
                            fn()
                    if step == 2:
                        fin_boundary()
                    # feed first-half output-projection groups into the cp1
                    # sweep (at_t tokens 0-1023 are final for all heads two
                    # boundaries into the sweep)
                    if ci_idx >= 18 and step == 3:
                        p3_slot()
                    step_fns = []
                    for u in range(2):
                        jt = j0 + u
                        def av(jt=jt, p2=p2, u=u, acc=acc, kvc=kvc,
                               i0=i0s[u], sjt=start_jt, pjt=stop_jt):
                            nc.tensor.matmul(
                                acc[:, i0:],
                                v_t[jt][:, kvc:kvc + HD],
                                p2[:, u, i0:],
                                start=(jt == sjt),
                                stop=(jt == pjt),
                            )
                        step_fns.append(av)
                    if not diag:
                        if quad is not None:
                            def den_quad(quad=quad, den=den,
                                         first=(step == 1)):
                                nc.tensor.matmul(
                                    den[:],
                                    ones_t[:, 0:128],
                                    quad[:],
                                    start=first,
                                    stop=False,
                                )
                            step_fns.append(den_quad)
                    else:
                        for u in range(2):
                            jt = j0 + u
                            def den_u(jt=jt, p2=p2, u=u, den=den,
                                      i0=i0s[u],
                                      sjt=start_jt, pjt=stop_jt):
                                nc.tensor.matmul(
                                    den[:, i0:],
                                    ones_t[:, 0:128],
                                    p2[:, u, i0:],
                                    start=(jt == sjt),
                                    stop=(jt == pjt),
                                )
                            step_fns.append(den_u)
                    pend_q.append(step_fns)
                fin_chains.append(make_fin_a(hl, c, acc, den, den2))
                if c % 2 == 1:
                    fin_chains.append(make_fin_bc(hl, c // 2, den2))
                if ci_idx >= 17:
                    p3_slot()
            while pend_q:
                for fn in pend_q.pop(0):
                    fn()
            # ready first-half output tiles keep the PE fed while the
            # final AV/den/normalize chains drain
            for _ in range(3):
                if p3_queue:
                    p3_group(*p3_queue.pop(0), tag="pso", bufs=1)()
            while fin_chains:
                fin_boundary()
                if p3_queue:
                    p3_group(*p3_queue.pop(0), tag="psacc", bufs=2)()

            # ================= Phase 3 tail: remaining output tiles ========
            while p3_queue:
                p3_group(*p3_queue.pop(0), tag="psacc", bufs=2)()
            for it in range(T // 256, T // 128):
                for eh in range(2):
                    for e2 in range(2):
                        p3_group(it, eh, e2, tag="psacc", bufs=2)()

    nc.compile()
    return nc


def _get_compiled():
    global _COMPILED
    if _COMPILED is None:
        _COMPILED = _build_nc()
    return _COMPILED


def _host_tables():
    half = np.arange(0, HD, 2, dtype=np.float64)
    inv_freq = 1.0 / (THETA ** (half / HD))
    t_idx = np.arange(T, dtype=np.float64)
    freqs = np.outer(t_idx, inv_freq)
    emb = np.concatenate([freqs, freqs], axis=-1)
    cos, sin = np.cos(emb), np.sin(emb)
    scale_vec = (half + 0.4 * HD) / (1.4 * HD)
    power = (t_idx - T // 2) / SCALE_BASE
    scale = scale_vec[None, :] ** power[:, None]
    scale = np.concatenate([scale, scale], axis=-1)
    sgn = np.where(np.arange(HD) < HD // 2, -1.0, 1.0)
    aq = (scale * cos).T
    bq = sgn[:, None] * (scale * sin).T
    ak = (cos / scale).T
    bk = sgn[:, None] * (sin / scale).T

    # within-tile causal mask, additive: -1e9 where j > i (applied to the
    # scores via an identity-stationary matmul before exp)
    dj = np.arange(128)[:, None]
    r = np.arange(128)[None, :]
    tri = np.where(dj > r, -1e9, 0.0)
    ident = np.eye(128)
    return (
        aq.astype(BF16), bq.astype(BF16), ak.astype(BF16), bk.astype(BF16),
        tri.astype(BF16), ident.astype(BF16),
    )


def _arrange_wqk(wq, wk):
    # [E, F] -> per 128-wide f-block: [128(part=e%128), NE(e//128), 128(f)]
    w = np.concatenate([wq, wk], axis=1)          # [E, NF*128]
    nf = w.shape[1] // 128
    w = w.reshape(NE, 128, nf, 128)               # [n, p, f_blk, fc]
    w = w.transpose(2, 1, 0, 3)                   # [f_blk, p, n, fc]
    return np.ascontiguousarray(w).astype(BF16)


def _arrange_xt(xt):
    # [E, T] -> [half, p, e, chunk, i]
    w = xt.reshape(NE, 128, 2, 2, CH)             # [e, p, half, cc, i]
    w = w.transpose(2, 1, 0, 3, 4)                # [half, p, e, cc, i]
    return np.ascontiguousarray(w).astype(BF16)


def _arrange_wv(wv):
    # [E, KVL*HD] -> [p, e, f]
    w = wv.reshape(NE, 128, KVL * HD)
    w = w.transpose(1, 0, 2)
    return np.ascontiguousarray(w).astype(BF16)


def _arrange_wo(wo):
    # [HL*HD, E] -> [Ehalf, p, fb*2+e2, i] so (fb, ec) slice is
    # [eh=ec//2][:, fb*2 + ec%2, :]
    w = wo.reshape(HL, 128, 2, 2, CH)             # [fb, p, eh, e2, i]
    w = w.transpose(2, 1, 0, 3, 4)                # [eh, p, fb, e2, i]
    w = w.reshape(2, 128, NE, CH)
    return np.ascontiguousarray(w).astype(BF16)


def _make_in_maps(x, W_q, W_k, W_v, W_o):
    aq, bq, ak, bk, tri, ident = _host_tables()
    xts = [_arrange_xt(np.ascontiguousarray(x[b].T)) for b in range(B)]
    in_maps = []
    for core in range(8):
        b, g = core // G, core % G
        in_maps.append({
            "xt": xts[b],
            "wqk": _arrange_wqk(W_q[:, g * HL * HD:(g + 1) * HL * HD],
                                W_k[:, g * KVL * HD:(g + 1) * KVL * HD]),
            "wv": _arrange_wv(W_v[:, g * KVL * HD:(g + 1) * KVL * HD]),
            "wo": _arrange_wo(W_o[g * HL * HD:(g + 1) * HL * HD, :] / 4096.0),
            "aq": aq, "bq": bq, "ak": ak, "bk": bk,
            "tri": tri,
            "ident": ident,
        })
    return in_maps


def _run(x, W_q, W_k, W_v, W_o, trace=False):
    nc = _get_compiled()
    in_maps = _make_in_maps(x, W_q, W_k, W_v, W_o)
    res = run_bass_kernel_spmd(nc, in_maps, list(range(8)), trace=trace)
    out = np.empty((B, T, E), np.float32)
    for b in range(B):
        out[b] = (res.results[2 * b]["out_p"].astype(np.float32)
                  + res.results[2 * b + 1]["out_p"].astype(np.float32))
    return out, res.exec_time_ns


def kernel(x, W_q, W_k, W_v, W_o):
    out, _ = _run(
        np.asarray(x), np.asarray(W_q), np.asarray(W_k),
        np.asarray(W_v), np.asarray(W_o),
    )
    return out


# revision 27
# speedup vs baseline: 1.2069x; 1.0020x over previous
"""Trainium2 Bass kernel for nn_MultiHeadSelfAttention_11158325035343.

GQA multi-head self-attention (B=4, T=2048, E=2048, H=16, HKV=8, HD=128)
with XPos rotary embedding and causal softmax.

Sharding: 8 cores = 4 batches x 2 head-groups. Each core computes, for its
batch b and head-group g (8 q heads, 4 kv heads):
  QT/KT = W.T @ x.T   ([head_dim, T] per head, head_dim on partitions)
  V     = x @ W_v     ([T, head_dim] per kv head)
  XPos rope applied via two host-precomputed fused tables + half-swap
  scoresT[j, i] per (head, i-chunk, j-tile), exp without max subtraction
  (scores are bounded: XPos decay keeps them small), causal mask applied
  post-exp: diagonal j-tiles narrow their score/exp/AV/den work to the
  causally-live columns and a [128,128] triangular 0/1 multiply on GpSimd
  zeroes the within-tile j>i region, softmax denominator via ones-matmul
  on PE over DVE-presummed prob pairs, AV/den matmuls deferred two steps
  behind the scores so the PE always has independent work while ACT runs
  exp, attnT = V.T-contraction with probs as moving operand, normalized
  by the broadcast reciprocal denominator (fin chain staged across chunk
  boundaries), partial out = attnT.T @ W_o rows-for-this-group, written
  as bf16 partials.
Host sums the two group partials per batch in f32.

Scheduling structure:
  - PE warmed up via memset-ones matmuls at t=0 (HAM clock-gate release)
  - x/W_v/W_o staged via single large prearranged DMAs (sequencer issue
    cost is ~600ns per dma_start, so fewer+bigger wins)
  - phase 2 runs chunk-pair-major (all heads' token pair 0 first): after
    ~26% of attention work the first half of phase 3 unlocks, giving the
    scheduler PE filler for the ACT-bound remainder of attention
"""

import sys
import types

sys.path.insert(0, "/opt/trn_rl_repo")

import numpy as np
import ml_dtypes

BF16 = ml_dtypes.bfloat16

# ---------------------------------------------------------------------------
# NTFF profile hook injection (missing antenv.axon_hooks in this image).
# Needed only when trace=True; harmless otherwise.
# ---------------------------------------------------------------------------
def _ensure_axon_hooks():
    if "antenv.axon_hooks" in sys.modules:
        return
    try:
        import antenv
        mod = types.ModuleType("antenv.axon_hooks")
        holder = {"hook": None}
        mod.set_axon_ntff_profile_hook = lambda h: holder.__setitem__("hook", h)
        mod.get_axon_ntff_profile_hook = lambda: holder["hook"]
        sys.modules["antenv.axon_hooks"] = mod
        antenv.axon_hooks = mod
        from trn_agent_boot.trn_boot import _ntff_profile_via_ctypes
        mod.set_axon_ntff_profile_hook(
            _ntff_profile_via_ctypes("/opt/axon/libaxon_pjrt.so")
        )
    except Exception:
        pass


_ensure_axon_hooks()

import concourse.bass as bass
import concourse.bacc as bacc
import concourse.mybir as mybir
import concourse.tile as tile
from concourse.bass_utils import run_bass_kernel_spmd

# Problem constants (hardcoded per spec).
B, T, E = 4, 2048, 2048
H, HKV, HD = 16, 8, 128
THETA, SCALE_BASE = 10000.0, 512.0
G = 2                   # head groups (cores per batch)
HL = H // G             # 8 local q heads
KVL = HKV // G          # 4 local kv heads
REP = H // HKV          # GQA repeat
CH = 512                # i-chunk / matmul free dim
NE = E // 128           # 16 contraction tiles
NF = HL + KVL           # 12 projection f-tiles (8 Q + 4 K)
HALFT = T // 2          # token half for phase-1 SBUF staging
NJT = T // 128          # 16 j tiles
NCH = T // CH           # 4 i chunks
INV_SQRT_D = 1.0 / float(np.sqrt(np.float32(HD)))

F32 = mybir.dt.float32
F16 = mybir.dt.float16
BF = mybir.dt.bfloat16

_COMPILED = None


def _build_nc():
    nc = bacc.Bacc("TRN2", target_bir_lowering=False, debug=False, num_devices=8)

    # xt prearranged on host: [half, p, e, chunk, i]
    xt_d = nc.dram_tensor("xt", [2, 128, NE, 2, CH], BF, kind="ExternalInput")
    wqk_d = nc.dram_tensor("wqk", [NF, 128, NE, 128], BF, kind="ExternalInput")
    # wv prearranged: [p, e, f]
    wv_d = nc.dram_tensor("wv", [128, NE, KVL * HD], BF, kind="ExternalInput")
    # wo prearranged: [Ehalf, p, fb*2+e2, i]
    wo_d = nc.dram_tensor("wo", [2, 128, NE, CH], BF, kind="ExternalInput")
    aq_d = nc.dram_tensor("aq", [HD, T], BF, kind="ExternalInput")
    bq_d = nc.dram_tensor("bq", [HD, T], BF, kind="ExternalInput")
    ak_d = nc.dram_tensor("ak", [HD, T], BF, kind="ExternalInput")
    bk_d = nc.dram_tensor("bk", [HD, T], BF, kind="ExternalInput")
    tri_d = nc.dram_tensor("tri", [128, 128], BF, kind="ExternalInput")
    ident_d = nc.dram_tensor("ident", [128, 128], BF, kind="ExternalInput")
    out_d = nc.dram_tensor("out_p", [T, E], BF, kind="ExternalOutput")

    with tile.TileContext(nc) as tc:
        with (
            tc.tile_pool(name="big", bufs=2) as pool_big,       # xt chunks / wo
            tc.tile_pool(name="qk", bufs=NF) as pool_qk,        # rope'd QT/KT bf16
            tc.tile_pool(name="v", bufs=NJT) as pool_v,         # V bf16
            tc.tile_pool(name="at", bufs=HL) as pool_at,        # attnT bf16
            tc.tile_pool(name="tab", bufs=4) as pool_tab,       # rope tables
            tc.tile_pool(name="wv", bufs=1) as pool_wv,         # resident W_v
            tc.tile_pool(name="w", bufs=3) as pool_w,           # streamed W_q/W_k
            tc.tile_pool(name="tmp", bufs=2) as pool_tmp,       # rope temp
            tc.tile_pool(name="p", bufs=4) as pool_p,           # exp probs bf16
            tc.tile_pool(name="sp", bufs=3) as pool_sp,         # den pair presums
            tc.tile_pool(name="o", bufs=2) as pool_o,           # out staging
            tc.tile_pool(name="sm", bufs=1) as pool_sm,         # small constants
            tc.tile_pool(name="dv", bufs=4) as pool_dv,         # recip denominators
            tc.tile_pool(name="dvr", bufs=4, space="DRAM") as pool_dvr,  # dinv DRAM bounce
            tc.tile_pool(name="bch", bufs=3) as pool_bch,       # dinv bcast per chunk
            tc.tile_pool(name="ps", bufs=2, space=bass.MemorySpace.PSUM) as pool_ps,
        ):
            # ---- PE warmup: all-ones tile via memset (no DMA dependency)
            # so the HAM clock gate releases at ~3.4us, before the input
            # DMA wave completes. The same tile serves the den ones-matmul.
            ones_t = pool_sm.tile([128, CH], BF, tag="oc", name="ones_t")
            nc.vector.memset(ones_t[:], 1.0)
            warm_ps = pool_ps.tile([128, CH], F32, tag="psden", bufs=1,
                                   name="warm_ps")
            for _wi in range(30):
                nc.tensor.matmul(warm_ps[:], ones_t[:, 0:128], ones_t[:],
                                 start=True, stop=True)

            # tiny mask constants on the gpsimd queue (needed in phase 2)
            tri_t = pool_sm.tile([128, 128], BF, tag="tri", name="tri_t")
            nc.gpsimd.dma_start(tri_t[:], tri_d[:])
            ident_t = pool_sm.tile([128, 128], BF, tag="id", name="ident_t")
            nc.gpsimd.dma_start(ident_t[:], ident_d[:])

            # ---- persistent activation tensors ----
            qk_t = [pool_qk.tile([128, T], BF, tag="qk", name=f"qk{i}") for i in range(NF)]
            v_t = [pool_v.tile([128, KVL * HD], BF, tag="v", name=f"v{i}") for i in range(NJT)]
            at_t = [pool_at.tile([128, T], BF, tag="at", name=f"at{i}") for i in range(HL)]

            # ================= Phase 1: QKV projections + rope =============
            # DMA priority on the sync queue: w0, then the chunk-0 x wave
            # (exactly what the first f-tile's first matmuls need), then
            # chunk 1, then rope tables (K first - K heads rope first),
            # then W_v. x/W_v go as single large prearranged transfers to
            # keep the sequencer issue count low.
            for half in range(2):
                hs = half * HALFT
                wq_pre = []

                def w_prefetch(f, half=half):
                    w = pool_w.tile([128, NE, 128], BF, tag="w",
                                    name=f"w_pre{half}_{f}")
                    nc.sync.dma_start(w[:], wqk_d[f])
                    wq_pre.append(w)

                forder = list(range(HL, NF)) + list(range(HL))
                w_prefetch(forder[0])
                xt_t = []
                for cc in range(2):
                    xx = pool_big.tile([128, NE, CH], BF, tag="big",
                                       name=f"xt{half}_{cc}")
                    nc.sync.dma_start(xx[:], xt_d[half, :, :, cc, :])
                    xt_t.append(xx)
                w_prefetch(forder[1])
                w_prefetch(forder[2])
                if half == 0:
                    ak_t = pool_tab.tile([HD, T], BF, tag="tab", name="ak_t")
                    nc.sync.dma_start(ak_t[:], ak_d[:])
                    bk_t = pool_tab.tile([HD, T], BF, tag="tab", name="bk_t")
                    nc.sync.dma_start(bk_t[:], bk_d[:])
                    aq_t = pool_tab.tile([HD, T], BF, tag="tab", name="aq_t")
                    nc.sync.dma_start(aq_t[:], aq_d[:])
                    bq_t = pool_tab.tile([HD, T], BF, tag="tab", name="bq_t")
                    nc.sync.dma_start(bq_t[:], bq_d[:])
                    wv_t = pool_wv.tile([128, NE, KVL * HD], BF, tag="wv",
                                        name="wv_t")
                    nc.sync.dma_start(wv_t[:], wv_d[:])

                for fi, f in enumerate(forder):
                    # host-prearranged W column block, contiguous per partition
                    w_t = wq_pre.pop(0)
                    if fi + 3 < NF:
                        w_prefetch(forder[fi + 3])
                    for c in range(HALFT // CH):
                        ps = pool_ps.tile([128, CH], F32, tag="psacc", bufs=2)
                        for e in range(NE):
                            nc.tensor.matmul(
                                ps[:],
                                w_t[:, e, :],
                                xt_t[c][:, e, :],
                                start=(e == 0),
                                stop=(e == NE - 1),
                            )
                        nc.vector.tensor_copy(
                            qk_t[f][:, hs + c * CH: hs + (c + 1) * CH], ps[:]
                        )
                    # rope over this token half
                    A_t, B_t = (aq_t, bq_t) if f < HL else (ak_t, bk_t)
                    q = qk_t[f]
                    sl = slice(hs, hs + HALFT)
                    qs = pool_tmp.tile([128, HALFT], BF, tag="qs")
                    nc.sync.dma_start(qs[0:64, :], q[64:128, sl])
                    nc.sync.dma_start(qs[64:128, :], q[0:64, sl])
                    nc.vector.tensor_mul(qs[:, :], qs[:, :], B_t[:, sl])
                    nc.vector.tensor_mul(q[:, sl], q[:, sl], A_t[:, sl])
                    nc.vector.tensor_add(q[:, sl], q[:, sl], qs[:])

                for tt in range(NJT // 2):
                    tglob = half * (NJT // 2) + tt
                    cc, co = tt // 4, (tt % 4) * 128
                    psv = pool_ps.tile([128, KVL * HD], F32, tag="psacc", bufs=2)
                    for e in range(NE):
                        nc.tensor.matmul(
                            psv[:],
                            xt_t[cc][:, e, co:co + 128],
                            wv_t[:, e, :],
                            start=(e == 0),
                            stop=(e == NE - 1),
                        )
                    nc.vector.tensor_copy(v_t[tglob][:], psv[:])

            # W_o loads reuse the xt big-tile ring (freed after phase 1):
            # two [128, NE, CH] tiles, mapping [p, fb*2+e2, i] so phase 3's
            # (fb, ec) slice is wo_t[ec//2][:, fb*2 + ec%2, :]
            wo_t = []
            for eh in range(2):
                w = pool_big.tile([128, NE, CH], BF, tag="big",
                                  name=f"wo{eh}")
                nc.sync.dma_start(w[:], wo_d[eh])
                wo_t.append(w)

            # ================= Phase 2: attention ==========================
            # Chunk-pair-major over heads: all heads' token pair 0 (chunks
            # 0-1) first, then pair 1 (chunks 2-3). Tokens 0-1023 of every
            # head finish after the first sweep, unblocking the first half
            # of phase 3 as PE filler while the rest of attention (which is
            # ACT-exp-bound per chunk) runs. AV/den matmuls of each j-block
            # are deferred TWO steps so the PE always has independent work
            # while ACT runs exp. Per-chunk normalization chains are staged
            # one boundary later per stage.
            pend_q = []     # deferred AV/den emitters, one list per j-block
            fin_chains = []  # normalization chains, one stage/boundary
            den2_map = {}

            def make_fin_a(hl, c, acc, den, den2):
                # per-chunk psum evacuation: acc -> at_t, den row -> its half
                # of the pair's den2 buffer (x 1/4096 for the fp16 recip)
                def stage_a():
                    nc.vector.tensor_copy(
                        at_t[hl][:, c * CH:(c + 1) * CH], acc[:]
                    )
                    nc.vector.tensor_scalar_mul(
                        den2[0:1, (c % 2) * CH:(c % 2 + 1) * CH],
                        den[0:1, :], 1.0 / 4096.0,
                    )
                return [stage_a]

            def make_fin_bc(hl, cp, den2):
                # per chunk-PAIR: reciprocal + broadcast + normalize over a
                # [128, 2*CH] region; half the DMA-descriptor bursts of the
                # per-chunk variant
                state = {}

                def stage_b():
                    d32 = pool_dv.tile([32, 2 * CH // 32], F32, tag="d32",
                                       bufs=2, name=f"d32_{hl}_{cp}")
                    nc.sync.dma_start(d32[:], den2[:])
                    dr = pool_dv.tile([32, 2 * CH // 32], F16, tag="dr",
                                      bufs=2, name=f"dr{hl}_{cp}")
                    with nc.allow_low_precision(reason="fp16 dinv; x4096 scaling keeps it normal"):
                        nc.vector.reciprocal(dr[:], d32[:])
                    dd_t = pool_dvr.tile([1, 2 * CH], F16, tag="dvrow",
                                         name=f"dinv_dram{hl}_{cp}")
                    nc.sync.dma_start(dd_t[:], dr[:])
                    bch = pool_bch.tile([128, 2 * CH], F16, tag="bch", bufs=2,
                                        name=f"bch{hl}_{cp}")
                    nc.sync.dma_start(bch[:], dd_t[:].to_broadcast((128, 2 * CH)))
                    state["bch"] = bch

                def stage_c():
                    for hh in range(2):
                        lo = cp * 2 * CH + hh * CH
                        nc.gpsimd.tensor_mul(
                            at_t[hl][:, lo:lo + CH],
                            at_t[hl][:, lo:lo + CH],
                            state["bch"][:, hh * CH:(hh + 1) * CH],
                        )

                return [stage_b, stage_c]

            def fin_boundary():
                for chain in fin_chains:
                    chain.pop(0)()
                fin_chains[:] = [ch for ch in fin_chains if ch]

            # Phase-3 output-projection emitters. The first token half
            # (it < 8) only needs the cp0 attention sweep, so those groups
            # are emitted INTO the cp1 sweep's PE stream (the PE queue runs
            # in emission order - work emitted later cannot fill earlier
            # stalls). Interleaved groups use their own 1-bank psum tag so
            # they never WAR against the still-accumulating attention psum.
            os_map = {}

            def p3_group(it, eh, e2, tag, bufs):
                def emit():
                    key = (it, eh)
                    if key not in os_map:
                        os_map[key] = pool_o.tile(
                            [128, E // 2], BF, tag="o", bufs=2,
                            name=f"os{it}_{eh}")
                    os_t = os_map[key]
                    po = pool_ps.tile([128, CH], F32, tag=tag, bufs=bufs,
                                      name=f"po{it}_{eh}_{e2}")
                    for fb in range(HL):
                        nc.tensor.matmul(
                            po[:],
                            at_t[fb][:, it * 128:(it + 1) * 128],
                            wo_t[eh][:, fb * 2 + e2, :],
                            start=(fb == 0),
                            stop=(fb == HL - 1),
                        )
                    nc.vector.tensor_copy(
                        os_t[:, e2 * CH:(e2 + 1) * CH], po[:]
                    )
                    if e2 == 1:
                        nc.sync.dma_start(
                            out_d[it * 128:(it + 1) * 128,
                                  eh * HALFT:(eh + 1) * HALFT],
                            os_t[:],
                        )
                return emit

            p3_queue = [(it, eh, e2)
                        for it in range(T // 256)
                        for eh in range(2)
                        for e2 in range(2)]
            p3_budget = [24]   # interleaved groups; 8 reserved for the tail

            def p3_slot():
                if p3_budget[0] > 0 and p3_queue:
                    p3_budget[0] -= 1
                    p3_group(*p3_queue.pop(0), tag="pso", bufs=1)()

            chunk_order = [(hl, cp * 2 + ci)
                           for cp in range(NCH // 2)
                           for hl in range(HL)
                           for ci in range(2)]
            for ci_idx, (hl, c) in enumerate(chunk_order):
                kf = HL + hl // REP
                kvc = (hl // REP) * HD
                njt = (c + 1) * (CH // 128)
                if c % 2 == 0:
                    den2 = pool_dv.tile([1, 2 * CH], F32, tag="den2",
                                        bufs=2, name=f"den2_{hl}_{c // 2}")
                    den2_map[hl] = den2
                else:
                    den2 = den2_map[hl]
                acc = pool_ps.tile([128, CH], F32, tag="psacc", bufs=2,
                                   name=f"acc{hl}_{c}")
                den = pool_ps.tile([128, CH], F32, tag="psden", bufs=1,
                                   name=f"den{hl}_{c}")
                j0_order = list(range(0, njt, 2))
                start_jt = 0
                stop_jt = njt - 1
                for step, j0 in enumerate(j0_order):
                    # causally-live column start per j-tile: diagonal
                    # tiles (d >= 0) only need cols [128*d, 512)
                    i0s = []
                    for u in range(2):
                        d = (j0 + u) - (njt - 4)
                        i0s.append(128 * d if d > 0 else 0)
                    diag = (j0 >= njt - 4)
                    s2 = pool_ps.tile([128, 2, CH], F32, tag="ps", bufs=2,
                                      name=f"s2_{hl}_{c}_{j0}")
                    for u in range(2):
                        jt = j0 + u
                        masked = (jt >= njt - 4)
                        nc.tensor.matmul(
                            s2[:, u, i0s[u]:],
                            qk_t[kf][:, jt * 128:(jt + 1) * 128],
                            qk_t[hl][:, c * CH + i0s[u]:(c + 1) * CH],
                            start=True,
                            stop=not masked,
                        )
                        if masked:
                            nc.tensor.matmul(
                                s2[:, u, i0s[u]:i0s[u] + 128],
                                ident_t[:],
                                tri_t[:],
                                start=False,
                                stop=True,
                            )
                    p2 = pool_p.tile([128, 2, CH], BF, tag="p", bufs=4,
                                     name=f"p2_{hl}_{c}_{j0}")
                    # one activation per step; for diagonal pairs the
                    # region [i0s[0], CH) covers both u-slices (u=1's
                    # cols [i0s[0], i0s[1]) hold unread garbage)
                    nc.scalar.activation(
                        p2[:, :, i0s[0]:], s2[:, :, i0s[0]:],
                        mybir.ActivationFunctionType.Exp,
                        scale=INV_SQRT_D,
                    )
                    psum2 = None
                    quad = None
                    if diag:
                        pass    # causal mask already folded into the scores
                    else:
                        # pre-sum the probs pair on DVE, then merge step
                        # pairs into quads so the den ones-matmul streams a
                        # quarter of the rows
                        psum2 = pool_sp.tile([128, CH], BF, tag="sp",
                                             name=f"sp{hl}_{c}_{j0}")
                        nc.vector.tensor_add(
                            psum2[:], p2[:, 0, :], p2[:, 1, :]
                        )
                        if step % 2 == 0:
                            prev_psum2 = psum2
                        else:
                            quad = pool_sp.tile([128, CH], BF, tag="qd",
                                                bufs=2,
                                                name=f"qd{hl}_{c}_{j0}")
                            nc.vector.tensor_add(
                                quad[:], prev_psum2[:], psum2[:]
                            )
                    if len(pend_q) >= 3:
                        for fn in pend_q.pop(0):
                            fn()
                    if step == 2:
                        fin_boundary()
                    # feed first-half output-projection groups into the cp1
                    # sweep (at_t tokens 0-1023 are final for all heads two
                    # boundaries into the sweep)
                    if ci_idx >= 18 and step == 3:
                        p3_slot()
                    step_fns = []
                    for u in range(2):
                        jt = j0 + u
                        def av(jt=jt, p2=p2, u=u, acc=acc, kvc=kvc,
                               i0=i0s[u], sjt=start_jt, pjt=stop_jt):
                            nc.tensor.matmul(
                                acc[:, i0:],
                                v_t[jt][:, kvc:kvc + HD],
                                p2[:, u, i0:],
                                start=(jt == sjt),
                                stop=(jt == pjt),
                            )
                        step_fns.append(av)
                    if not diag:
                        if quad is not None:
                            def den_quad(quad=quad, den=den,
                                         first=(step == 1)):
                                nc.tensor.matmul(
                                    den[:],
                                    ones_t[:, 0:128],
                                    quad[:],
                                    start=first,
                                    stop=False,
                                )
                            step_fns.append(den_quad)
                    else:
                        for u in range(2):
                            jt = j0 + u
                            def den_u(jt=jt, p2=p2, u=u, den=den,
                                      i0=i0s[u],
                                      sjt=start_jt, pjt=stop_jt):
                                nc.tensor.matmul(
                                    den[:, i0:],
                                    ones_t[:, 0:128],
                                    p2[:, u, i0:],
                                    start=(jt == sjt),
                                    stop=(jt == pjt),
                                )
                            step_fns.append(den_u)
                    pend_q.append(step_fns)
                fin_chains.append(make_fin_a(hl, c, acc, den, den2))
                if c % 2 == 1:
                    fin_chains.append(make_fin_bc(hl, c // 2, den2))
                if ci_idx >= 17:
                    p3_slot()
            while pend_q:
                for fn in pend_q.pop(0):
                    fn()
            # ready first-half output tiles keep the PE fed while the
            # final AV/den/normalize chains drain
            for _ in range(3):
                if p3_queue:
                    p3_group(*p3_queue.pop(0), tag="pso", bufs=1)()
            while fin_chains:
                fin_boundary()
                if p3_queue:
                    p3_group(*p3_queue.pop(0), tag="psacc", bufs=2)()

            # ================= Phase 3 tail: remaining output tiles ========
            while p3_queue:
                p3_group(*p3_queue.pop(0), tag="psacc", bufs=2)()
            for it in range(T // 256, T // 128):
                for eh in range(2):
                    for e2 in range(2):
                        p3_group(it, eh, e2, tag="psacc", bufs=2)()

    nc.compile()
    return nc


def _get_compiled():
    global _COMPILED
    if _COMPILED is None:
        _COMPILED = _build_nc()
    return _COMPILED


def _host_tables():
    half = np.arange(0, HD, 2, dtype=np.float64)
    inv_freq = 1.0 / (THETA ** (half / HD))
    t_idx = np.arange(T, dtype=np.float64)
    freqs = np.outer(t_idx, inv_freq)
    emb = np.concatenate([freqs, freqs], axis=-1)
    cos, sin = np.cos(emb), np.sin(emb)
    scale_vec = (half + 0.4 * HD) / (1.4 * HD)
    power = (t_idx - T // 2) / SCALE_BASE
    scale = scale_vec[None, :] ** power[:, None]
    scale = np.concatenate([scale, scale], axis=-1)
    sgn = np.where(np.arange(HD) < HD // 2, -1.0, 1.0)
    aq = (scale * cos).T
    bq = sgn[:, None] * (scale * sin).T
    ak = (cos / scale).T
    bk = sgn[:, None] * (sin / scale).T

    # within-tile causal mask, additive: -1e9 where j > i (applied to the
    # scores via an identity-stationary matmul before exp)
    dj = np.arange(128)[:, None]
    r = np.arange(128)[None, :]
    tri = np.where(dj > r, -1e9, 0.0)
    ident = np.eye(128)
    return (
        aq.astype(BF16), bq.astype(BF16), ak.astype(BF16), bk.astype(BF16),
        tri.astype(BF16), ident.astype(BF16),
    )


def _arrange_wqk(wq, wk):
    # [E, F] -> per 128-wide f-block: [128(part=e%128), NE(e//128), 128(f)]
    w = np.concatenate([wq, wk], axis=1)          # [E, NF*128]
    nf = w.shape[1] // 128
    w = w.reshape(NE, 128, nf, 128)               # [n, p, f_blk, fc]
    w = w.transpose(2, 1, 0, 3)                   # [f_blk, p, n, fc]
    return np.ascontiguousarray(w).astype(BF16)


def _arrange_xt(xt):
    # [E, T] -> [half, p, e, chunk, i]
    w = xt.reshape(NE, 128, 2, 2, CH)             # [e, p, half, cc, i]
    w = w.transpose(2, 1, 0, 3, 4)                # [half, p, e, cc, i]
    return np.ascontiguousarray(w).astype(BF16)


def _arrange_wv(wv):
    # [E, KVL*HD] -> [p, e, f]
    w = wv.reshape(NE, 128, KVL * HD)
    w = w.transpose(1, 0, 2)
    return np.ascontiguousarray(w).astype(BF16)


def _arrange_wo(wo):
    # [HL*HD, E] -> [Ehalf, p, fb*2+e2, i] so (fb, ec) slice is
    # [eh=ec//2][:, fb*2 + ec%2, :]
    w = wo.reshape(HL, 128, 2, 2, CH)             # [fb, p, eh, e2, i]
    w = w.transpose(2, 1, 0, 3, 4)                # [eh, p, fb, e2, i]
    w = w.reshape(2, 128, NE, CH)
    return np.ascontiguousarray(w).astype(BF16)


def _make_in_maps(x, W_q, W_k, W_v, W_o):
    aq, bq, ak, bk, tri, ident = _host_tables()
    xts = [_arrange_xt(np.ascontiguousarray(x[b].T)) for b in range(B)]
    in_maps = []
    for core in range(8):
        b, g = core // G, core % G
        in_maps.append({
            "xt": xts[b],
            "wqk": _arrange_wqk(W_q[:, g * HL * HD:(g + 1) * HL * HD],
                                W_k[:, g * KVL * HD:(g + 1) * KVL * HD]),
            "wv": _arrange_wv(W_v[:, g * KVL * HD:(g + 1) * KVL * HD]),
            "wo": _arrange_wo(W_o[g * HL * HD:(g + 1) * HL * HD, :] / 4096.0),
            "aq": aq, "bq": bq, "ak": ak, "bk": bk,
            "tri": tri,
            "ident": ident,
        })
    return in_maps


def _run(x, W_q, W_k, W_v, W_o, trace=False):
    nc = _get_compiled()
    in_maps = _make_in_maps(x, W_q, W_k, W_v, W_o)
    res = run_bass_kernel_spmd(nc, in_maps, list(range(8)), trace=trace)
    out = np.empty((B, T, E), np.float32)
    for b in range(B):
        out[b] = (res.results[2 * b]["out_p"].astype(np.float32)
                  + res.results[2 * b + 1]["out_p"].astype(np.float32))
    return out, res.exec_time_ns


def kernel(x, W_q, W_k, W_v, W_o):
    out, _ = _run(
        np.asarray(x), np.asarray(W_q), np.asarray(W_k),
        np.asarray(W_v), np.asarray(W_o),
    )
    return out


# revision 29
# speedup vs baseline: 1.2111x; 1.0035x over previous
"""Trainium2 Bass kernel for nn_MultiHeadSelfAttention_11158325035343.

GQA multi-head self-attention (B=4, T=2048, E=2048, H=16, HKV=8, HD=128)
with XPos rotary embedding and causal softmax.

Sharding: 8 cores = 4 batches x 2 head-groups. Each core computes, for its
batch b and head-group g (8 q heads, 4 kv heads):
  QT/KT = W.T @ x.T   ([head_dim, T] per head, head_dim on partitions)
  V     = x @ W_v     ([T, head_dim] per kv head)
  XPos rope applied via two host-precomputed fused tables + half-swap
  scoresT[j, i] per (head, i-chunk, j-tile), exp without max subtraction
  (scores are bounded: XPos decay keeps them small), causal mask applied
  post-exp: diagonal j-tiles narrow their score/exp/AV/den work to the
  causally-live columns and a [128,128] triangular 0/1 multiply on GpSimd
  zeroes the within-tile j>i region, softmax denominator via ones-matmul
  on PE over DVE-presummed prob pairs, AV/den matmuls deferred two steps
  behind the scores so the PE always has independent work while ACT runs
  exp, attnT = V.T-contraction with probs as moving operand, normalized
  by the broadcast reciprocal denominator (fin chain staged across chunk
  boundaries), partial out = attnT.T @ W_o rows-for-this-group, written
  as bf16 partials.
Host sums the two group partials per batch in f32.

Scheduling structure:
  - PE warmed up via memset-ones matmuls at t=0 (HAM clock-gate release)
  - x/W_v/W_o staged via single large prearranged DMAs (sequencer issue
    cost is ~600ns per dma_start, so fewer+bigger wins)
  - phase 2 runs chunk-pair-major (all heads' token pair 0 first): after
    ~26% of attention work the first half of phase 3 unlocks, giving the
    scheduler PE filler for the ACT-bound remainder of attention
"""

import sys
import types

sys.path.insert(0, "/opt/trn_rl_repo")

import numpy as np
import ml_dtypes

BF16 = ml_dtypes.bfloat16

# ---------------------------------------------------------------------------
# NTFF profile hook injection (missing antenv.axon_hooks in this image).
# Needed only when trace=True; harmless otherwise.
# ---------------------------------------------------------------------------
def _ensure_axon_hooks():
    if "antenv.axon_hooks" in sys.modules:
        return
    try:
        import antenv
        mod = types.ModuleType("antenv.axon_hooks")
        holder = {"hook": None}
        mod.set_axon_ntff_profile_hook = lambda h: holder.__setitem__("hook", h)
        mod.get_axon_ntff_profile_hook = lambda: holder["hook"]
        sys.modules["antenv.axon_hooks"] = mod
        antenv.axon_hooks = mod
        from trn_agent_boot.trn_boot import _ntff_profile_via_ctypes
        mod.set_axon_ntff_profile_hook(
            _ntff_profile_via_ctypes("/opt/axon/libaxon_pjrt.so")
        )
    except Exception:
        pass


_ensure_axon_hooks()

import concourse.bass as bass
import concourse.bacc as bacc
import concourse.mybir as mybir
import concourse.tile as tile
from concourse.bass_utils import run_bass_kernel_spmd

# Problem constants (hardcoded per spec).
B, T, E = 4, 2048, 2048
H, HKV, HD = 16, 8, 128
THETA, SCALE_BASE = 10000.0, 512.0
G = 2                   # head groups (cores per batch)
HL = H // G             # 8 local q heads
KVL = HKV // G          # 4 local kv heads
REP = H // HKV          # GQA repeat
CH = 512                # i-chunk / matmul free dim
NE = E // 128           # 16 contraction tiles
NF = HL + KVL           # 12 projection f-tiles (8 Q + 4 K)
HALFT = T // 2          # token half for phase-1 SBUF staging
NJT = T // 128          # 16 j tiles
NCH = T // CH           # 4 i chunks
INV_SQRT_D = 1.0 / float(np.sqrt(np.float32(HD)))

F32 = mybir.dt.float32
F16 = mybir.dt.float16
BF = mybir.dt.bfloat16

_COMPILED = None


def _build_nc():
    nc = bacc.Bacc("TRN2", target_bir_lowering=False, debug=False, num_devices=8)

    # xt prearranged on host: [half, p, e, chunk, i]
    xt_d = nc.dram_tensor("xt", [2, 128, NE, 2, CH], BF, kind="ExternalInput")
    wqk_d = nc.dram_tensor("wqk", [NF, 128, NE, 128], BF, kind="ExternalInput")
    # wv prearranged: [p, e, f]
    wv_d = nc.dram_tensor("wv", [128, NE, KVL * HD], BF, kind="ExternalInput")
    # wo prearranged: [Ehalf, p, fb*2+e2, i]
    wo_d = nc.dram_tensor("wo", [2, 128, NE, CH], BF, kind="ExternalInput")
    aq_d = nc.dram_tensor("aq", [HD, T], BF, kind="ExternalInput")
    bq_d = nc.dram_tensor("bq", [HD, T], BF, kind="ExternalInput")
    ak_d = nc.dram_tensor("ak", [HD, T], BF, kind="ExternalInput")
    bk_d = nc.dram_tensor("bk", [HD, T], BF, kind="ExternalInput")
    tri_d = nc.dram_tensor("tri", [128, 128], BF, kind="ExternalInput")
    ident_d = nc.dram_tensor("ident", [128, 128], BF, kind="ExternalInput")
    out_d = nc.dram_tensor("out_p", [T, E], BF, kind="ExternalOutput")

    with tile.TileContext(nc) as tc:
        with (
            tc.tile_pool(name="big", bufs=2) as pool_big,       # xt chunks / wo
            tc.tile_pool(name="qk", bufs=NF) as pool_qk,        # rope'd QT/KT bf16
            tc.tile_pool(name="v", bufs=NJT) as pool_v,         # V bf16
            tc.tile_pool(name="at", bufs=HL) as pool_at,        # attnT bf16
            tc.tile_pool(name="tab", bufs=4) as pool_tab,       # rope tables
            tc.tile_pool(name="wv", bufs=1) as pool_wv,         # resident W_v
            tc.tile_pool(name="w", bufs=3) as pool_w,           # streamed W_q/W_k
            tc.tile_pool(name="tmp", bufs=2) as pool_tmp,       # rope temp
            tc.tile_pool(name="p", bufs=4) as pool_p,           # exp probs bf16
            tc.tile_pool(name="sp", bufs=2) as pool_sp,         # den pair presums
            tc.tile_pool(name="o", bufs=2) as pool_o,           # out staging
            tc.tile_pool(name="sm", bufs=1) as pool_sm,         # small constants
            tc.tile_pool(name="dv", bufs=4) as pool_dv,         # recip denominators
            tc.tile_pool(name="dvr", bufs=4, space="DRAM") as pool_dvr,  # dinv DRAM bounce
            tc.tile_pool(name="bch", bufs=3) as pool_bch,       # dinv bcast per chunk
            tc.tile_pool(name="ps", bufs=2, space=bass.MemorySpace.PSUM) as pool_ps,
        ):
            # ---- PE warmup: all-ones tile via memset (no DMA dependency)
            # so the HAM clock gate releases at ~3.4us, before the input
            # DMA wave completes. The same tile serves the den ones-matmul.
            ones_t = pool_sm.tile([128, CH], BF, tag="oc", name="ones_t")
            nc.vector.memset(ones_t[:], 1.0)
            warm_ps = pool_ps.tile([128, CH], F32, tag="psden", bufs=1,
                                   name="warm_ps")
            for _wi in range(30):
                nc.tensor.matmul(warm_ps[:], ones_t[:, 0:128], ones_t[:],
                                 start=True, stop=True)

            # tiny mask constants on the gpsimd queue (needed in phase 2)
            tri_t = pool_sm.tile([128, 128], BF, tag="tri", name="tri_t")
            nc.gpsimd.dma_start(tri_t[:], tri_d[:])
            ident_t = pool_sm.tile([128, 128], BF, tag="id", name="ident_t")
            nc.gpsimd.dma_start(ident_t[:], ident_d[:])

            # ---- persistent activation tensors ----
            qk_t = [pool_qk.tile([128, T], BF, tag="qk", name=f"qk{i}") for i in range(NF)]
            v_t = [pool_v.tile([128, KVL * HD], BF, tag="v", name=f"v{i}") for i in range(NJT)]
            at_t = [pool_at.tile([128, T], BF, tag="at", name=f"at{i}") for i in range(HL)]

            # ================= Phase 1: QKV projections + rope =============
            # DMA priority on the sync queue: w0, then the chunk-0 x wave
            # (exactly what the first f-tile's first matmuls need), then
            # chunk 1, then rope tables (K first - K heads rope first),
            # then W_v. x/W_v go as single large prearranged transfers to
            # keep the sequencer issue count low.
            for half in range(2):
                hs = half * HALFT
                wq_pre = []

                def w_prefetch(f, half=half):
                    w = pool_w.tile([128, NE, 128], BF, tag="w",
                                    name=f"w_pre{half}_{f}")
                    nc.sync.dma_start(w[:], wqk_d[f])
                    wq_pre.append(w)

                forder = list(range(HL, NF)) + list(range(HL))
                w_prefetch(forder[0])
                xt_t = []
                for cc in range(2):
                    xx = pool_big.tile([128, NE, CH], BF, tag="big",
                                       name=f"xt{half}_{cc}")
                    nc.sync.dma_start(xx[:], xt_d[half, :, :, cc, :])
                    xt_t.append(xx)
                w_prefetch(forder[1])
                w_prefetch(forder[2])
                if half == 0:
                    ak_t = pool_tab.tile([HD, T], BF, tag="tab", name="ak_t")
                    nc.sync.dma_start(ak_t[:], ak_d[:])
                    bk_t = pool_tab.tile([HD, T], BF, tag="tab", name="bk_t")
                    nc.sync.dma_start(bk_t[:], bk_d[:])
                    aq_t = pool_tab.tile([HD, T], BF, tag="tab", name="aq_t")
                    nc.sync.dma_start(aq_t[:], aq_d[:])
                    bq_t = pool_tab.tile([HD, T], BF, tag="tab", name="bq_t")
                    nc.sync.dma_start(bq_t[:], bq_d[:])
                    wv_t = pool_wv.tile([128, NE, KVL * HD], BF, tag="wv",
                                        name="wv_t")
                    nc.sync.dma_start(wv_t[:], wv_d[:])

                for fi, f in enumerate(forder):
                    # host-prearranged W column block, contiguous per partition
                    w_t = wq_pre.pop(0)
                    if fi + 3 < NF:
                        w_prefetch(forder[fi + 3])
                    for c in range(HALFT // CH):
                        ps = pool_ps.tile([128, CH], F32, tag="psacc", bufs=2)
                        for e in range(NE):
                            nc.tensor.matmul(
                                ps[:],
                                w_t[:, e, :],
                                xt_t[c][:, e, :],
                                start=(e == 0),
                                stop=(e == NE - 1),
                            )
                        nc.vector.tensor_copy(
                            qk_t[f][:, hs + c * CH: hs + (c + 1) * CH], ps[:]
                        )
                    # rope over this token half
                    A_t, B_t = (aq_t, bq_t) if f < HL else (ak_t, bk_t)
                    q = qk_t[f]
                    sl = slice(hs, hs + HALFT)
                    qs = pool_tmp.tile([128, HALFT], BF, tag="qs")
                    nc.sync.dma_start(qs[0:64, :], q[64:128, sl])
                    nc.sync.dma_start(qs[64:128, :], q[0:64, sl])
                    nc.vector.tensor_mul(qs[:, :], qs[:, :], B_t[:, sl])
                    nc.vector.tensor_mul(q[:, sl], q[:, sl], A_t[:, sl])
                    nc.vector.tensor_add(q[:, sl], q[:, sl], qs[:])

                for tt in range(NJT // 2):
                    tglob = half * (NJT // 2) + tt
                    cc, co = tt // 4, (tt % 4) * 128
                    psv = pool_ps.tile([128, KVL * HD], F32, tag="psacc", bufs=2)
                    for e in range(NE):
                        nc.tensor.matmul(
                            psv[:],
                            xt_t[cc][:, e, co:co + 128],
                            wv_t[:, e, :],
                            start=(e == 0),
                            stop=(e == NE - 1),
                        )
                    nc.vector.tensor_copy(v_t[tglob][:], psv[:])

            # W_o loads reuse the xt big-tile ring (freed after phase 1):
            # two [128, NE, CH] tiles, mapping [p, fb*2+e2, i] so phase 3's
            # (fb, ec) slice is wo_t[ec//2][:, fb*2 + ec%2, :]
            wo_t = []
            for eh in range(2):
                w = pool_big.tile([128, NE, CH], BF, tag="big",
                                  name=f"wo{eh}")
                nc.sync.dma_start(w[:], wo_d[eh])
                wo_t.append(w)

            # ================= Phase 2: attention ==========================
            # Chunk-pair-major over heads: all heads' token pair 0 (chunks
            # 0-1) first, then pair 1 (chunks 2-3). Tokens 0-1023 of every
            # head finish after the first sweep, unblocking the first half
            # of phase 3 as PE filler while the rest of attention (which is
            # ACT-exp-bound per chunk) runs. AV/den matmuls of each j-block
            # are deferred TWO steps so the PE always has independent work
            # while ACT runs exp. Per-chunk normalization chains are staged
            # one boundary later per stage.
            pend_q = []     # deferred AV/den emitters, one list per j-block
            fin_chains = []  # normalization chains, one stage/boundary
            den2_map = {}

            def make_fin_a(hl, c, acc, den, den2):
                # per-chunk psum evacuation: acc -> at_t, den row -> its half
                # of the pair's den2 buffer (x 1/4096 for the fp16 recip)
                def stage_a():
                    nc.vector.tensor_copy(
                        at_t[hl][:, c * CH:(c + 1) * CH], acc[:]
                    )
                    nc.vector.tensor_scalar_mul(
                        den2[0:1, (c % 2) * CH:(c % 2 + 1) * CH],
                        den[0:1, :], 1.0 / 4096.0,
                    )
                return [stage_a]

            def make_fin_bc(hl, cp, den2):
                # per chunk-PAIR: reciprocal + broadcast + normalize over a
                # [128, 2*CH] region; half the DMA-descriptor bursts of the
                # per-chunk variant
                state = {}

                def stage_b():
                    d32 = pool_dv.tile([32, 2 * CH // 32], F32, tag="d32",
                                       bufs=1, name=f"d32_{hl}_{cp}")
                    nc.sync.dma_start(d32[:], den2[:])
                    dr = pool_dv.tile([32, 2 * CH // 32], F16, tag="dr",
                                      bufs=1, name=f"dr{hl}_{cp}")
                    with nc.allow_low_precision(reason="fp16 dinv; x4096 scaling keeps it normal"):
                        nc.vector.reciprocal(dr[:], d32[:])
                    dd_t = pool_dvr.tile([1, 2 * CH], F16, tag="dvrow",
                                         name=f"dinv_dram{hl}_{cp}")
                    nc.sync.dma_start(dd_t[:], dr[:])
                    bch = pool_bch.tile([128, 2 * CH], F16, tag="bch", bufs=2,
                                        name=f"bch{hl}_{cp}")
                    nc.sync.dma_start(bch[:], dd_t[:].to_broadcast((128, 2 * CH)))
                    state["bch"] = bch

                def stage_c():
                    for hh in range(2):
                        lo = cp * 2 * CH + hh * CH
                        nc.gpsimd.tensor_mul(
                            at_t[hl][:, lo:lo + CH],
                            at_t[hl][:, lo:lo + CH],
                            state["bch"][:, hh * CH:(hh + 1) * CH],
                        )

                return [stage_b, stage_c]

            def fin_boundary():
                for chain in fin_chains:
                    chain.pop(0)()
                fin_chains[:] = [ch for ch in fin_chains if ch]

            # Phase-3 output-projection emitters. The first token half
            # (it < 8) only needs the cp0 attention sweep, so those groups
            # are emitted INTO the cp1 sweep's PE stream (the PE queue runs
            # in emission order - work emitted later cannot fill earlier
            # stalls). Interleaved groups use their own 1-bank psum tag so
            # they never WAR against the still-accumulating attention psum.
            os_map = {}

            def p3_group(it, eh, e2, tag, bufs):
                def emit():
                    key = (it, eh)
                    if key not in os_map:
                        os_map[key] = pool_o.tile(
                            [128, E // 2], BF, tag="o", bufs=2,
                            name=f"os{it}_{eh}")
                    os_t = os_map[key]
                    po = pool_ps.tile([128, CH], F32, tag=tag, bufs=bufs,
                                      name=f"po{it}_{eh}_{e2}")
                    for fb in range(HL):
                        nc.tensor.matmul(
                            po[:],
                            at_t[fb][:, it * 128:(it + 1) * 128],
                            wo_t[eh][:, fb * 2 + e2, :],
                            start=(fb == 0),
                            stop=(fb == HL - 1),
                        )
                    nc.vector.tensor_copy(
                        os_t[:, e2 * CH:(e2 + 1) * CH], po[:]
                    )
                    if e2 == 1:
                        nc.sync.dma_start(
                            out_d[it * 128:(it + 1) * 128,
                                  eh * HALFT:(eh + 1) * HALFT],
                            os_t[:],
                        )
                return emit

            p3_queue = [(it, eh, e2)
                        for it in range(T // 256)
                        for eh in range(2)
                        for e2 in range(2)]
            p3_budget = [24]   # interleaved groups; 8 reserved for the tail

            def p3_slot():
                if p3_budget[0] > 0 and p3_queue:
                    p3_budget[0] -= 1
                    p3_group(*p3_queue.pop(0), tag="pso", bufs=1)()

            chunk_order = [(hl, cp * 2 + ci)
                           for cp in range(NCH // 2)
                           for hl in range(HL)
                           for ci in range(2)]
            for ci_idx, (hl, c) in enumerate(chunk_order):
                kf = HL + hl // REP
                kvc = (hl // REP) * HD
                njt = (c + 1) * (CH // 128)
                if c % 2 == 0:
                    den2 = pool_dv.tile([1, 2 * CH], F32, tag="den2",
                                        bufs=2, name=f"den2_{hl}_{c // 2}")
                    den2_map[hl] = den2
                else:
                    den2 = den2_map[hl]
                acc = pool_ps.tile([128, CH], F32, tag="psacc", bufs=2,
                                   name=f"acc{hl}_{c}")
                den = pool_ps.tile([128, CH], F32, tag="psden", bufs=1,
                                   name=f"den{hl}_{c}")
                j0_order = list(range(0, njt, 2))
                start_jt = 0
                stop_jt = njt - 1
                for step, j0 in enumerate(j0_order):
                    # causally-live column start per j-tile: diagonal
                    # tiles (d >= 0) only need cols [128*d, 512)
                    i0s = []
                    for u in range(2):
                        d = (j0 + u) - (njt - 4)
                        i0s.append(128 * d if d > 0 else 0)
                    diag = (j0 >= njt - 4)
                    s2 = pool_ps.tile([128, 2, CH], F32, tag="ps", bufs=2,
                                      name=f"s2_{hl}_{c}_{j0}")
                    for u in range(2):
                        jt = j0 + u
                        masked = (jt >= njt - 4)
                        nc.tensor.matmul(
                            s2[:, u, i0s[u]:],
                            qk_t[kf][:, jt * 128:(jt + 1) * 128],
                            qk_t[hl][:, c * CH + i0s[u]:(c + 1) * CH],
                            start=True,
                            stop=not masked,
                        )
                        if masked:
                            nc.tensor.matmul(
                                s2[:, u, i0s[u]:i0s[u] + 128],
                                ident_t[:],
                                tri_t[:],
                                start=False,
                                stop=True,
                            )
                    p2 = pool_p.tile([128, 2, CH], BF, tag="p", bufs=4,
                                     name=f"p2_{hl}_{c}_{j0}")
                    # one activation per step; for diagonal pairs the
                    # region [i0s[0], CH) covers both u-slices (u=1's
                    # cols [i0s[0], i0s[1]) hold unread garbage)
                    nc.scalar.activation(
                        p2[:, :, i0s[0]:], s2[:, :, i0s[0]:],
                        mybir.ActivationFunctionType.Exp,
                        scale=INV_SQRT_D,
                    )
                    psum2 = None
                    quad = None
                    den_mm = None   # (tile, first) queued for this step
                    if diag:
                        pass    # causal mask already folded into the scores
                    else:
                        # pre-sum the probs pair on DVE, then merge step
                        # pairs into quads so the den ones-matmul streams a
                        # quarter of the rows
                        psum2 = pool_sp.tile([128, CH], BF, tag="sp",
                                             name=f"sp{hl}_{c}_{j0}")
                        nc.vector.tensor_add(
                            psum2[:], p2[:, 0, :], p2[:, 1, :]
                        )
                        if step % 2 == 0:
                            prev_psum2 = psum2
                        else:
                            quad = pool_sp.tile([128, CH], BF, tag="qd",
                                                bufs=2,
                                                name=f"qd{hl}_{c}_{j0}")
                            nc.vector.tensor_add(
                                quad[:], prev_psum2[:], psum2[:]
                            )
                            if c == 1:
                                den_mm = (quad, True)
                            elif step == 1:
                                prev_quad = quad
                            elif step == 3:
                                oct8 = pool_sp.tile(
                                    [128, CH], BF, tag="oc8", bufs=2,
                                    name=f"oc8_{hl}_{c}")
                                nc.vector.tensor_add(
                                    oct8[:], prev_quad[:], quad[:]
                                )
                                den_mm = (oct8, True)
                            else:
                                den_mm = (quad, False)
                    if len(pend_q) >= 3:
                        for fn in pend_q.pop(0):
                            fn()
                    if step == 2:
                        fin_boundary()
                    # feed first-half output-projection groups into the cp1
                    # sweep (at_t tokens 0-1023 are final for all heads two
                    # boundaries into the sweep)
                    if ci_idx >= 18 and step == 3:
                        p3_slot()
                    step_fns = []
                    for u in range(2):
                        jt = j0 + u
                        def av(jt=jt, p2=p2, u=u, acc=acc, kvc=kvc,
                               i0=i0s[u], sjt=start_jt, pjt=stop_jt):
                            nc.tensor.matmul(
                                acc[:, i0:],
                                v_t[jt][:, kvc:kvc + HD],
                                p2[:, u, i0:],
                                start=(jt == sjt),
                                stop=(jt == pjt),
                            )
                        step_fns.append(av)
                    if not diag:
                        if den_mm is not None:
                            def den_quad(src_t=den_mm[0], den=den,
                                         first=den_mm[1]):
                                nc.tensor.matmul(
                                    den[:],
                                    ones_t[:, 0:128],
                                    src_t[:],
                                    start=first,
                                    stop=False,
                                )
                            step_fns.append(den_quad)
                    else:
                        for u in range(2):
                            jt = j0 + u
                            def den_u(jt=jt, p2=p2, u=u, den=den,
                                      i0=i0s[u],
                                      sjt=start_jt, pjt=stop_jt):
                                nc.tensor.matmul(
                                    den[:, i0:],
                                    ones_t[:, 0:128],
                                    p2[:, u, i0:],
                                    start=(jt == sjt),
                                    stop=(jt == pjt),
                                )
                            step_fns.append(den_u)
                    pend_q.append(step_fns)
                fin_chains.append(make_fin_a(hl, c, acc, den, den2))
                if c % 2 == 1:
                    fin_chains.append(make_fin_bc(hl, c // 2, den2))
                if ci_idx >= 17:
                    p3_slot()
            while pend_q:
                for fn in pend_q.pop(0):
                    fn()
            # ready first-half output tiles keep the PE fed while the
            # final AV/den/normalize chains drain
            for _ in range(3):
                if p3_queue:
                    p3_group(*p3_queue.pop(0), tag="pso", bufs=1)()
            while fin_chains:
                fin_boundary()
                if p3_queue:
                    p3_group(*p3_queue.pop(0), tag="psacc", bufs=2)()

            # ================= Phase 3 tail: remaining output tiles ========
            while p3_queue:
                p3_group(*p3_queue.pop(0), tag="psacc", bufs=2)()
            for it in range(T // 256, T // 128):
                for eh in range(2):
                    for e2 in range(2):
                        p3_group(it, eh, e2, tag="psacc", bufs=2)()

    nc.compile()
    return nc


def _get_compiled():
    global _COMPILED
    if _COMPILED is None:
        _COMPILED = _build_nc()
    return _COMPILED


def _host_tables():
    half = np.arange(0, HD, 2, dtype=np.float64)
    inv_freq = 1.0 / (THETA ** (half / HD))
    t_idx = np.arange(T, dtype=np.float64)
    freqs = np.outer(t_idx, inv_freq)
    emb = np.concatenate([freqs, freqs], axis=-1)
    cos, sin = np.cos(emb), np.sin(emb)
    scale_vec = (half + 0.4 * HD) / (1.4 * HD)
    power = (t_idx - T // 2) / SCALE_BASE
    scale = scale_vec[None, :] ** power[:, None]
    scale = np.concatenate([scale, scale], axis=-1)
    sgn = np.where(np.arange(HD) < HD // 2, -1.0, 1.0)
    aq = (scale * cos).T
    bq = sgn[:, None] * (scale * sin).T
    ak = (cos / scale).T
    bk = sgn[:, None] * (sin / scale).T

    # within-tile causal mask, additive: -1e9 where j > i (applied to the
    # scores via an identity-stationary matmul before exp)
    dj = np.arange(128)[:, None]
    r = np.arange(128)[None, :]
    tri = np.where(dj > r, -1e9, 0.0)
    ident = np.eye(128)
    return (
        aq.astype(BF16), bq.astype(BF16), ak.astype(BF16), bk.astype(BF16),
        tri.astype(BF16), ident.astype(BF16),
    )


def _arrange_wqk(wq, wk):
    # [E, F] -> per 128-wide f-block: [128(part=e%128), NE(e//128), 128(f)]
    w = np.concatenate([wq, wk], axis=1)          # [E, NF*128]
    nf = w.shape[1] // 128
    w = w.reshape(NE, 128, nf, 128)               # [n, p, f_blk, fc]
    w = w.transpose(2, 1, 0, 3)                   # [f_blk, p, n, fc]
    return np.ascontiguousarray(w).astype(BF16)


def _arrange_xt(xt):
    # [E, T] -> [half, p, e, chunk, i]
    w = xt.reshape(NE, 128, 2, 2, CH)             # [e, p, half, cc, i]
    w = w.transpose(2, 1, 0, 3, 4)                # [half, p, e, cc, i]
    return np.ascontiguousarray(w).astype(BF16)


def _arrange_wv(wv):
    # [E, KVL*HD] -> [p, e, f]
    w = wv.reshape(NE, 128, KVL * HD)
    w = w.transpose(1, 0, 2)
    return np.ascontiguousarray(w).astype(BF16)


def _arrange_wo(wo):
    # [HL*HD, E] -> [Ehalf, p, fb*2+e2, i] so (fb, ec) slice is
    # [eh=ec//2][:, fb*2 + ec%2, :]
    w = wo.reshape(HL, 128, 2, 2, CH)             # [fb, p, eh, e2, i]
    w = w.transpose(2, 1, 0, 3, 4)                # [eh, p, fb, e2, i]
    w = w.reshape(2, 128, NE, CH)
    return np.ascontiguousarray(w).astype(BF16)


def _make_in_maps(x, W_q, W_k, W_v, W_o):
    aq, bq, ak, bk, tri, ident = _host_tables()
    xts = [_arrange_xt(np.ascontiguousarray(x[b].T)) for b in range(B)]
    in_maps = []
    for core in range(8):
        b, g = core // G, core % G
        in_maps.append({
            "xt": xts[b],
            "wqk": _arrange_wqk(W_q[:, g * HL * HD:(g + 1) * HL * HD],
                                W_k[:, g * KVL * HD:(g + 1) * KVL * HD]),
            "wv": _arrange_wv(W_v[:, g * KVL * HD:(g + 1) * KVL * HD]),
            "wo": _arrange_wo(W_o[g * HL * HD:(g + 1) * HL * HD, :] / 4096.0),
            "aq": aq, "bq": bq, "ak": ak, "bk": bk,
            "tri": tri,
            "ident": ident,
        })
    return in_maps


def _run(x, W_q, W_k, W_v, W_o, trace=False):
    nc = _get_compiled()
    in_maps = _make_in_maps(x, W_q, W_k, W_v, W_o)
    res = run_bass_kernel_spmd(nc, in_maps, list(range(8)), trace=trace)
    out = np.empty((B, T, E), np.float32)
    for b in range(B):
        out[b] = (res.results[2 * b]["out_p"].astype(np.float32)
                  + res.results[2 * b + 1]["out_p"].astype(np.float32))
    return out, res.exec_time_ns


def kernel(x, W_q, W_k, W_v, W_o):
    out, _ = _run(
        np.asarray(x), np.asarray(W_q), np.asarray(W_k),
        np.asarray(W_v), np.asarray(W_o),
    )
    return out


# revision 32
# speedup vs baseline: 1.2146x; 1.0028x over previous
"""Trainium2 Bass kernel for nn_MultiHeadSelfAttention_11158325035343.

GQA multi-head self-attention (B=4, T=2048, E=2048, H=16, HKV=8, HD=128)
with XPos rotary embedding and causal softmax.

Sharding: 8 cores = 4 batches x 2 head-groups. Each core computes, for its
batch b and head-group g (8 q heads, 4 kv heads):
  QT/KT = W.T @ x.T   ([head_dim, T] per head, head_dim on partitions)
  V     = x @ W_v     ([T, head_dim] per kv head)
  XPos rope applied via two host-precomputed fused tables + half-swap
  scoresT[j, i] per (head, i-chunk, j-tile), exp without max subtraction
  (scores are bounded: XPos decay keeps them small), causal mask applied
  post-exp: diagonal j-tiles narrow their score/exp/AV/den work to the
  causally-live columns and a [128,128] triangular 0/1 multiply on GpSimd
  zeroes the within-tile j>i region, softmax denominator via ones-matmul
  on PE over DVE-presummed prob pairs, AV/den matmuls deferred two steps
  behind the scores so the PE always has independent work while ACT runs
  exp, attnT = V.T-contraction with probs as moving operand, normalized
  by the broadcast reciprocal denominator (fin chain staged across chunk
  boundaries), partial out = attnT.T @ W_o rows-for-this-group, written
  as bf16 partials.
Host sums the two group partials per batch in f32.

Scheduling structure:
  - PE warmed up via memset-ones matmuls at t=0 (HAM clock-gate release)
  - x/W_v/W_o staged via single large prearranged DMAs (sequencer issue
    cost is ~600ns per dma_start, so fewer+bigger wins)
  - phase 2 runs chunk-pair-major (all heads' token pair 0 first): after
    ~26% of attention work the first half of phase 3 unlocks, giving the
    scheduler PE filler for the ACT-bound remainder of attention
"""

import sys
import types

sys.path.insert(0, "/opt/trn_rl_repo")

import numpy as np
import ml_dtypes

BF16 = ml_dtypes.bfloat16

# ---------------------------------------------------------------------------
# NTFF profile hook injection (missing antenv.axon_hooks in this image).
# Needed only when trace=True; harmless otherwise.
# ---------------------------------------------------------------------------
def _ensure_axon_hooks():
    if "antenv.axon_hooks" in sys.modules:
        return
    try:
        import antenv
        mod = types.ModuleType("antenv.axon_hooks")
        holder = {"hook": None}
        mod.set_axon_ntff_profile_hook = lambda h: holder.__setitem__("hook", h)
        mod.get_axon_ntff_profile_hook = lambda: holder["hook"]
        sys.modules["antenv.axon_hooks"] = mod
        antenv.axon_hooks = mod
        from trn_agent_boot.trn_boot import _ntff_profile_via_ctypes
        mod.set_axon_ntff_profile_hook(
            _ntff_profile_via_ctypes("/opt/axon/libaxon_pjrt.so")
        )
    except Exception:
        pass


_ensure_axon_hooks()

import concourse.bass as bass
import concourse.bacc as bacc
import concourse.mybir as mybir
import concourse.tile as tile
from concourse.bass_utils import run_bass_kernel_spmd

# Problem constants (hardcoded per spec).
B, T, E = 4, 2048, 2048
H, HKV, HD = 16, 8, 128
THETA, SCALE_BASE = 10000.0, 512.0
G = 2                   # head groups (cores per batch)
HL = H // G             # 8 local q heads
KVL = HKV // G          # 4 local kv heads
REP = H // HKV          # GQA repeat
CH = 512                # i-chunk / matmul free dim
NE = E // 128           # 16 contraction tiles
NF = HL + KVL           # 12 projection f-tiles (8 Q + 4 K)
HALFT = T // 2          # token half for phase-1 SBUF staging
NJT = T // 128          # 16 j tiles
NCH = T // CH           # 4 i chunks
INV_SQRT_D = 1.0 / float(np.sqrt(np.float32(HD)))

F32 = mybir.dt.float32
F16 = mybir.dt.float16
BF = mybir.dt.bfloat16

_COMPILED = None


def _build_nc():
    nc = bacc.Bacc("TRN2", target_bir_lowering=False, debug=False, num_devices=8)

    # xt prearranged on host: [half, p, e, chunk, i]
    xt_d = nc.dram_tensor("xt", [2, 128, NE, 2, CH], BF, kind="ExternalInput")
    wqk_d = nc.dram_tensor("wqk", [NF, 128, NE, 128], BF, kind="ExternalInput")
    # wv prearranged: [p, e, f]
    wv_d = nc.dram_tensor("wv", [128, NE, KVL * HD], BF, kind="ExternalInput")
    # wo prearranged: [Ehalf, p, fb*2+e2, i]
    wo_d = nc.dram_tensor("wo", [2, 128, NE, CH], BF, kind="ExternalInput")
    aq_d = nc.dram_tensor("aq", [HD, T], BF, kind="ExternalInput")
    bq_d = nc.dram_tensor("bq", [HD, T], BF, kind="ExternalInput")
    ak_d = nc.dram_tensor("ak", [HD, T], BF, kind="ExternalInput")
    bk_d = nc.dram_tensor("bk", [HD, T], BF, kind="ExternalInput")
    tri_d = nc.dram_tensor("tri", [128, 128], BF, kind="ExternalInput")
    ident_d = nc.dram_tensor("ident", [128, 128], BF, kind="ExternalInput")
    out_d = nc.dram_tensor("out_p", [T, E], BF, kind="ExternalOutput")

    with tile.TileContext(nc) as tc:
        with (
            tc.tile_pool(name="big", bufs=2) as pool_big,       # xt chunks / wo
            tc.tile_pool(name="qk", bufs=NF) as pool_qk,        # rope'd QT/KT bf16
            tc.tile_pool(name="v", bufs=NJT) as pool_v,         # V bf16
            tc.tile_pool(name="at", bufs=HL) as pool_at,        # attnT bf16
            tc.tile_pool(name="tab", bufs=4) as pool_tab,       # rope tables
            tc.tile_pool(name="wv", bufs=1) as pool_wv,         # resident W_v
            tc.tile_pool(name="w", bufs=3) as pool_w,           # streamed W_q/W_k
            tc.tile_pool(name="tmp", bufs=2) as pool_tmp,       # rope temp
            tc.tile_pool(name="p", bufs=4) as pool_p,           # exp probs bf16
            tc.tile_pool(name="sp", bufs=2) as pool_sp,         # den pair presums
            tc.tile_pool(name="o", bufs=2) as pool_o,           # out staging
            tc.tile_pool(name="sm", bufs=1) as pool_sm,         # small constants
            tc.tile_pool(name="dv", bufs=4) as pool_dv,         # recip denominators
            tc.tile_pool(name="dvr", bufs=4, space="DRAM") as pool_dvr,  # dinv DRAM bounce
            tc.tile_pool(name="bch", bufs=3) as pool_bch,       # dinv bcast per chunk
            tc.tile_pool(name="ps", bufs=2, space=bass.MemorySpace.PSUM) as pool_ps,
        ):
            # ---- PE warmup: all-ones tile via memset (no DMA dependency)
            # so the HAM clock gate releases at ~3.4us, before the input
            # DMA wave completes. The same tile serves the den ones-matmul.
            ones_t = pool_sm.tile([128, CH], BF, tag="oc", name="ones_t")
            nc.vector.memset(ones_t[:], 1.0)
            warm_ps = pool_ps.tile([128, CH], F32, tag="psden", bufs=1,
                                   name="warm_ps")
            for _wi in range(38):
                nc.tensor.matmul(warm_ps[:], ones_t[:, 0:128], ones_t[:],
                                 start=True, stop=True)

            # tiny mask constants on the gpsimd queue (needed in phase 2)
            tri_t = pool_sm.tile([128, 128], BF, tag="tri", name="tri_t")
            nc.gpsimd.dma_start(tri_t[:], tri_d[:])
            ident_t = pool_sm.tile([128, 128], BF, tag="id", name="ident_t")
            nc.gpsimd.dma_start(ident_t[:], ident_d[:])

            # ---- persistent activation tensors ----
            qk_t = [pool_qk.tile([128, T], BF, tag="qk", name=f"qk{i}") for i in range(NF)]
            v_t = [pool_v.tile([128, KVL * HD], BF, tag="v", name=f"v{i}") for i in range(NJT)]
            at_t = [pool_at.tile([128, T], BF, tag="at", name=f"at{i}") for i in range(HL)]

            # ================= Phase 2: attention ==========================
            # Chunk-pair-major over heads: all heads' token pair 0 (chunks
            # 0-1) first, then pair 1 (chunks 2-3). Tokens 0-1023 of every
            # head finish after the first sweep, unblocking the first half
            # of phase 3 as PE filler while the rest of attention (which is
            # ACT-exp-bound per chunk) runs. AV/den matmuls of each j-block
            # are deferred TWO steps so the PE always has independent work
            # while ACT runs exp. Per-chunk normalization chains are staged
            # one boundary later per stage.
            pend_q = []     # deferred AV/den emitters, one list per j-block
            fin_chains = []  # normalization chains, one stage/boundary
            den2_map = {}

            def make_fin_a(hl, c, acc, den, den2):
                # per-chunk psum evacuation: acc -> at_t, den row -> its half
                # of the pair's den2 buffer (x 1/4096 for the fp16 recip)
                def stage_a():
                    nc.vector.tensor_copy(
                        at_t[hl][:, c * CH:(c + 1) * CH], acc[:]
                    )
                    nc.vector.tensor_scalar_mul(
                        den2[0:1, (c % 2) * CH:(c % 2 + 1) * CH],
                        den[0:1, :], 1.0 / 4096.0,
                    )
                return [stage_a]

            def make_fin_bc(hl, cp, den2):
                # per chunk-PAIR: reciprocal + broadcast + normalize over a
                # [128, 2*CH] region; half the DMA-descriptor bursts of the
                # per-chunk variant
                state = {}

                def stage_b():
                    d32 = pool_dv.tile([32, 2 * CH // 32], F32, tag="d32",
                                       bufs=1, name=f"d32_{hl}_{cp}")
                    nc.sync.dma_start(d32[:], den2[:])
                    dr = pool_dv.tile([32, 2 * CH // 32], F16, tag="dr",
                                      bufs=1, name=f"dr{hl}_{cp}")
                    with nc.allow_low_precision(reason="fp16 dinv; x4096 scaling keeps it normal"):
                        nc.vector.reciprocal(dr[:], d32[:])
                    dd_t = pool_dvr.tile([1, 2 * CH], F16, tag="dvrow",
                                         name=f"dinv_dram{hl}_{cp}")
                    nc.sync.dma_start(dd_t[:], dr[:])
                    bch = pool_bch.tile([128, 2 * CH], F16, tag="bch", bufs=2,
                                        name=f"bch{hl}_{cp}")
                    nc.sync.dma_start(bch[:], dd_t[:].to_broadcast((128, 2 * CH)))
                    state["bch"] = bch

                def stage_c():
                    for hh in range(2):
                        lo = cp * 2 * CH + hh * CH
                        nc.gpsimd.tensor_mul(
                            at_t[hl][:, lo:lo + CH],
                            at_t[hl][:, lo:lo + CH],
                            state["bch"][:, hh * CH:(hh + 1) * CH],
                        )

                return [stage_b, stage_c]

            def fin_boundary():
                for chain in fin_chains:
                    chain.pop(0)()
                fin_chains[:] = [ch for ch in fin_chains if ch]

            # Phase-3 output-projection emitters. The first token half
            # (it < 8) only needs the cp0 attention sweep, so those groups
            # are emitted INTO the cp1 sweep's PE stream (the PE queue runs
            # in emission order - work emitted later cannot fill earlier
            # stalls). Interleaved groups use their own 1-bank psum tag so
            # they never WAR against the still-accumulating attention psum.
            os_map = {}

            def p3_group(it, eh, e2, tag, bufs):
                def emit():
                    key = (it, eh)
                    if key not in os_map:
                        os_map[key] = pool_o.tile(
                            [128, E // 2], BF, tag="o", bufs=2,
                            name=f"os{it}_{eh}")
                    os_t = os_map[key]
                    po = pool_ps.tile([128, CH], F32, tag=tag, bufs=bufs,
                                      name=f"po{it}_{eh}_{e2}")
                    for fb in range(HL):
                        nc.tensor.matmul(
                            po[:],
                            at_t[fb][:, it * 128:(it + 1) * 128],
                            wo_t[eh][:, fb * 2 + e2, :],
                            start=(fb == 0),
                            stop=(fb == HL - 1),
                        )
                    nc.vector.tensor_copy(
                        os_t[:, e2 * CH:(e2 + 1) * CH], po[:]
                    )
                    if e2 == 1:
                        nc.sync.dma_start(
                            out_d[it * 128:(it + 1) * 128,
                                  eh * HALFT:(eh + 1) * HALFT],
                            os_t[:],
                        )
                return emit

            p3_queue = [(it, eh, e2)
                        for it in range(T // 256)
                        for eh in range(2)
                        for e2 in range(2)]
            p3_budget = [24]   # interleaved groups; 8 reserved for the tail

            def p3_slot():
                if p3_budget[0] > 0 and p3_queue:
                    p3_budget[0] -= 1
                    p3_group(*p3_queue.pop(0), tag="pso", bufs=1)()

            chunk_order = [(hl, cp * 2 + ci)
                           for cp in range(NCH // 2)
                           for hl in range(HL)
                           for ci in range(2)]
            chunk_pos = [0]

            def emit_chunk():
                ci_idx = chunk_pos[0]
                if ci_idx >= len(chunk_order):
                    return
                chunk_pos[0] += 1
                hl, c = chunk_order[ci_idx]
                kf = HL + hl // REP
                kvc = (hl // REP) * HD
                njt = (c + 1) * (CH // 128)
                if c % 2 == 0:
                    den2 = pool_dv.tile([1, 2 * CH], F32, tag="den2",
                                        bufs=2, name=f"den2_{hl}_{c // 2}")
                    den2_map[hl] = den2
                else:
                    den2 = den2_map[hl]
                if c < 2:
                    acc = pool_ps.tile([128, CH], F32, tag="pso", bufs=1,
                                       name=f"acc{hl}_{c}")
                else:
                    acc = pool_ps.tile([128, CH], F32, tag="psacc", bufs=2,
                                       name=f"acc{hl}_{c}")
                den = pool_ps.tile([128, CH], F32, tag="psden", bufs=1,
                                   name=f"den{hl}_{c}")
                j0_order = list(range(0, njt, 2))
                start_jt = 0
                stop_jt = njt - 1
                for step, j0 in enumerate(j0_order):
                    # causally-live column start per j-tile: diagonal
                    # tiles (d >= 0) only need cols [128*d, 512)
                    i0s = []
                    for u in range(2):
                        d = (j0 + u) - (njt - 4)
                        i0s.append(128 * d if d > 0 else 0)
                    diag = (j0 >= njt - 4)
                    s2 = pool_ps.tile([128, 2, CH], F32, tag="ps", bufs=2,
                                      name=f"s2_{hl}_{c}_{j0}")
                    for u in range(2):
                        jt = j0 + u
                        masked = (jt >= njt - 4)
                        nc.tensor.matmul(
                            s2[:, u, i0s[u]:],
                            qk_t[kf][:, jt * 128:(jt + 1) * 128],
                            qk_t[hl][:, c * CH + i0s[u]:(c + 1) * CH],
                            start=True,
                            stop=not masked,
                        )
                        if masked:
                            nc.tensor.matmul(
                                s2[:, u, i0s[u]:i0s[u] + 128],
                                ident_t[:],
                                tri_t[:],
                                start=False,
                                stop=True,
                            )
                    p2 = pool_p.tile([128, 2, CH], BF, tag="p", bufs=4,
                                     name=f"p2_{hl}_{c}_{j0}")
                    # one activation per step; for diagonal pairs the
                    # region [i0s[0], CH) covers both u-slices (u=1's
                    # cols [i0s[0], i0s[1]) hold unread garbage)
                    nc.scalar.activation(
                        p2[:, :, i0s[0]:], s2[:, :, i0s[0]:],
                        mybir.ActivationFunctionType.Exp,
                        scale=INV_SQRT_D,
                    )
                    psum2 = None
                    quad = None
                    den_mm = None   # (tile, first) queued for this step
                    if diag:
                        pass    # causal mask already folded into the scores
                    else:
                        # pre-sum the probs pair on DVE, then merge step
                        # pairs into quads so the den ones-matmul streams a
                        # quarter of the rows
                        psum2 = pool_sp.tile([128, CH], BF, tag="sp",
                                             name=f"sp{hl}_{c}_{j0}")
                        nc.vector.tensor_add(
                            psum2[:], p2[:, 0, :], p2[:, 1, :]
                        )
                        if step % 2 == 0:
                            prev_psum2 = psum2
                        else:
                            quad = pool_sp.tile([128, CH], BF, tag="qd",
                                                bufs=2,
                                                name=f"qd{hl}_{c}_{j0}")
                            nc.vector.tensor_add(
                                quad[:], prev_psum2[:], psum2[:]
                            )
                            if c == 1:
                                den_mm = (quad, True)
                            elif step == 1:
                                prev_quad = quad
                            elif step == 3:
                                oct8 = pool_sp.tile(
                                    [128, CH], BF, tag="oc8", bufs=2,
                                    name=f"oc8_{hl}_{c}")
                                nc.vector.tensor_add(
                                    oct8[:], prev_quad[:], quad[:]
                                )
                                den_mm = (oct8, True)
                            else:
                                den_mm = (quad, False)
                    if len(pend_q) >= 3:
                        for fn in pend_q.pop(0):
                            fn()
                    if step == 2:
                        fin_boundary()
                    # feed first-half output-projection groups into the cp1
                    # sweep (at_t tokens 0-1023 are final for all heads two
                    # boundaries into the sweep)
                    if ci_idx >= 18 and step == 3:
                        p3_slot()
                    step_fns = []
                    for u in range(2):
                        jt = j0 + u
                        def av(jt=jt, p2=p2, u=u, acc=acc, kvc=kvc,
                               i0=i0s[u], sjt=start_jt, pjt=stop_jt):
                            nc.tensor.matmul(
                                acc[:, i0:],
                                v_t[jt][:, kvc:kvc + HD],
                                p2[:, u, i0:],
                                start=(jt == sjt),
                                stop=(jt == pjt),
                            )
                        step_fns.append(av)
                    if not diag:
                        if den_mm is not None:
                            def den_quad(src_t=den_mm[0], den=den,
                                         first=den_mm[1]):
                                nc.tensor.matmul(
                                    den[:],
                                    ones_t[:, 0:128],
                                    src_t[:],
                                    start=first,
                                    stop=False,
                                )
                            step_fns.append(den_quad)
                    else:
                        for u in range(2):
                            jt = j0 + u
                            def den_u(jt=jt, p2=p2, u=u, den=den,
                                      i0=i0s[u],
                                      sjt=start_jt, pjt=stop_jt):
                                nc.tensor.matmul(
                                    den[:, i0:],
                                    ones_t[:, 0:128],
                                    p2[:, u, i0:],
                                    start=(jt == sjt),
                                    stop=(jt == pjt),
                                )
                            step_fns.append(den_u)
                    pend_q.append(step_fns)
                fin_chains.append(make_fin_a(hl, c, acc, den, den2))
                if c % 2 == 1:
                    fin_chains.append(make_fin_bc(hl, c // 2, den2))
                if ci_idx >= 17:
                    p3_slot()
            # ================= Phase 1: QKV projections + rope =============
            # DMA priority on the sync queue: w0, then the chunk-0 x wave
            # (exactly what the first f-tile's first matmuls need), then
            # chunk 1, then rope tables (K first - K heads rope first),
            # then W_v. x/W_v go as single large prearranged transfers to
            # keep the sequencer issue count low.
            for half in range(2):
                hs = half * HALFT
                wq_pre = []

                def w_prefetch(f, half=half):
                    w = pool_w.tile([128, NE, 128], BF, tag="w",
                                    name=f"w_pre{half}_{f}")
                    nc.sync.dma_start(w[:], wqk_d[f])
                    wq_pre.append(w)

                forder = list(range(HL, NF)) + list(range(HL))
                w_prefetch(forder[0])
                xt_t = []
                for cc in range(2):
                    xx = pool_big.tile([128, NE, CH], BF, tag="big",
                                       name=f"xt{half}_{cc}")
                    nc.sync.dma_start(xx[:], xt_d[half, :, :, cc, :])
                    xt_t.append(xx)
                w_prefetch(forder[1])
                w_prefetch(forder[2])
                if half == 0:
                    ak_t = pool_tab.tile([HD, T], BF, tag="tab", name="ak_t")
                    nc.sync.dma_start(ak_t[:], ak_d[:])
                    bk_t = pool_tab.tile([HD, T], BF, tag="tab", name="bk_t")
                    nc.sync.dma_start(bk_t[:], bk_d[:])
                    aq_t = pool_tab.tile([HD, T], BF, tag="tab", name="aq_t")
                    nc.sync.dma_start(aq_t[:], aq_d[:])
                    bq_t = pool_tab.tile([HD, T], BF, tag="tab", name="bq_t")
                    nc.sync.dma_start(bq_t[:], bq_d[:])
                    wv_t = pool_wv.tile([128, NE, KVL * HD], BF, tag="wv",
                                        name="wv_t")
                    nc.sync.dma_start(wv_t[:], wv_d[:])

                for fi, f in enumerate(forder):
                    # host-prearranged W column block, contiguous per partition
                    w_t = wq_pre.pop(0)
                    if fi + 3 < NF:
                        w_prefetch(forder[fi + 3])
                    for c in range(HALFT // CH):
                        ps = pool_ps.tile([128, CH], F32, tag="psacc", bufs=2)
                        for e in range(NE):
                            nc.tensor.matmul(
                                ps[:],
                                w_t[:, e, :],
                                xt_t[c][:, e, :],
                                start=(e == 0),
                                stop=(e == NE - 1),
                            )
                        nc.vector.tensor_copy(
                            qk_t[f][:, hs + c * CH: hs + (c + 1) * CH], ps[:]
                        )
                    # rope over this token half
                    A_t, B_t = (aq_t, bq_t) if f < HL else (ak_t, bk_t)
                    q = qk_t[f]
                    sl = slice(hs, hs + HALFT)
                    qs = pool_tmp.tile([128, HALFT], BF, tag="qs")
                    nc.sync.dma_start(qs[0:64, :], q[64:128, sl])
                    nc.sync.dma_start(qs[64:128, :], q[0:64, sl])
                    nc.vector.tensor_mul(qs[:, :], qs[:, :], B_t[:, sl])
                    nc.vector.tensor_mul(q[:, sl], q[:, sl], A_t[:, sl])
                    nc.vector.tensor_add(q[:, sl], q[:, sl], qs[:])
                    if half == 1 and chunk_pos[0] < 16:
                        emit_chunk()

                for tt in range(NJT // 2):
                    tglob = half * (NJT // 2) + tt
                    cc, co = tt // 4, (tt % 4) * 128
                    psv = pool_ps.tile([128, KVL * HD], F32, tag="psacc", bufs=2)
                    for e in range(NE):
                        nc.tensor.matmul(
                            psv[:],
                            xt_t[cc][:, e, co:co + 128],
                            wv_t[:, e, :],
                            start=(e == 0),
                            stop=(e == NE - 1),
                        )
                    nc.vector.tensor_copy(v_t[tglob][:], psv[:])
                    if half == 1 and chunk_pos[0] < 16:
                        emit_chunk()

            # W_o loads reuse the xt big-tile ring (freed after phase 1):
            # two [128, NE, CH] tiles, mapping [p, fb*2+e2, i] so phase 3's
            # (fb, ec) slice is wo_t[ec//2][:, fb*2 + ec%2, :]
            wo_t = []
            for eh in range(2):
                w = pool_big.tile([128, NE, CH], BF, tag="big",
                                  name=f"wo{eh}")
                nc.sync.dma_start(w[:], wo_d[eh])
                wo_t.append(w)

            while chunk_pos[0] < len(chunk_order):
                emit_chunk()
            while pend_q:
                for fn in pend_q.pop(0):
                    fn()
            # ready first-half output tiles keep the PE fed while the
            # final AV/den/normalize chains drain
            for _ in range(3):
                if p3_queue:
                    p3_group(*p3_queue.pop(0), tag="pso", bufs=1)()
            while fin_chains:
                fin_boundary()
                if p3_queue:
                    p3_group(*p3_queue.pop(0), tag="psacc", bufs=2)()

            # ================= Phase 3 tail: remaining output tiles ========
            while p3_queue:
                p3_group(*p3_queue.pop(0), tag="psacc", bufs=2)()
            for it in range(T // 256, T // 128):
                for eh in range(2):
                    for e2 in range(2):
                        p3_group(it, eh, e2, tag="psacc", bufs=2)()

    nc.compile()
    return nc


def _get_compiled():
    global _COMPILED
    if _COMPILED is None:
        _COMPILED = _build_nc()
    return _COMPILED


def _host_tables():
    half = np.arange(0, HD, 2, dtype=np.float64)
    inv_freq = 1.0 / (THETA ** (half / HD))
    t_idx = np.arange(T, dtype=np.float64)
    freqs = np.outer(t_idx, inv_freq)
    emb = np.concatenate([freqs, freqs], axis=-1)
    cos, sin = np.cos(emb), np.sin(emb)
    scale_vec = (half + 0.4 * HD) / (1.4 * HD)
    power = (t_idx - T // 2) / SCALE_BASE
    scale = scale_vec[None, :] ** power[:, None]
    scale = np.concatenate([scale, scale], axis=-1)
    sgn = np.where(np.arange(HD) < HD // 2, -1.0, 1.0)
    aq = (scale * cos).T
    bq = sgn[:, None] * (scale * sin).T
    ak = (cos / scale).T
    bk = sgn[:, None] * (sin / scale).T

    # within-tile causal mask, additive: -1e9 where j > i (applied to the
    # scores via an identity-stationary matmul before exp)
    dj = np.arange(128)[:, None]
    r = np.arange(128)[None, :]
    tri = np.where(dj > r, -1e9, 0.0)
    ident = np.eye(128)
    return (
        aq.astype(BF16), bq.astype(BF16), ak.astype(BF16), bk.astype(BF16),
        tri.astype(BF16), ident.astype(BF16),
    )


def _arrange_wqk(wq, wk):
    # [E, F] -> per 128-wide f-block: [128(part=e%128), NE(e//128), 128(f)]
    w = np.concatenate([wq, wk], axis=1)          # [E, NF*128]
    nf = w.shape[1] // 128
    w = w.reshape(NE, 128, nf, 128)               # [n, p, f_blk, fc]
    w = w.transpose(2, 1, 0, 3)                   # [f_blk, p, n, fc]
    return np.ascontiguousarray(w).astype(BF16)


def _arrange_xt(xt):
    # [E, T] -> [half, p, e, chunk, i]
    w = xt.reshape(NE, 128, 2, 2, CH)             # [e, p, half, cc, i]
    w = w.transpose(2, 1, 0, 3, 4)                # [half, p, e, cc, i]
    return np.ascontiguousarray(w).astype(BF16)


def _arrange_wv(wv):
    # [E, KVL*HD] -> [p, e, f]
    w = wv.reshape(NE, 128, KVL * HD)
    w = w.transpose(1, 0, 2)
    return np.ascontiguousarray(w).astype(BF16)


def _arrange_wo(wo):
    # [HL*HD, E] -> [Ehalf, p, fb*2+e2, i] so (fb, ec) slice is
    # [eh=ec//2][:, fb*2 + ec%2, :]
    w = wo.reshape(HL, 128, 2, 2, CH)             # [fb, p, eh, e2, i]
    w = w.transpose(2, 1, 0, 3, 4)                # [eh, p, fb, e2, i]
    w = w.reshape(2, 128, NE, CH)
    return np.ascontiguousarray(w).astype(BF16)


def _make_in_maps(x, W_q, W_k, W_v, W_o):
    aq, bq, ak, bk, tri, ident = _host_tables()
    xts = [_arrange_xt(np.ascontiguousarray(x[b].T)) for b in range(B)]
    in_maps = []
    for core in range(8):
        b, g = core // G, core % G
        in_maps.append({
            "xt": xts[b],
            "wqk": _arrange_wqk(W_q[:, g * HL * HD:(g + 1) * HL * HD],
                                W_k[:, g * KVL * HD:(g + 1) * KVL * HD]),
            "wv": _arrange_wv(W_v[:, g * KVL * HD:(g + 1) * KVL * HD]),
            "wo": _arrange_wo(W_o[g * HL * HD:(g + 1) * HL * HD, :] / 4096.0),
            "aq": aq, "bq": bq, "ak": ak, "bk": bk,
            "tri": tri,
            "ident": ident,
        })
    return in_maps


def _run(x, W_q, W_k, W_v, W_o, trace=False):
    nc = _get_compiled()
    in_maps = _make_in_maps(x, W_q, W_k, W_v, W_o)
    res = run_bass_kernel_spmd(nc, in_maps, list(range(8)), trace=trace)
    out = np.empty((B, T, E), np.float32)
    for b in range(B):
        out[b] = (res.results[2 * b]["out_p"].astype(np.float32)
                  + res.results[2 * b + 1]["out_p"].astype(np.float32))
    return out, res.exec_time_ns


def kernel(x, W_q, W_k, W_v, W_o):
    out, _ = _run(
        np.asarray(x), np.asarray(W_q), np.asarray(W_k),
        np.asarray(W_v), np.asarray(W_o),
    )
    return out


# revision 33
# speedup vs baseline: 1.2154x; 1.0007x over previous
"""Trainium2 Bass kernel for nn_MultiHeadSelfAttention_11158325035343.

GQA multi-head self-attention (B=4, T=2048, E=2048, H=16, HKV=8, HD=128)
with XPos rotary embedding and causal softmax.

Sharding: 8 cores = 4 batches x 2 head-groups. Each core computes, for its
batch b and head-group g (8 q heads, 4 kv heads):
  QT/KT = W.T @ x.T   ([head_dim, T] per head, head_dim on partitions)
  V     = x @ W_v     ([T, head_dim] per kv head)
  XPos rope applied via two host-precomputed fused tables + half-swap
  scoresT[j, i] per (head, i-chunk, j-tile); diagonal j-tiles narrow
  score/exp/AV/den work to the causally-live columns and get the
  within-tile causal mask folded INTO the scores as an accumulated
  identity @ (-1e9 upper-triangle) matmul, so exp lands exact zeros and
  nothing sits between ACT and the AV matmuls; softmax denominator via
  ones-matmul over DVE-presummed probs (pairs -> quads -> octs);
  AV/den matmuls deferred three steps behind the scores; attnT
  normalized by the broadcast reciprocal denominator (fin chains staged
  across chunk boundaries); partial out = attnT.T @ W_o, written bf16.
Host sums the two group partials per batch in f32.

Scheduling structure (the PE queue executes strictly in emission order,
so all overlap must be laid out at emission time):
  - PE warmed up via memset-ones matmuls at t=0 (HAM clock-gate release
    at ~3.4us, bridging the ~7.7us framework startup + input DMA wave)
  - x/W_v/W_o staged via single large host-prearranged DMAs (sequencer
    issue cost is ~600ns per dma_start, so fewer+bigger wins)
  - phase 2 runs chunk-pair-major (all heads' token pair 0 first), with
    the cp0 sweep emitted interleaved into phase-1's second token half;
  - after the cp0 sweep the first half of phase 3 unlocks: its output
    tiles are emitted into the cp1 sweep (chunk boundaries + mid-chunk)
    and into the final flush/drain as PE filler for the ACT-exp-bound
    stretches and the last normalize chain.

Measured on trn2 (best of 3): 515.3us, rel err 6.1e-3 (gate 2e-2).
Session start baseline: 561.5us. NOTE: the part sometimes runs a 2.0GHz
PE clock profile instead of 2.4 (all engines ~20% slower, visible as
median N=512 matmul 259ns vs 216ns) - wall times then read ~620us for
the same kernel.
"""

import sys
import types

sys.path.insert(0, "/opt/trn_rl_repo")

import numpy as np
import ml_dtypes

BF16 = ml_dtypes.bfloat16

# ---------------------------------------------------------------------------
# NTFF profile hook injection (missing antenv.axon_hooks in this image).
# Needed only when trace=True; harmless otherwise.
# ---------------------------------------------------------------------------
def _ensure_axon_hooks():
    if "antenv.axon_hooks" in sys.modules:
        return
    try:
        import antenv
        mod = types.ModuleType("antenv.axon_hooks")
        holder = {"hook": None}
        mod.set_axon_ntff_profile_hook = lambda h: holder.__setitem__("hook", h)
        mod.get_axon_ntff_profile_hook = lambda: holder["hook"]
        sys.modules["antenv.axon_hooks"] = mod
        antenv.axon_hooks = mod
        from trn_agent_boot.trn_boot import _ntff_profile_via_ctypes
        mod.set_axon_ntff_profile_hook(
            _ntff_profile_via_ctypes("/opt/axon/libaxon_pjrt.so")
        )
    except Exception:
        pass


_ensure_axon_hooks()

import concourse.bass as bass
import concourse.bacc as bacc
import concourse.mybir as mybir
import concourse.tile as tile
from concourse.bass_utils import run_bass_kernel_spmd

# Problem constants (hardcoded per spec).
B, T, E = 4, 2048, 2048
H, HKV, HD = 16, 8, 128
THETA, SCALE_BASE = 10000.0, 512.0
G = 2                   # head groups (cores per batch)
HL = H // G             # 8 local q heads
KVL = HKV // G          # 4 local kv heads
REP = H // HKV          # GQA repeat
CH = 512                # i-chunk / matmul free dim
NE = E // 128           # 16 contraction tiles
NF = HL + KVL           # 12 projection f-tiles (8 Q + 4 K)
HALFT = T // 2          # token half for phase-1 SBUF staging
NJT = T // 128          # 16 j tiles
NCH = T // CH           # 4 i chunks
INV_SQRT_D = 1.0 / float(np.sqrt(np.float32(HD)))

F32 = mybir.dt.float32
F16 = mybir.dt.float16
BF = mybir.dt.bfloat16

_COMPILED = None


def _build_nc():
    nc = bacc.Bacc("TRN2", target_bir_lowering=False, debug=False, num_devices=8)

    # xt prearranged on host: [half, p, e, chunk, i]
    xt_d = nc.dram_tensor("xt", [2, 128, NE, 2, CH], BF, kind="ExternalInput")
    wqk_d = nc.dram_tensor("wqk", [NF, 128, NE, 128], BF, kind="ExternalInput")
    # wv prearranged: [p, e, f]
    wv_d = nc.dram_tensor("wv", [128, NE, KVL * HD], BF, kind="ExternalInput")
    # wo prearranged: [Ehalf, p, fb*2+e2, i]
    wo_d = nc.dram_tensor("wo", [2, 128, NE, CH], BF, kind="ExternalInput")
    aq_d = nc.dram_tensor("aq", [HD, T], BF, kind="ExternalInput")
    bq_d = nc.dram_tensor("bq", [HD, T], BF, kind="ExternalInput")
    ak_d = nc.dram_tensor("ak", [HD, T], BF, kind="ExternalInput")
    bk_d = nc.dram_tensor("bk", [HD, T], BF, kind="ExternalInput")
    tri_d = nc.dram_tensor("tri", [128, 128], BF, kind="ExternalInput")
    ident_d = nc.dram_tensor("ident", [128, 128], BF, kind="ExternalInput")
    out_d = nc.dram_tensor("out_p", [T, E], BF, kind="ExternalOutput")

    with tile.TileContext(nc) as tc:
        with (
            tc.tile_pool(name="big", bufs=2) as pool_big,       # xt chunks / wo
            tc.tile_pool(name="qk", bufs=NF) as pool_qk,        # rope'd QT/KT bf16
            tc.tile_pool(name="v", bufs=NJT) as pool_v,         # V bf16
            tc.tile_pool(name="at", bufs=HL) as pool_at,        # attnT bf16
            tc.tile_pool(name="tab", bufs=4) as pool_tab,       # rope tables
            tc.tile_pool(name="wv", bufs=1) as pool_wv,         # resident W_v
            tc.tile_pool(name="w", bufs=3) as pool_w,           # streamed W_q/W_k
            tc.tile_pool(name="tmp", bufs=2) as pool_tmp,       # rope temp
            tc.tile_pool(name="p", bufs=4) as pool_p,           # exp probs bf16
            tc.tile_pool(name="sp", bufs=2) as pool_sp,         # den pair presums
            tc.tile_pool(name="o", bufs=2) as pool_o,           # out staging
            tc.tile_pool(name="sm", bufs=1) as pool_sm,         # small constants
            tc.tile_pool(name="dv", bufs=4) as pool_dv,         # recip denominators
            tc.tile_pool(name="dvr", bufs=4, space="DRAM") as pool_dvr,  # dinv DRAM bounce
            tc.tile_pool(name="bch", bufs=3) as pool_bch,       # dinv bcast per chunk
            tc.tile_pool(name="ps", bufs=2, space=bass.MemorySpace.PSUM) as pool_ps,
        ):
            # ---- PE warmup: all-ones tile via memset (no DMA dependency)
            # so the HAM clock gate releases at ~3.4us, before the input
            # DMA wave completes. The same tile serves the den ones-matmul.
            ones_t = pool_sm.tile([128, CH], BF, tag="oc", name="ones_t")
            nc.vector.memset(ones_t[:], 1.0)
            warm_ps = pool_ps.tile([128, CH], F32, tag="psden", bufs=1,
                                   name="warm_ps")
            for _wi in range(38):
                nc.tensor.matmul(warm_ps[:], ones_t[:, 0:128], ones_t[:],
                                 start=True, stop=True)

            # tiny mask constants on the gpsimd queue (needed in phase 2)
            tri_t = pool_sm.tile([128, 128], BF, tag="tri", name="tri_t")
            nc.gpsimd.dma_start(tri_t[:], tri_d[:])
            ident_t = pool_sm.tile([128, 128], BF, tag="id", name="ident_t")
            nc.gpsimd.dma_start(ident_t[:], ident_d[:])

            # ---- persistent activation tensors ----
            qk_t = [pool_qk.tile([128, T], BF, tag="qk", name=f"qk{i}") for i in range(NF)]
            v_t = [pool_v.tile([128, KVL * HD], BF, tag="v", name=f"v{i}") for i in range(NJT)]
            at_t = [pool_at.tile([128, T], BF, tag="at", name=f"at{i}") for i in range(HL)]

            # ================= Phase 2: attention ==========================
            # Chunk-pair-major over heads: all heads' token pair 0 (chunks
            # 0-1) first, then pair 1 (chunks 2-3). Tokens 0-1023 of every
            # head finish after the first sweep, unblocking the first half
            # of phase 3 as PE filler while the rest of attention (which is
            # ACT-exp-bound per chunk) runs. AV/den matmuls of each j-block
            # are deferred TWO steps so the PE always has independent work
            # while ACT runs exp. Per-chunk normalization chains are staged
            # one boundary later per stage.
            pend_q = []     # deferred AV/den emitters, one list per j-block
            fin_chains = []  # normalization chains, one stage/boundary
            den2_map = {}

            def make_fin_a(hl, c, acc, den, den2):
                # per-chunk psum evacuation: acc -> at_t, den row -> its half
                # of the pair's den2 buffer (x 1/4096 for the fp16 recip)
                def stage_a():
                    nc.vector.tensor_copy(
                        at_t[hl][:, c * CH:(c + 1) * CH], acc[:]
                    )
                    nc.vector.tensor_scalar_mul(
                        den2[0:1, (c % 2) * CH:(c % 2 + 1) * CH],
                        den[0:1, :], 1.0 / 4096.0,
                    )
                return [stage_a]

            def make_fin_bc(hl, cp, den2):
                # per chunk-PAIR: reciprocal + broadcast + normalize over a
                # [128, 2*CH] region; half the DMA-descriptor bursts of the
                # per-chunk variant
                state = {}

                def stage_b():
                    d32 = pool_dv.tile([32, 2 * CH // 32], F32, tag="d32",
                                       bufs=1, name=f"d32_{hl}_{cp}")
                    nc.sync.dma_start(d32[:], den2[:])
                    dr = pool_dv.tile([32, 2 * CH // 32], F16, tag="dr",
                                      bufs=1, name=f"dr{hl}_{cp}")
                    with nc.allow_low_precision(reason="fp16 dinv; x4096 scaling keeps it normal"):
                        nc.vector.reciprocal(dr[:], d32[:])
                    dd_t = pool_dvr.tile([1, 2 * CH], F16, tag="dvrow",
                                         name=f"dinv_dram{hl}_{cp}")
                    nc.sync.dma_start(dd_t[:], dr[:])
                    bch = pool_bch.tile([128, 2 * CH], F16, tag="bch", bufs=2,
                                        name=f"bch{hl}_{cp}")
                    nc.sync.dma_start(bch[:], dd_t[:].to_broadcast((128, 2 * CH)))
                    state["bch"] = bch

                def stage_c():
                    for hh in range(2):
                        lo = cp * 2 * CH + hh * CH
                        nc.gpsimd.tensor_mul(
                            at_t[hl][:, lo:lo + CH],
                            at_t[hl][:, lo:lo + CH],
                            state["bch"][:, hh * CH:(hh + 1) * CH],
                        )

                return [stage_b, stage_c]

            def fin_boundary():
                for chain in fin_chains:
                    chain.pop(0)()
                fin_chains[:] = [ch for ch in fin_chains if ch]

            # Phase-3 output-projection emitters. The first token half
            # (it < 8) only needs the cp0 attention sweep, so those groups
            # are emitted INTO the cp1 sweep's PE stream (the PE queue runs
            # in emission order - work emitted later cannot fill earlier
            # stalls). Interleaved groups use their own 1-bank psum tag so
            # they never WAR against the still-accumulating attention psum.
            os_map = {}

            def p3_group(it, eh, e2, tag, bufs):
                def emit():
                    key = (it, eh)
                    if key not in os_map:
                        os_map[key] = pool_o.tile(
                            [128, E // 2], BF, tag="o", bufs=2,
                            name=f"os{it}_{eh}")
                    os_t = os_map[key]
                    po = pool_ps.tile([128, CH], F32, tag=tag, bufs=bufs,
                                      name=f"po{it}_{eh}_{e2}")
                    for fb in range(HL):
                        nc.tensor.matmul(
                            po[:],
                            at_t[fb][:, it * 128:(it + 1) * 128],
                            wo_t[eh][:, fb * 2 + e2, :],
                            start=(fb == 0),
                            stop=(fb == HL - 1),
                        )
                    nc.vector.tensor_copy(
                        os_t[:, e2 * CH:(e2 + 1) * CH], po[:]
                    )
                    if e2 == 1:
                        nc.sync.dma_start(
                            out_d[it * 128:(it + 1) * 128,
                                  eh * HALFT:(eh + 1) * HALFT],
                            os_t[:],
                        )
                return emit

            p3_queue = [(it, eh, e2)
                        for it in range(T // 256)
                        for eh in range(2)
                        for e2 in range(2)]
            p3_budget = [24]   # interleaved groups; 8 reserved for the tail

            def p3_slot():
                if p3_budget[0] > 0 and p3_queue:
                    p3_budget[0] -= 1
                    p3_group(*p3_queue.pop(0), tag="pso", bufs=1)()

            chunk_order = [(hl, cp * 2 + ci)
                           for cp in range(NCH // 2)
                           for hl in range(HL)
                           for ci in range(2)]
            chunk_pos = [0]

            def emit_chunk():
                ci_idx = chunk_pos[0]
                if ci_idx >= len(chunk_order):
                    return
                chunk_pos[0] += 1
                hl, c = chunk_order[ci_idx]
                kf = HL + hl // REP
                kvc = (hl // REP) * HD
                njt = (c + 1) * (CH // 128)
                if c % 2 == 0:
                    den2 = pool_dv.tile([1, 2 * CH], F32, tag="den2",
                                        bufs=2, name=f"den2_{hl}_{c // 2}")
                    den2_map[hl] = den2
                else:
                    den2 = den2_map[hl]
                if c < 2:
                    acc = pool_ps.tile([128, CH], F32, tag="pso", bufs=1,
                                       name=f"acc{hl}_{c}")
                else:
                    acc = pool_ps.tile([128, CH], F32, tag="psacc", bufs=2,
                                       name=f"acc{hl}_{c}")
                den = pool_ps.tile([128, CH], F32, tag="psden", bufs=1,
                                   name=f"den{hl}_{c}")
                j0_order = list(range(0, njt, 2))
                start_jt = 0
                stop_jt = njt - 1
                for step, j0 in enumerate(j0_order):
                    # causally-live column start per j-tile: diagonal
                    # tiles (d >= 0) only need cols [128*d, 512)
                    i0s = []
                    for u in range(2):
                        d = (j0 + u) - (njt - 4)
                        i0s.append(128 * d if d > 0 else 0)
                    diag = (j0 >= njt - 4)
                    s2 = pool_ps.tile([128, 2, CH], F32, tag="ps", bufs=2,
                                      name=f"s2_{hl}_{c}_{j0}")
                    for u in range(2):
                        jt = j0 + u
                        masked = (jt >= njt - 4)
                        nc.tensor.matmul(
                            s2[:, u, i0s[u]:],
                            qk_t[kf][:, jt * 128:(jt + 1) * 128],
                            qk_t[hl][:, c * CH + i0s[u]:(c + 1) * CH],
                            start=True,
                            stop=not masked,
                        )
                        if masked:
                            nc.tensor.matmul(
                                s2[:, u, i0s[u]:i0s[u] + 128],
                                ident_t[:],
                                tri_t[:],
                                start=False,
                                stop=True,
                            )
                    p2 = pool_p.tile([128, 2, CH], BF, tag="p", bufs=4,
                                     name=f"p2_{hl}_{c}_{j0}")
                    # one activation per step; for diagonal pairs the
                    # region [i0s[0], CH) covers both u-slices (u=1's
                    # cols [i0s[0], i0s[1]) hold unread garbage)
                    nc.scalar.activation(
                        p2[:, :, i0s[0]:], s2[:, :, i0s[0]:],
                        mybir.ActivationFunctionType.Exp,
                        scale=INV_SQRT_D,
                    )
                    psum2 = None
                    quad = None
                    den_mm = None   # (tile, first) queued for this step
                    if diag:
                        pass    # causal mask already folded into the scores
                    else:
                        # pre-sum the probs pair on DVE, then merge step
                        # pairs into quads so the den ones-matmul streams a
                        # quarter of the rows
                        psum2 = pool_sp.tile([128, CH], BF, tag="sp",
                                             name=f"sp{hl}_{c}_{j0}")
                        nc.vector.tensor_add(
                            psum2[:], p2[:, 0, :], p2[:, 1, :]
                        )
                        if step % 2 == 0:
                            prev_psum2 = psum2
                        else:
                            quad = pool_sp.tile([128, CH], BF, tag="qd",
                                                bufs=2,
                                                name=f"qd{hl}_{c}_{j0}")
                            nc.vector.tensor_add(
                                quad[:], prev_psum2[:], psum2[:]
                            )
                            if c == 1:
                                den_mm = (quad, True)
                            elif step == 1:
                                prev_quad = quad
                            elif step == 3:
                                oct8 = pool_sp.tile(
                                    [128, CH], BF, tag="oc8", bufs=2,
                                    name=f"oc8_{hl}_{c}")
                                nc.vector.tensor_add(
                                    oct8[:], prev_quad[:], quad[:]
                                )
                                den_mm = (oct8, True)
                            else:
                                den_mm = (quad, False)
                    if len(pend_q) >= 3:
                        for fn in pend_q.pop(0):
                            fn()
                    if step == 2:
                        fin_boundary()
                    # feed first-half output-projection groups into the cp1
                    # sweep (at_t tokens 0-1023 are final for all heads two
                    # boundaries into the sweep)
                    if ci_idx >= 18 and step == 3:
                        p3_slot()
                    step_fns = []
                    for u in range(2):
                        jt = j0 + u
                        def av(jt=jt, p2=p2, u=u, acc=acc, kvc=kvc,
                               i0=i0s[u], sjt=start_jt, pjt=stop_jt):
                            nc.tensor.matmul(
                                acc[:, i0:],
                                v_t[jt][:, kvc:kvc + HD],
                                p2[:, u, i0:],
                                start=(jt == sjt),
                                stop=(jt == pjt),
                            )
                        step_fns.append(av)
                    if not diag:
                        if den_mm is not None:
                            def den_quad(src_t=den_mm[0], den=den,
                                         first=den_mm[1]):
                                nc.tensor.matmul(
                                    den[:],
                                    ones_t[:, 0:128],
                                    src_t[:],
                                    start=first,
                                    stop=False,
                                )
                            step_fns.append(den_quad)
                    else:
                        for u in range(2):
                            jt = j0 + u
                            def den_u(jt=jt, p2=p2, u=u, den=den,
                                      i0=i0s[u],
                                      sjt=start_jt, pjt=stop_jt):
                                nc.tensor.matmul(
                                    den[:, i0:],
                                    ones_t[:, 0:128],
                                    p2[:, u, i0:],
                                    start=(jt == sjt),
                                    stop=(jt == pjt),
                                )
                            step_fns.append(den_u)
                    pend_q.append(step_fns)
                fin_chains.append(make_fin_a(hl, c, acc, den, den2))
                if c % 2 == 1:
                    fin_chains.append(make_fin_bc(hl, c // 2, den2))
                if ci_idx >= 17:
                    p3_slot()
            # ================= Phase 1: QKV projections + rope =============
            # DMA priority on the sync queue: w0, then the chunk-0 x wave
            # (exactly what the first f-tile's first matmuls need), then
            # chunk 1, then rope tables (K first - K heads rope first),
            # then W_v. x/W_v go as single large prearranged transfers to
            # keep the sequencer issue count low.
            for half in range(2):
                hs = half * HALFT
                wq_pre = []

                def w_prefetch(f, half=half):
                    w = pool_w.tile([128, NE, 128], BF, tag="w",
                                    name=f"w_pre{half}_{f}")
                    nc.sync.dma_start(w[:], wqk_d[f])
                    wq_pre.append(w)

                forder = list(range(HL, NF)) + list(range(HL))
                w_prefetch(forder[0])
                xt_t = []
                for cc in range(2):
                    xx = pool_big.tile([128, NE, CH], BF, tag="big",
                                       name=f"xt{half}_{cc}")
                    nc.sync.dma_start(xx[:], xt_d[half, :, :, cc, :])
                    xt_t.append(xx)
                w_prefetch(forder[1])
                w_prefetch(forder[2])
                if half == 0:
                    ak_t = pool_tab.tile([HD, T], BF, tag="tab", name="ak_t")
                    nc.sync.dma_start(ak_t[:], ak_d[:])
                    bk_t = pool_tab.tile([HD, T], BF, tag="tab", name="bk_t")
                    nc.sync.dma_start(bk_t[:], bk_d[:])
                    aq_t = pool_tab.tile([HD, T], BF, tag="tab", name="aq_t")
                    nc.sync.dma_start(aq_t[:], aq_d[:])
                    bq_t = pool_tab.tile([HD, T], BF, tag="tab", name="bq_t")
                    nc.sync.dma_start(bq_t[:], bq_d[:])
                    wv_t = pool_wv.tile([128, NE, KVL * HD], BF, tag="wv",
                                        name="wv_t")
                    nc.sync.dma_start(wv_t[:], wv_d[:])

                for fi, f in enumerate(forder):
                    # host-prearranged W column block, contiguous per partition
                    w_t = wq_pre.pop(0)
                    if fi + 3 < NF:
                        w_prefetch(forder[fi + 3])
                    for c in range(HALFT // CH):
                        ps = pool_ps.tile([128, CH], F32, tag="psacc", bufs=2)
                        for e in range(NE):
                            nc.tensor.matmul(
                                ps[:],
                                w_t[:, e, :],
                                xt_t[c][:, e, :],
                                start=(e == 0),
                                stop=(e == NE - 1),
                            )
                        nc.vector.tensor_copy(
                            qk_t[f][:, hs + c * CH: hs + (c + 1) * CH], ps[:]
                        )
                    # rope over this token half
                    A_t, B_t = (aq_t, bq_t) if f < HL else (ak_t, bk_t)
                    q = qk_t[f]
                    sl = slice(hs, hs + HALFT)
                    qs = pool_tmp.tile([128, HALFT], BF, tag="qs")
                    nc.sync.dma_start(qs[0:64, :], q[64:128, sl])
                    nc.sync.dma_start(qs[64:128, :], q[0:64, sl])
                    nc.vector.tensor_mul(qs[:, :], qs[:, :], B_t[:, sl])
                    nc.vector.tensor_mul(q[:, sl], q[:, sl], A_t[:, sl])
                    nc.vector.tensor_add(q[:, sl], q[:, sl], qs[:])
                    if half == 1 and chunk_pos[0] < 16:
                        emit_chunk()

                for tt in range(NJT // 2):
                    tglob = half * (NJT // 2) + tt
                    cc, co = tt // 4, (tt % 4) * 128
                    psv = pool_ps.tile([128, KVL * HD], F32, tag="psacc", bufs=2)
                    for e in range(NE):
                        nc.tensor.matmul(
                            psv[:],
                            xt_t[cc][:, e, co:co + 128],
                            wv_t[:, e, :],
                            start=(e == 0),
                            stop=(e == NE - 1),
                        )
                    nc.vector.tensor_copy(v_t[tglob][:], psv[:])
                    if half == 1 and chunk_pos[0] < 16:
                        emit_chunk()

            # W_o loads reuse the xt big-tile ring (freed after phase 1):
            # two [128, NE, CH] tiles, mapping [p, fb*2+e2, i] so phase 3's
            # (fb, ec) slice is wo_t[ec//2][:, fb*2 + ec%2, :]
            wo_t = []
            for eh in range(2):
                w = pool_big.tile([128, NE, CH], BF, tag="big",
                                  name=f"wo{eh}")
                nc.sync.dma_start(w[:], wo_d[eh])
                wo_t.append(w)

            while chunk_pos[0] < len(chunk_order):
                emit_chunk()
            while pend_q:
                for fn in pend_q.pop(0):
                    fn()
            # ready first-half output tiles keep the PE fed while the
            # final AV/den/normalize chains drain
            for _ in range(3):
                if p3_queue:
                    p3_group(*p3_queue.pop(0), tag="pso", bufs=1)()
            while fin_chains:
                fin_boundary()
                if p3_queue:
                    p3_group(*p3_queue.pop(0), tag="psacc", bufs=2)()

            # ================= Phase 3 tail: remaining output tiles ========
            while p3_queue:
                p3_group(*p3_queue.pop(0), tag="psacc", bufs=2)()
            for it in range(T // 256, T // 128):
                for eh in range(2):
                    for e2 in range(2):
                        p3_group(it, eh, e2, tag="psacc", bufs=2)()

    nc.compile()
    return nc


def _get_compiled():
    global _COMPILED
    if _COMPILED is None:
        _COMPILED = _build_nc()
    return _COMPILED


def _host_tables():
    half = np.arange(0, HD, 2, dtype=np.float64)
    inv_freq = 1.0 / (THETA ** (half / HD))
    t_idx = np.arange(T, dtype=np.float64)
    freqs = np.outer(t_idx, inv_freq)
    emb = np.concatenate([freqs, freqs], axis=-1)
    cos, sin = np.cos(emb), np.sin(emb)
    scale_vec = (half + 0.4 * HD) / (1.4 * HD)
    power = (t_idx - T // 2) / SCALE_BASE
    scale = scale_vec[None, :] ** power[:, None]
    scale = np.concatenate([scale, scale], axis=-1)
    sgn = np.where(np.arange(HD) < HD // 2, -1.0, 1.0)
    aq = (scale * cos).T
    bq = sgn[:, None] * (scale * sin).T
    ak = (cos / scale).T
    bk = sgn[:, None] * (sin / scale).T

    # within-tile causal mask, additive: -1e9 where j > i (applied to the
    # scores via an identity-stationary matmul before exp)
    dj = np.arange(128)[:, None]
    r = np.arange(128)[None, :]
    tri = np.where(dj > r, -1e9, 0.0)
    ident = np.eye(128)
    return (
        aq.astype(BF16), bq.astype(BF16), ak.astype(BF16), bk.astype(BF16),
        tri.astype(BF16), ident.astype(BF16),
    )


def _arrange_wqk(wq, wk):
    # [E, F] -> per 128-wide f-block: [128(part=e%128), NE(e//128), 128(f)]
    w = np.concatenate([wq, wk], axis=1)          # [E, NF*128]
    nf = w.shape[1] // 128
    w = w.reshape(NE, 128, nf, 128)               # [n, p, f_blk, fc]
    w = w.transpose(2, 1, 0, 3)                   # [f_blk, p, n, fc]
    return np.ascontiguousarray(w).astype(BF16)


def _arrange_xt(xt):
    # [E, T] -> [half, p, e, chunk, i]
    w = xt.reshape(NE, 128, 2, 2, CH)             # [e, p, half, cc, i]
    w = w.transpose(2, 1, 0, 3, 4)                # [half, p, e, cc, i]
    return np.ascontiguousarray(w).astype(BF16)


def _arrange_wv(wv):
    # [E, KVL*HD] -> [p, e, f]
    w = wv.reshape(NE, 128, KVL * HD)
    w = w.transpose(1, 0, 2)
    return np.ascontiguousarray(w).astype(BF16)


def _arrange_wo(wo):
    # [HL*HD, E] -> [Ehalf, p, fb*2+e2, i] so (fb, ec) slice is
    # [eh=ec//2][:, fb*2 + ec%2, :]
    w = wo.reshape(HL, 128, 2, 2, CH)             # [fb, p, eh, e2, i]
    w = w.transpose(2, 1, 0, 3, 4)                # [eh, p, fb, e2, i]
    w = w.reshape(2, 128, NE, CH)
    return np.ascontiguousarray(w).astype(BF16)


def _make_in_maps(x, W_q, W_k, W_v, W_o):
    aq, bq, ak, bk, tri, ident = _host_tables()
    xts = [_arrange_xt(np.ascontiguousarray(x[b].T)) for b in range(B)]
    in_maps = []
    for core in range(8):
        b, g = core // G, core % G
        in_maps.append({
            "xt": xts[b],
            "wqk": _arrange_wqk(W_q[:, g * HL * HD:(g + 1) * HL * HD],
                                W_k[:, g * KVL * HD:(g + 1) * KVL * HD]),
            "wv": _arrange_wv(W_v[:, g * KVL * HD:(g + 1) * KVL * HD]),
            "wo": _arrange_wo(W_o[g * HL * HD:(g + 1) * HL * HD, :] / 4096.0),
            "aq": aq, "bq": bq, "ak": ak, "bk": bk,
            "tri": tri,
            "ident": ident,
        })
    return in_maps


def _run(x, W_q, W_k, W_v, W_o, trace=False):
    nc = _get_compiled()
    in_maps = _make_in_maps(x, W_q, W_k, W_v, W_o)
    res = run_bass_kernel_spmd(nc, in_maps, list(range(8)), trace=trace)
    out = np.empty((B, T, E), np.float32)
    for b in range(B):
        out[b] = (res.results[2 * b]["out_p"].astype(np.float32)
                  + res.results[2 * b + 1]["out_p"].astype(np.float32))
    return out, res.exec_time_ns


def kernel(x, W_q, W_k, W_v, W_o):
    out, _ = _run(
        np.asarray(x), np.asarray(W_q), np.asarray(W_k),
        np.asarray(W_v), np.asarray(W_o),
    )
    return out


# revision 34
# speedup vs baseline: 1.2203x; 1.0041x over previous
"""Trainium2 Bass kernel for nn_MultiHeadSelfAttention_11158325035343.

GQA multi-head self-attention (B=4, T=2048, E=2048, H=16, HKV=8, HD=128)
with XPos rotary embedding and causal softmax.

Sharding: 8 cores = 4 batches x 2 head-groups. Each core computes, for its
batch b and head-group g (8 q heads, 4 kv heads):
  QT/KT = W.T @ x.T   ([head_dim, T] per head, head_dim on partitions)
  V     = x @ W_v     ([T, head_dim] per kv head)
  XPos rope applied via two host-precomputed fused tables + half-swap
  scoresT[j, i] per (head, i-chunk, j-tile); diagonal j-tiles narrow
  score/exp/AV/den work to the causally-live columns and get the
  within-tile causal mask folded INTO the scores as an accumulated
  identity @ (-1e9 upper-triangle) matmul, so exp lands exact zeros and
  nothing sits between ACT and the AV matmuls; softmax denominator via
  ones-matmul over DVE-presummed probs (pairs -> quads -> octs);
  AV/den matmuls deferred three steps behind the scores; attnT
  normalized by the broadcast reciprocal denominator (fin chains staged
  across chunk boundaries); partial out = attnT.T @ W_o, written bf16.
Host sums the two group partials per batch in f32.

Scheduling structure (the PE queue executes strictly in emission order,
so all overlap must be laid out at emission time):
  - PE warmed up via memset-ones matmuls at t=0 (HAM clock-gate release
    at ~3.4us, bridging the ~7.7us framework startup + input DMA wave)
  - x/W_v/W_o staged via single large host-prearranged DMAs (sequencer
    issue cost is ~600ns per dma_start, so fewer+bigger wins)
  - phase 2 runs chunk-pair-major (all heads' token pair 0 first), with
    the cp0 sweep emitted interleaved into phase-1's second token half;
  - after the cp0 sweep the first half of phase 3 unlocks: its output
    tiles are emitted into the cp1 sweep (chunk boundaries + mid-chunk)
    and into the final flush/drain as PE filler for the ACT-exp-bound
    stretches and the last normalize chain.

Measured on trn2 (best of 3): 515.3us, rel err 6.1e-3 (gate 2e-2).
Session start baseline: 561.5us. NOTE: the part sometimes runs a 2.0GHz
PE clock profile instead of 2.4 (all engines ~20% slower, visible as
median N=512 matmul 259ns vs 216ns) - wall times then read ~620us for
the same kernel.
"""

import sys
import types

sys.path.insert(0, "/opt/trn_rl_repo")

import numpy as np
import ml_dtypes

BF16 = ml_dtypes.bfloat16

# ---------------------------------------------------------------------------
# NTFF profile hook injection (missing antenv.axon_hooks in this image).
# Needed only when trace=True; harmless otherwise.
# ---------------------------------------------------------------------------
def _ensure_axon_hooks():
    if "antenv.axon_hooks" in sys.modules:
        return
    try:
        import antenv
        mod = types.ModuleType("antenv.axon_hooks")
        holder = {"hook": None}
        mod.set_axon_ntff_profile_hook = lambda h: holder.__setitem__("hook", h)
        mod.get_axon_ntff_profile_hook = lambda: holder["hook"]
        sys.modules["antenv.axon_hooks"] = mod
        antenv.axon_hooks = mod
        from trn_agent_boot.trn_boot import _ntff_profile_via_ctypes
        mod.set_axon_ntff_profile_hook(
            _ntff_profile_via_ctypes("/opt/axon/libaxon_pjrt.so")
        )
    except Exception:
        pass


_ensure_axon_hooks()

import concourse.bass as bass
import concourse.bacc as bacc
import concourse.mybir as mybir
import concourse.tile as tile
from concourse.bass_utils import run_bass_kernel_spmd

# Problem constants (hardcoded per spec).
B, T, E = 4, 2048, 2048
H, HKV, HD = 16, 8, 128
THETA, SCALE_BASE = 10000.0, 512.0
G = 2                   # head groups (cores per batch)
HL = H // G             # 8 local q heads
KVL = HKV // G          # 4 local kv heads
REP = H // HKV          # GQA repeat
CH = 512                # i-chunk / matmul free dim
NE = E // 128           # 16 contraction tiles
NF = HL + KVL           # 12 projection f-tiles (8 Q + 4 K)
HALFT = T // 2          # token half for phase-1 SBUF staging
NJT = T // 128          # 16 j tiles
NCH = T // CH           # 4 i chunks
INV_SQRT_D = 1.0 / float(np.sqrt(np.float32(HD)))

F32 = mybir.dt.float32
F16 = mybir.dt.float16
BF = mybir.dt.bfloat16

_COMPILED = None


def _build_nc():
    nc = bacc.Bacc("TRN2", target_bir_lowering=False, debug=False, num_devices=8)

    # xt prearranged on host: [half, p, e, chunk, i]
    xt_d = nc.dram_tensor("xt", [2, 128, NE, 2, CH], BF, kind="ExternalInput")
    wqk_d = nc.dram_tensor("wqk", [NF, 128, NE, 128], BF, kind="ExternalInput")
    # wv prearranged: [p, e, f]
    wv_d = nc.dram_tensor("wv", [128, NE, KVL * HD], BF, kind="ExternalInput")
    # wo prearranged: [Ehalf, p, fb*2+e2, i]
    wo_d = nc.dram_tensor("wo", [2, 128, NE, CH], BF, kind="ExternalInput")
    aq_d = nc.dram_tensor("aq", [HD, T], BF, kind="ExternalInput")
    bq_d = nc.dram_tensor("bq", [HD, T], BF, kind="ExternalInput")
    ak_d = nc.dram_tensor("ak", [HD, T], BF, kind="ExternalInput")
    bk_d = nc.dram_tensor("bk", [HD, T], BF, kind="ExternalInput")
    tri_d = nc.dram_tensor("tri", [128, 128], BF, kind="ExternalInput")
    ident_d = nc.dram_tensor("ident", [128, 128], BF, kind="ExternalInput")
    out_d = nc.dram_tensor("out_p", [T, E], BF, kind="ExternalOutput")

    with tile.TileContext(nc) as tc:
        with (
            tc.tile_pool(name="big", bufs=2) as pool_big,       # xt chunks / wo
            tc.tile_pool(name="qk", bufs=NF) as pool_qk,        # rope'd QT/KT bf16
            tc.tile_pool(name="v", bufs=NJT) as pool_v,         # V bf16
            tc.tile_pool(name="at", bufs=HL) as pool_at,        # attnT bf16
            tc.tile_pool(name="tab", bufs=4) as pool_tab,       # rope tables
            tc.tile_pool(name="wv", bufs=1) as pool_wv,         # resident W_v
            tc.tile_pool(name="w", bufs=3) as pool_w,           # streamed W_q/W_k
            tc.tile_pool(name="tmp", bufs=2) as pool_tmp,       # rope temp
            tc.tile_pool(name="p", bufs=4) as pool_p,           # exp probs bf16
            tc.tile_pool(name="sp", bufs=2) as pool_sp,         # den pair presums
            tc.tile_pool(name="o", bufs=2) as pool_o,           # out staging
            tc.tile_pool(name="sm", bufs=1) as pool_sm,         # small constants
            tc.tile_pool(name="dv", bufs=4) as pool_dv,         # recip denominators
            tc.tile_pool(name="dvr", bufs=4, space="DRAM") as pool_dvr,  # dinv DRAM bounce
            tc.tile_pool(name="bch", bufs=3) as pool_bch,       # dinv bcast per chunk
            tc.tile_pool(name="ps", bufs=2, space=bass.MemorySpace.PSUM) as pool_ps,
        ):
            # ---- PE warmup: all-ones tile via memset (no DMA dependency)
            # so the HAM clock gate releases at ~3.4us, before the input
            # DMA wave completes. The same tile serves the den ones-matmul.
            ones_t = pool_sm.tile([128, CH], BF, tag="oc", name="ones_t")
            nc.vector.memset(ones_t[:], 1.0)
            warm_ps = pool_ps.tile([128, CH], F32, tag="psden", bufs=1,
                                   name="warm_ps")
            for _wi in range(16):
                nc.tensor.matmul(warm_ps[:], ones_t[:, 0:128], ones_t[:],
                                 start=True, stop=True)

            # tiny mask constants on the gpsimd queue (needed in phase 2)
            tri_t = pool_sm.tile([128, 128], BF, tag="tri", name="tri_t")
            nc.gpsimd.dma_start(tri_t[:], tri_d[:])
            ident_t = pool_sm.tile([128, 128], BF, tag="id", name="ident_t")
            nc.gpsimd.dma_start(ident_t[:], ident_d[:])

            # ---- persistent activation tensors ----
            qk_t = [pool_qk.tile([128, T], BF, tag="qk", name=f"qk{i}") for i in range(NF)]
            v_t = [pool_v.tile([128, KVL * HD], BF, tag="v", name=f"v{i}") for i in range(NJT)]
            at_t = [pool_at.tile([128, T], BF, tag="at", name=f"at{i}") for i in range(HL)]

            # ================= Phase 2: attention ==========================
            # Chunk-pair-major over heads: all heads' token pair 0 (chunks
            # 0-1) first, then pair 1 (chunks 2-3). Tokens 0-1023 of every
            # head finish after the first sweep, unblocking the first half
            # of phase 3 as PE filler while the rest of attention (which is
            # ACT-exp-bound per chunk) runs. AV/den matmuls of each j-block
            # are deferred TWO steps so the PE always has independent work
            # while ACT runs exp. Per-chunk normalization chains are staged
            # one boundary later per stage.
            pend_q = []     # deferred AV/den emitters, one list per j-block
            fin_chains = []  # normalization chains, one stage/boundary
            den2_map = {}

            def make_fin_a(hl, c, acc, den, den2):
                # per-chunk psum evacuation: acc -> at_t, den row -> its half
                # of the pair's den2 buffer (x 1/4096 for the fp16 recip)
                def stage_a():
                    nc.vector.tensor_copy(
                        at_t[hl][:, c * CH:(c + 1) * CH], acc[:]
                    )
                    nc.vector.tensor_scalar_mul(
                        den2[0:1, (c % 2) * CH:(c % 2 + 1) * CH],
                        den[0:1, :], 1.0 / 4096.0,
                    )
                return [stage_a]

            def make_fin_bc(hl, cp, den2):
                # per chunk-PAIR: reciprocal + broadcast + normalize over a
                # [128, 2*CH] region; half the DMA-descriptor bursts of the
                # per-chunk variant
                state = {}

                def stage_b():
                    d32 = pool_dv.tile([32, 2 * CH // 32], F32, tag="d32",
                                       bufs=1, name=f"d32_{hl}_{cp}")
                    nc.sync.dma_start(d32[:], den2[:])
                    dr = pool_dv.tile([32, 2 * CH // 32], F16, tag="dr",
                                      bufs=1, name=f"dr{hl}_{cp}")
                    with nc.allow_low_precision(reason="fp16 dinv; x4096 scaling keeps it normal"):
                        nc.vector.reciprocal(dr[:], d32[:])
                    dd_t = pool_dvr.tile([1, 2 * CH], F16, tag="dvrow",
                                         name=f"dinv_dram{hl}_{cp}")
                    nc.sync.dma_start(dd_t[:], dr[:])
                    bch = pool_bch.tile([128, 2 * CH], F16, tag="bch", bufs=2,
                                        name=f"bch{hl}_{cp}")
                    nc.sync.dma_start(bch[:], dd_t[:].to_broadcast((128, 2 * CH)))
                    state["bch"] = bch

                def stage_c():
                    for hh in range(2):
                        lo = cp * 2 * CH + hh * CH
                        nc.gpsimd.tensor_mul(
                            at_t[hl][:, lo:lo + CH],
                            at_t[hl][:, lo:lo + CH],
                            state["bch"][:, hh * CH:(hh + 1) * CH],
                        )

                return [stage_b, stage_c]

            def fin_boundary():
                for chain in fin_chains:
                    chain.pop(0)()
                fin_chains[:] = [ch for ch in fin_chains if ch]

            # Phase-3 output-projection emitters. The first token half
            # (it < 8) only needs the cp0 attention sweep, so those groups
            # are emitted INTO the cp1 sweep's PE stream (the PE queue runs
            # in emission order - work emitted later cannot fill earlier
            # stalls). Interleaved groups use their own 1-bank psum tag so
            # they never WAR against the still-accumulating attention psum.
            os_map = {}

            def p3_group(it, eh, e2, tag, bufs):
                def emit():
                    key = (it, eh)
                    if key not in os_map:
                        os_map[key] = pool_o.tile(
                            [128, E // 2], BF, tag="o", bufs=2,
                            name=f"os{it}_{eh}")
                    os_t = os_map[key]
                    po = pool_ps.tile([128, CH], F32, tag=tag, bufs=bufs,
                                      name=f"po{it}_{eh}_{e2}")
                    for fb in range(HL):
                        nc.tensor.matmul(
                            po[:],
                            at_t[fb][:, it * 128:(it + 1) * 128],
                            wo_t[eh][:, fb * 2 + e2, :],
                            start=(fb == 0),
                            stop=(fb == HL - 1),
                        )
                    nc.vector.tensor_copy(
                        os_t[:, e2 * CH:(e2 + 1) * CH], po[:]
                    )
                    if e2 == 1:
                        nc.sync.dma_start(
                            out_d[it * 128:(it + 1) * 128,
                                  eh * HALFT:(eh + 1) * HALFT],
                            os_t[:],
                        )
                return emit

            p3_queue = [(it, eh, e2)
                        for it in range(T // 256)
                        for eh in range(2)
                        for e2 in range(2)]
            p3_budget = [24]   # interleaved groups; 8 reserved for the tail

            def p3_slot():
                if p3_budget[0] > 0 and p3_queue:
                    p3_budget[0] -= 1
                    p3_group(*p3_queue.pop(0), tag="pso", bufs=1)()

            chunk_order = [(hl, cp * 2 + ci)
                           for cp in range(NCH // 2)
                           for hl in range(HL)
                           for ci in range(2)]
            chunk_pos = [0]

            def emit_chunk():
                ci_idx = chunk_pos[0]
                if ci_idx >= len(chunk_order):
                    return
                chunk_pos[0] += 1
                hl, c = chunk_order[ci_idx]
                kf = HL + hl // REP
                kvc = (hl // REP) * HD
                njt = (c + 1) * (CH // 128)
                if c % 2 == 0:
                    den2 = pool_dv.tile([1, 2 * CH], F32, tag="den2",
                                        bufs=2, name=f"den2_{hl}_{c // 2}")
                    den2_map[hl] = den2
                else:
                    den2 = den2_map[hl]
                if c < 2:
                    acc = pool_ps.tile([128, CH], F32, tag="pso", bufs=1,
                                       name=f"acc{hl}_{c}")
                else:
                    acc = pool_ps.tile([128, CH], F32, tag="psacc", bufs=2,
                                       name=f"acc{hl}_{c}")
                den = pool_ps.tile([128, CH], F32, tag="psden", bufs=1,
                                   name=f"den{hl}_{c}")
                j0_order = list(range(0, njt, 2))
                start_jt = 0
                stop_jt = njt - 1
                for step, j0 in enumerate(j0_order):
                    # causally-live column start per j-tile: diagonal
                    # tiles (d >= 0) only need cols [128*d, 512)
                    i0s = []
                    for u in range(2):
                        d = (j0 + u) - (njt - 4)
                        i0s.append(128 * d if d > 0 else 0)
                    diag = (j0 >= njt - 4)
                    s2 = pool_ps.tile([128, 2, CH], F32, tag="ps", bufs=2,
                                      name=f"s2_{hl}_{c}_{j0}")
                    for u in range(2):
                        jt = j0 + u
                        masked = (jt >= njt - 4)
                        nc.tensor.matmul(
                            s2[:, u, i0s[u]:],
                            qk_t[kf][:, jt * 128:(jt + 1) * 128],
                            qk_t[hl][:, c * CH + i0s[u]:(c + 1) * CH],
                            start=True,
                            stop=not masked,
                        )
                        if masked:
                            nc.tensor.matmul(
                                s2[:, u, i0s[u]:i0s[u] + 128],
                                ident_t[:],
                                tri_t[:],
                                start=False,
                                stop=True,
                            )
                    p2 = pool_p.tile([128, 2, CH], BF, tag="p", bufs=4,
                                     name=f"p2_{hl}_{c}_{j0}")
                    # one activation per step; for diagonal pairs the
                    # region [i0s[0], CH) covers both u-slices (u=1's
                    # cols [i0s[0], i0s[1]) hold unread garbage)
                    nc.scalar.activation(
                        p2[:, :, i0s[0]:], s2[:, :, i0s[0]:],
                        mybir.ActivationFunctionType.Exp,
                        scale=INV_SQRT_D,
                    )
                    psum2 = None
                    quad = None
                    den_mm = None   # (tile, first) queued for this step
                    if diag:
                        pass    # causal mask already folded into the scores
                    else:
                        # pre-sum the probs pair on DVE, then merge step
                        # pairs into quads so the den ones-matmul streams a
                        # quarter of the rows
                        psum2 = pool_sp.tile([128, CH], BF, tag="sp",
                                             name=f"sp{hl}_{c}_{j0}")
                        nc.vector.tensor_add(
                            psum2[:], p2[:, 0, :], p2[:, 1, :]
                        )
                        if step % 2 == 0:
                            prev_psum2 = psum2
                        else:
                            quad = pool_sp.tile([128, CH], BF, tag="qd",
                                                bufs=2,
                                                name=f"qd{hl}_{c}_{j0}")
                            nc.vector.tensor_add(
                                quad[:], prev_psum2[:], psum2[:]
                            )
                            if c == 1:
                                den_mm = (quad, True)
                            elif step == 1:
                                prev_quad = quad
                            elif step == 3:
                                oct8 = pool_sp.tile(
                                    [128, CH], BF, tag="oc8", bufs=2,
                                    name=f"oc8_{hl}_{c}")
                                nc.vector.tensor_add(
                                    oct8[:], prev_quad[:], quad[:]
                                )
                                den_mm = (oct8, True)
                            else:
                                den_mm = (quad, False)
                    if len(pend_q) >= 3:
                        for fn in pend_q.pop(0):
                            fn()
                    if step == 2:
                        fin_boundary()
                    # feed first-half output-projection groups into the cp1
                    # sweep (at_t tokens 0-1023 are final for all heads two
                    # boundaries into the sweep)
                    if ci_idx >= 18 and step == 3:
                        p3_slot()
                    step_fns = []
                    for u in range(2):
                        jt = j0 + u
                        def av(jt=jt, p2=p2, u=u, acc=acc, kvc=kvc,
                               i0=i0s[u], sjt=start_jt, pjt=stop_jt):
                            nc.tensor.matmul(
                                acc[:, i0:],
                                v_t[jt][:, kvc:kvc + HD],
                                p2[:, u, i0:],
                                start=(jt == sjt),
                                stop=(jt == pjt),
                            )
                        step_fns.append(av)
                    if not diag:
                        if den_mm is not None:
                            def den_quad(src_t=den_mm[0], den=den,
                                         first=den_mm[1]):
                                nc.tensor.matmul(
                                    den[:],
                                    ones_t[:, 0:128],
                                    src_t[:],
                                    start=first,
                                    stop=False,
                                )
                            step_fns.append(den_quad)
                    else:
                        for u in range(2):
                            jt = j0 + u
                            def den_u(jt=jt, p2=p2, u=u, den=den,
                                      i0=i0s[u],
                                      sjt=start_jt, pjt=stop_jt):
                                nc.tensor.matmul(
                                    den[:, i0:],
                                    ones_t[:, 0:128],
                                    p2[:, u, i0:],
                                    start=(jt == sjt),
                                    stop=(jt == pjt),
                                )
                            step_fns.append(den_u)
                    pend_q.append(step_fns)
                fin_chains.append(make_fin_a(hl, c, acc, den, den2))
                if c % 2 == 1:
                    fin_chains.append(make_fin_bc(hl, c // 2, den2))
                if ci_idx >= 17:
                    p3_slot()
            # ================= Phase 1: QKV projections + rope =============
            # DMA priority on the sync queue: w0, then the chunk-0 x wave
            # (exactly what the first f-tile's first matmuls need), then
            # chunk 1, then rope tables (K first - K heads rope first),
            # then W_v. x/W_v go as single large prearranged transfers to
            # keep the sequencer issue count low.
            for half in range(2):
                hs = half * HALFT
                wq_pre = []

                def w_prefetch(f, half=half):
                    w = pool_w.tile([128, NE, 128], BF, tag="w",
                                    name=f"w_pre{half}_{f}")
                    nc.sync.dma_start(w[:], wqk_d[f])
                    wq_pre.append(w)

                forder = list(range(HL, NF)) + list(range(HL))
                w_prefetch(forder[0])
                xt_t = []
                for cc in range(2):
                    xx = pool_big.tile([128, NE, CH], BF, tag="big",
                                       name=f"xt{half}_{cc}")
                    # split per e-half so the first matmuls start ~3us
                    # earlier (they consume e-tiles in order)
                    for eh2 in range(2):
                        nc.sync.dma_start(
                            xx[:, eh2 * 8:(eh2 + 1) * 8, :],
                            xt_d[half, :, eh2 * 8:(eh2 + 1) * 8, cc, :],
                        )
                    xt_t.append(xx)
                w_prefetch(forder[1])
                w_prefetch(forder[2])
                if half == 0:
                    ak_t = pool_tab.tile([HD, T], BF, tag="tab", name="ak_t")
                    nc.sync.dma_start(ak_t[:], ak_d[:])
                    bk_t = pool_tab.tile([HD, T], BF, tag="tab", name="bk_t")
                    nc.sync.dma_start(bk_t[:], bk_d[:])
                    aq_t = pool_tab.tile([HD, T], BF, tag="tab", name="aq_t")
                    nc.sync.dma_start(aq_t[:], aq_d[:])
                    bq_t = pool_tab.tile([HD, T], BF, tag="tab", name="bq_t")
                    nc.sync.dma_start(bq_t[:], bq_d[:])
                    wv_t = pool_wv.tile([128, NE, KVL * HD], BF, tag="wv",
                                        name="wv_t")
                    nc.sync.dma_start(wv_t[:], wv_d[:])

                for fi, f in enumerate(forder):
                    # host-prearranged W column block, contiguous per partition
                    w_t = wq_pre.pop(0)
                    if fi + 3 < NF:
                        w_prefetch(forder[fi + 3])
                    for c in range(HALFT // CH):
                        ps = pool_ps.tile([128, CH], F32, tag="psacc", bufs=2)
                        for e in range(NE):
                            nc.tensor.matmul(
                                ps[:],
                                w_t[:, e, :],
                                xt_t[c][:, e, :],
                                start=(e == 0),
                                stop=(e == NE - 1),
                            )
                        nc.vector.tensor_copy(
                            qk_t[f][:, hs + c * CH: hs + (c + 1) * CH], ps[:]
                        )
                    # rope over this token half
                    A_t, B_t = (aq_t, bq_t) if f < HL else (ak_t, bk_t)
                    q = qk_t[f]
                    sl = slice(hs, hs + HALFT)
                    qs = pool_tmp.tile([128, HALFT], BF, tag="qs")
                    nc.sync.dma_start(qs[0:64, :], q[64:128, sl])
                    nc.sync.dma_start(qs[64:128, :], q[0:64, sl])
                    nc.vector.tensor_mul(qs[:, :], qs[:, :], B_t[:, sl])
                    nc.vector.tensor_mul(q[:, sl], q[:, sl], A_t[:, sl])
                    nc.vector.tensor_add(q[:, sl], q[:, sl], qs[:])
                    if half == 1 and chunk_pos[0] < 16:
                        emit_chunk()

                for tt in range(NJT // 2):
                    tglob = half * (NJT // 2) + tt
                    cc, co = tt // 4, (tt % 4) * 128
                    psv = pool_ps.tile([128, KVL * HD], F32, tag="psacc", bufs=2)
                    for e in range(NE):
                        nc.tensor.matmul(
                            psv[:],
                            xt_t[cc][:, e, co:co + 128],
                            wv_t[:, e, :],
                            start=(e == 0),
                            stop=(e == NE - 1),
                        )
                    nc.vector.tensor_copy(v_t[tglob][:], psv[:])
                    if half == 1 and chunk_pos[0] < 16:
                        emit_chunk()

            # W_o loads reuse the xt big-tile ring (freed after phase 1):
            # two [128, NE, CH] tiles, mapping [p, fb*2+e2, i] so phase 3's
            # (fb, ec) slice is wo_t[ec//2][:, fb*2 + ec%2, :]
            wo_t = []
            for eh in range(2):
                w = pool_big.tile([128, NE, CH], BF, tag="big",
                                  name=f"wo{eh}")
                nc.sync.dma_start(w[:], wo_d[eh])
                wo_t.append(w)

            while chunk_pos[0] < len(chunk_order):
                emit_chunk()
            while pend_q:
                for fn in pend_q.pop(0):
                    fn()
            # ready first-half output tiles keep the PE fed while the
            # final AV/den/normalize chains drain
            for _ in range(3):
                if p3_queue:
                    p3_group(*p3_queue.pop(0), tag="pso", bufs=1)()
            while fin_chains:
                fin_boundary()
                if p3_queue:
                    p3_group(*p3_queue.pop(0), tag="psacc", bufs=2)()

            # ================= Phase 3 tail: remaining output tiles ========
            while p3_queue:
                p3_group(*p3_queue.pop(0), tag="psacc", bufs=2)()
            for it in range(T // 256, T // 128):
                for eh in range(2):
                    for e2 in range(2):
                        p3_group(it, eh, e2, tag="psacc", bufs=2)()

    nc.compile()
    return nc


def _get_compiled():
    global _COMPILED
    if _COMPILED is None:
        _COMPILED = _build_nc()
    return _COMPILED


def _host_tables():
    half = np.arange(0, HD, 2, dtype=np.float64)
    inv_freq = 1.0 / (THETA ** (half / HD))
    t_idx = np.arange(T, dtype=np.float64)
    freqs = np.outer(t_idx, inv_freq)
    emb = np.concatenate([freqs, freqs], axis=-1)
    cos, sin = np.cos(emb), np.sin(emb)
    scale_vec = (half + 0.4 * HD) / (1.4 * HD)
    power = (t_idx - T // 2) / SCALE_BASE
    scale = scale_vec[None, :] ** power[:, None]
    scale = np.concatenate([scale, scale], axis=-1)
    sgn = np.where(np.arange(HD) < HD // 2, -1.0, 1.0)
    aq = (scale * cos).T
    bq = sgn[:, None] * (scale * sin).T
    ak = (cos / scale).T
    bk = sgn[:, None] * (sin / scale).T

    # within-tile causal mask, additive: -1e9 where j > i (applied to the
    # scores via an identity-stationary matmul before exp)
    dj = np.arange(128)[:, None]
    r = np.arange(128)[None, :]
    tri = np.where(dj > r, -1e9, 0.0)
    ident = np.eye(128)
    return (
        aq.astype(BF16), bq.astype(BF16), ak.astype(BF16), bk.astype(BF16),
        tri.astype(BF16), ident.astype(BF16),
    )


def _arrange_wqk(wq, wk):
    # [E, F] -> per 128-wide f-block: [128(part=e%128), NE(e//128), 128(f)]
    w = np.concatenate([wq, wk], axis=1)          # [E, NF*128]
    nf = w.shape[1] // 128
    w = w.reshape(NE, 128, nf, 128)               # [n, p, f_blk, fc]
    w = w.transpose(2, 1, 0, 3)                   # [f_blk, p, n, fc]
    return np.ascontiguousarray(w).astype(BF16)


def _arrange_xt(xt):
    # [E, T] -> [half, p, e, chunk, i]
    w = xt.reshape(NE, 128, 2, 2, CH)             # [e, p, half, cc, i]
    w = w.transpose(2, 1, 0, 3, 4)                # [half, p, e, cc, i]
    return np.ascontiguousarray(w).astype(BF16)


def _arrange_wv(wv):
    # [E, KVL*HD] -> [p, e, f]
    w = wv.reshape(NE, 128, KVL * HD)
    w = w.transpose(1, 0, 2)
    return np.ascontiguousarray(w).astype(BF16)


def _arrange_wo(wo):
    # [HL*HD, E] -> [Ehalf, p, fb*2+e2, i] so (fb, ec) slice is
    # [eh=ec//2][:, fb*2 + ec%2, :]
    w = wo.reshape(HL, 128, 2, 2, CH)             # [fb, p, eh, e2, i]
    w = w.transpose(2, 1, 0, 3, 4)                # [eh, p, fb, e2, i]
    w = w.reshape(2, 128, NE, CH)
    return np.ascontiguousarray(w).astype(BF16)


def _make_in_maps(x, W_q, W_k, W_v, W_o):
    aq, bq, ak, bk, tri, ident = _host_tables()
    xts = [_arrange_xt(np.ascontiguousarray(x[b].T)) for b in range(B)]
    in_maps = []
    for core in range(8):
        b, g = core // G, core % G
        in_maps.append({
            "xt": xts[b],
            "wqk": _arrange_wqk(W_q[:, g * HL * HD:(g + 1) * HL * HD],
                                W_k[:, g * KVL * HD:(g + 1) * KVL * HD]),
            "wv": _arrange_wv(W_v[:, g * KVL * HD:(g + 1) * KVL * HD]),
            "wo": _arrange_wo(W_o[g * HL * HD:(g + 1) * HL * HD, :] / 4096.0),
            "aq": aq, "bq": bq, "ak": ak, "bk": bk,
            "tri": tri,
            "ident": ident,
        })
    return in_maps


def _run(x, W_q, W_k, W_v, W_o, trace=False):
    nc = _get_compiled()
    in_maps = _make_in_maps(x, W_q, W_k, W_v, W_o)
    res = run_bass_kernel_spmd(nc, in_maps, list(range(8)), trace=trace)
    out = np.empty((B, T, E), np.float32)
    for b in range(B):
        out[b] = (res.results[2 * b]["out_p"].astype(np.float32)
                  + res.results[2 * b + 1]["out_p"].astype(np.float32))
    return out, res.exec_time_ns


def kernel(x, W_q, W_k, W_v, W_o):
    out, _ = _run(
        np.asarray(x), np.asarray(W_q), np.asarray(W_k),
        np.asarray(W_v), np.asarray(W_o),
    )
    return out
